# revision 14
# baseline (speedup 1.0000x reference)
"""Trainium2 Bass kernel for nn_BeyazKusAIEnhanced (moe_routing).

Model (T=2048 tokens, D=1024):
  x = emb[ids]
  h = LN1(x); attention collapses exactly to: ao = (h @ Wv) @ WoSum
    (softmax over a size-1 axis is exactly 1, so out = tile(v, 16 heads)
     and out @ Wo == v @ WoSum with WoSum[r,:] = sum_h Wo[h*64+r, :])
  x1 = x + ao
  t = LN2(x1); router probs = softmax(t @ Wr + br); top-8 -> combine [T,32]
  moe = sum_e combine[:,e] * (silu(t@We1[e]+be1[e]) @ We2[e] + be2[e])
  shared = sum_s silu(t@Ws1[s]+bs1[s]) @ Ws2[s] + bs2[s]
  out = (x1 + moe + shared) @ Wout + bout        [T, 32000]

Sharding (8 cores):
  - front part (gather/LN/attn/router) replicated on all cores
  - routed experts: 4 per core (dense compute; combine weights of
    non-selected experts are exactly 0, so dense == sparse w/ weights)
  - shared experts: inter dim (2*4096 = 8192) split 1024 per core;
    bs2 biases summed on host and added post-allreduce on every core
  - partial (moe+shared) accumulated in DRAM via accum-DMA, AllReduce'd
    across cores; x2 = x1 + reduced + bs2sum
  - output projection vocab-split: 4000 cols/core (padded to 4096)

Layout: activations feature-major [128 part, 8 kchunk, 2048 tok] in SBUF;
matmuls fp32r (full PE rate at moving free dim >= 256, ~1e-4 rel err).
LN stats via all-ones [128,128] matmul (partition-broadcast sums, no
explicit broadcast step); per-core expert selection via one-hot inputs.
Router runs in plain fp32 from x1 with LN folded (host folds g2 into Wr
and beta2@Wr into br) so top-8 selection is as close to the f32
reference as possible.
"""

import numpy as np

import concourse.bass as bass
import concourse.mybir as mybir
import concourse.tile as tile
from concourse import bacc
from concourse.bass import ts
from concourse.bass_utils import run_bass_kernel_spmd
from concourse.masks import make_identity

P = 128
B, S = 2, 1024
T = 2048          # tokens
D = 1024          # model dim
KD = D // P       # 8 k-chunks
H = 16            # heads
R = 64            # kv rank / head dim
E = 32            # routed experts
ELOC = 4          # experts per core
F = 1024          # moe inter dim
FC = F // P       # 8
NS = 2            # shared experts
ILOC = 1024       # shared inter slice per core
V = 32000
VLOC = 4000       # real vocab cols per core
VPAD = 4096       # padded to 8 x 512
NCH = VPAD // 512
TC = 4            # token chunks
TW = 512          # token chunk width
NT = T // P       # 16 token tiles
EPS = 1e-5
NCORES = 8

F32 = mybir.dt.float32
F32R = mybir.dt.float32r
I32 = mybir.dt.int32
AF = mybir.ActivationFunctionType
OP = mybir.AluOpType
AX = mybir.AxisListType

_NC_CACHE = {}


def _build_nc():
    nc = bacc.Bacc(None)

    ids_d = nc.declare_dram_parameter("ids", [T, 1], I32, isOutput=False)
    emb_d = nc.declare_dram_parameter("emb", [V, D], F32, isOutput=False)
    ones_d = nc.declare_dram_parameter("ones128", [P, P], F32R, isOutput=False)
    wv_d = nc.declare_dram_parameter("Wv", [D, R], F32, isOutput=False)
    wos_d = nc.declare_dram_parameter("WoS", [R, D], F32, isOutput=False)
    wrg_d = nc.declare_dram_parameter("Wrg", [D, E], F32, isOutput=False)
    breff_d = nc.declare_dram_parameter("breff", [E, 1], F32, isOutput=False)
    g1_d = nc.declare_dram_parameter("g1v", [D], F32, isOutput=False)
    b1_d = nc.declare_dram_parameter("b1v", [D], F32, isOutput=False)
    g2_d = nc.declare_dram_parameter("g2v", [D], F32, isOutput=False)
    b2_d = nc.declare_dram_parameter("b2v", [D], F32, isOutput=False)
    we1_d = nc.declare_dram_parameter("We1L", [ELOC, D, F], F32R, isOutput=False)
    be1_d = nc.declare_dram_parameter("be1L", [ELOC, F], F32, isOutput=False)
    we2_d = nc.declare_dram_parameter("We2L", [ELOC, F, D], F32R, isOutput=False)
    be2_d = nc.declare_dram_parameter("be2L", [ELOC, D], F32R, isOutput=False)
    ws1_d = nc.declare_dram_parameter("Ws1L", [D, ILOC], F32R, isOutput=False)
    bs1_d = nc.declare_dram_parameter("bs1L", [ILOC], F32, isOutput=False)
    ws2_d = nc.declare_dram_parameter("Ws2L", [ILOC, D], F32R, isOutput=False)
    bs2_d = nc.declare_dram_parameter("bs2S", [D], F32, isOutput=False)
    sbc_d = nc.declare_dram_parameter("Sbc", [E, ELOC * P], F32R, isOutput=False)
    ssel_d = nc.declare_dram_parameter("Ssel", [E, ELOC], F32R, isOutput=False)
    wout_d = nc.declare_dram_parameter("WoutL", [NCH, D, TW], F32R, isOutput=False)
    bout_d = nc.declare_dram_parameter("boutBC", [P, VPAD], F32, isOutput=False)
    logits_d = nc.declare_dram_parameter("logits", [T, VPAD], F32, isOutput=True)

    with tile.TileContext(nc) as tc:
        pconst = tc.alloc_tile_pool(name="pconst", bufs=1)
        pbig = tc.alloc_tile_pool(name="pbig", bufs=1)
        ppsum = tc.alloc_tile_pool(name="ppsum", bufs=6, space="PSUM")
        pstg = tc.alloc_tile_pool(name="pstg", bufs=3)
        pdram = tc.alloc_tile_pool(name="pdram", bufs=1, space="DRAM")

        def psum_tile():
            return ppsum.tile([P, TW], F32, tag="ps", name="ps", space="PSUM")

        # ---- small constants (~8.6 KB/partition) ----
        ident = pconst.tile([P, P], F32)
        make_identity(nc, ident[:])
        ones_sb = pconst.tile([P, P], F32R)
        nc.sync.dma_start(ones_sb[:], ones_d[:, :])
        wv_sb = pconst.tile([P, KD, R], F32)
        nc.sync.dma_start(wv_sb[:], wv_d.rearrange("(ko p) r -> p ko r", p=P))
        wos_sb = pconst.tile([R, KD, P], F32)
        nc.sync.dma_start(wos_sb[:], wos_d.rearrange("r (ko p) -> r ko p", p=P))
        wrg_sb = pconst.tile([P, KD, E], F32)
        nc.sync.dma_start(wrg_sb[:], wrg_d.rearrange("(ko p) e -> p ko e", p=P))
        breff_sb = pconst.tile([E, 1], F32)
        nc.sync.dma_start(breff_sb[:], breff_d[:, :])
        g1_sb = pconst.tile([P, KD], F32)
        nc.sync.dma_start(g1_sb[:], g1_d.rearrange("(ko p) -> p ko", p=P))
        b1_sb = pconst.tile([P, KD], F32)
        nc.sync.dma_start(b1_sb[:], b1_d.rearrange("(ko p) -> p ko", p=P))
        g2_sb = pconst.tile([P, KD], F32)
        nc.sync.dma_start(g2_sb[:], g2_d.rearrange("(ko p) -> p ko", p=P))
        b2_sb = pconst.tile([P, KD], F32)
        nc.sync.dma_start(b2_sb[:], b2_d.rearrange("(ko p) -> p ko", p=P))
        be1_sb = pconst.tile([P, ELOC, FC], F32)
        nc.sync.dma_start(be1_sb[:], be1_d.rearrange("e (ko p) -> p e ko", p=P))
        bs1_sb = pconst.tile([P, FC], F32)
        nc.sync.dma_start(bs1_sb[:], bs1_d.rearrange("(ko p) -> p ko", p=P))
        bs2_sb = pconst.tile([P, KD], F32)
        nc.sync.dma_start(bs2_sb[:], bs2_d.rearrange("(ko p) -> p ko", p=P))
        eps_sb = pconst.tile([P, 1], F32)
        nc.gpsimd.memset(eps_sb[:], EPS)

        # DRAM scratch
        x1_dram = pdram.tile([P, KD, T], F32, tag="x1d")
        acc_t = [pdram.tile([P, KD, TW], F32, tag=f"acc{t}", name=f"acc{t}")
                 for t in range(TC)]
        red_t = [pdram.tile([P, KD, TW], F32, tag=f"red{t}", name=f"red{t}",
                            addr_space="Shared")
                 for t in range(TC)]

        xa = pbig.tile([P, KD, T], F32, tag="A")  # x, then x1 (in place)
        hb = pbig.tile([P, KD, T], F32, tag="B")  # h (fp32, feeds attention)

        # ---- phase 1: embedding gather + PE transpose to feature-major ----
        with tc.tile_pool(name="pgather", bufs=2) as pgather:
            for i in range(NT):
                idx_sb = pgather.tile([P, 1], I32, tag="idx", name="idx")
                nc.sync.dma_start(idx_sb[:], ids_d[i * P:(i + 1) * P, :])
                gx = pgather.tile([P, D], F32, tag="gx", name="gx")
                nc.gpsimd.indirect_dma_start(
                    out=gx[:],
                    out_offset=None,
                    in_=emb_d[:, :],
                    in_offset=bass.IndirectOffsetOnAxis(ap=idx_sb[:, :1], axis=0),
                )
                for kc in range(KD):
                    tp = psum_tile()
                    nc.tensor.transpose(tp[:, :P], gx[:, ts(kc, P)], ident[:])
                    nc.vector.tensor_copy(xa[:, kc, i * P:(i + 1) * P], tp[:, :P])

        # combine-weight tiles + MoE selection constants (outlive front pools)
        pmoec = tc.alloc_tile_pool(name="pmoec", bufs=1)
        c_fm = pmoec.tile([E, T], F32R, tag="cfm")
        c_loc = pmoec.tile([ELOC, T], F32R, tag="cloc")
        sbc_sb = pmoec.tile([E, ELOC * P], F32R, tag="sbc")
        nc.sync.dma_start(sbc_sb[:], sbc_d[:, :])
        ssel_sb = pmoec.tile([E, ELOC], F32R, tag="ssel")
        nc.sync.dma_start(ssel_sb[:], ssel_d[:, :])
        be2_sb = pmoec.tile([ELOC, KD, P], F32R, tag="be2")
        nc.sync.dma_start(be2_sb[:], be2_d.rearrange("e (ko p) -> e ko p", p=P))

        # ---- phases 2-5 (LN1, attention, LN2+router fused) ----
        with (
            tc.tile_pool(name="pfC", bufs=1) as pfC,
            tc.tile_pool(name="pfM", bufs=2) as pfM,
        ):
            pfA = tc.alloc_tile_pool(name="pfA", bufs=3)
            pfB = tc.alloc_tile_pool(name="pfB", bufs=4)

            def ln_stats(src, t):
                """LN stats for token chunk t -> (mu, rstd) tiles [P, TW]
                (every partition holds the same per-token row)."""
                ps_mu = psum_tile()
                ps_sq = psum_tile()
                for kc in range(KD):
                    xr = pfA.tile([P, TW], F32R, tag="sq", name="xr")
                    nc.vector.tensor_copy(xr[:], src[:, kc, ts(t, TW)])
                    nc.tensor.matmul(
                        ps_mu[:], lhsT=ones_sb[:], rhs=xr[:],
                        start=(kc == 0), stop=(kc == KD - 1))
                    sq = pfA.tile([P, TW], F32R, tag="sq", name="sq")
                    nc.scalar.activation(sq[:], src[:, kc, ts(t, TW)], AF.Square)
                    nc.tensor.matmul(
                        ps_sq[:], lhsT=ones_sb[:], rhs=sq[:],
                        start=(kc == 0), stop=(kc == KD - 1))
                mu = pfB.tile([P, TW], F32, tag="bc", name="mu")
                nc.vector.tensor_scalar_mul(mu[:], ps_mu[:], 1.0 / D)
                msq = pfA.tile([P, TW], F32, tag="lntmp", name="msq")
                nc.vector.tensor_scalar_mul(msq[:], ps_sq[:], 1.0 / D)
                mu2 = pfA.tile([P, TW], F32, tag="lntmp", name="mu2")
                nc.vector.tensor_mul(out=mu2[:], in0=mu[:], in1=mu[:])
                nc.vector.tensor_tensor(msq[:], msq[:], mu2[:], op=OP.subtract)
                nc.scalar.activation(msq[:], msq[:], AF.Sqrt, bias=eps_sb[:, 0:1])
                rstd = pfB.tile([P, TW], F32, tag="bc", name="rstd")
                nc.vector.reciprocal(rstd[:], msq[:])
                return mu, rstd

            def ln_apply(src, dst, t, mu, rstd, g_sb, b_sb):
                for kc in range(KD):
                    nc.vector.tensor_tensor(
                        dst[:, kc, ts(t, TW)], src[:, kc, ts(t, TW)], mu[:],
                        op=OP.subtract)
                    nc.vector.tensor_tensor(
                        dst[:, kc, ts(t, TW)], dst[:, kc, ts(t, TW)], rstd[:],
                        op=OP.mult)
                    nc.vector.tensor_scalar(
                        dst[:, kc, ts(t, TW)], dst[:, kc, ts(t, TW)],
                        g_sb[:, kc:kc + 1], b_sb[:, kc:kc + 1],
                        op0=OP.mult, op1=OP.add)

            # LN1 -> h
            for t in range(TC):
                mu, rstd = ln_stats(xa, t)
                ln_apply(xa, hb, t, mu, rstd, g1_sb, b1_sb)

            # v = h @ Wv  [R, T]
            v_sb = pfC.tile([R, T], F32, tag="v")
            for t in range(TC):
                ps = psum_tile()
                for kc in range(KD):
                    nc.tensor.matmul(
                        ps[:R, :], lhsT=wv_sb[:, kc, :], rhs=hb[:, kc, ts(t, TW)],
                        start=(kc == 0), stop=(kc == KD - 1))
                nc.vector.tensor_copy(v_sb[:, ts(t, TW)], ps[:R, :])
            # x1 = x + v @ WoSum  (in place into xa)
            for dc in range(KD):
                for t in range(TC):
                    ps = psum_tile()
                    nc.tensor.matmul(
                        ps[:], lhsT=wos_sb[:, dc, :], rhs=v_sb[:, ts(t, TW)],
                        start=True, stop=True)
                    nc.vector.tensor_add(
                        out=xa[:, dc, ts(t, TW)], in0=xa[:, dc, ts(t, TW)],
                        in1=ps[:])
            nc.sync.dma_start(x1_dram[:], xa[:])

            # LN2 -> t (f32r, into slot B), fused with fp32 router matmul
            tb = pbig.tile([P, KD, T], F32R, tag="B", name="tb")
            r_fm = pfC.tile([E, T], F32, tag="v", name="r_fm")
            for t in range(TC):
                mu, rstd = ln_stats(xa, t)
                ln_apply(xa, tb, t, mu, rstd, g2_sb, b2_sb)
                ps = psum_tile()
                for kc in range(KD):
                    rt = pfA.tile([P, TW], F32, tag="rt", name="rt")
                    nc.vector.tensor_tensor(
                        rt[:], xa[:, kc, ts(t, TW)], mu[:],
                        op=OP.subtract)
                    nc.tensor.matmul(
                        ps[:E, :], lhsT=wrg_sb[:, kc, :], rhs=rt[:],
                        start=(kc == 0), stop=(kc == KD - 1))
                nc.vector.tensor_tensor(
                    r_fm[:, ts(t, TW)], ps[:E, :], rstd[:E, :], op=OP.mult)
                nc.vector.tensor_scalar_add(
                    r_fm[:, ts(t, TW)], r_fm[:, ts(t, TW)], breff_sb[:E, 0:1])

            pfB.release()
            pfA.release()

            # softmax + top-8 in token-major
            r_tm = pfC.tile([P, NT, E], F32, tag="rtm")
            for i in range(NT):
                tp = psum_tile()
                nc.tensor.transpose(
                    tp[:, :E], r_fm[:, i * P:(i + 1) * P], ident[:E, :E])
                nc.vector.tensor_copy(r_tm[:, i, :], tp[:, :E])
            m_sb = pfM.tile([P, NT], F32, tag="m", name="m1")
            nc.vector.reduce_max(m_sb[:, :, None], r_tm[:], axis=AX.X)
            nc.vector.tensor_tensor(
                r_tm[:], r_tm[:], m_sb[:, :, None].to_broadcast([P, NT, E]),
                op=OP.subtract)
            nc.scalar.activation(r_tm[:], r_tm[:], AF.Exp)
            s_sb = pfM.tile([P, NT], F32, tag="m", name="m2")
            nc.vector.reduce_sum(s_sb[:, :, None], r_tm[:], axis=AX.X)
            rs_sb = pfM.tile([P, NT], F32, tag="m", name="m3")
            nc.vector.reciprocal(rs_sb[:], s_sb[:])
            nc.vector.tensor_tensor(
                r_tm[:], r_tm[:], rs_sb[:, :, None].to_broadcast([P, NT, E]),
                op=OP.mult)
            work = pfC.tile([P, NT, E], F32, tag="work")
            msk = pfC.tile([P, NT, E], F32, tag="msk")
            nc.vector.tensor_copy(work[:], r_tm[:])
            thr = pfM.tile([P, NT], F32, tag="m", name="m4")
            for it in range(8):
                nc.vector.reduce_max(thr[:, :, None], work[:], axis=AX.X)
                if it < 7:
                    nc.vector.tensor_tensor(
                        msk[:], work[:], thr[:, :, None].to_broadcast([P, NT, E]),
                        op=OP.is_lt)
                    nc.vector.tensor_tensor(work[:], work[:], msk[:], op=OP.mult)
            nc.vector.tensor_tensor(
                msk[:], r_tm[:], thr[:, :, None].to_broadcast([P, NT, E]),
                op=OP.is_ge)
            nc.vector.tensor_tensor(work[:], r_tm[:], msk[:], op=OP.mult)
            wsum = pfM.tile([P, NT], F32, tag="m", name="m5")
            nc.vector.reduce_sum(wsum[:, :, None], work[:], axis=AX.X)
            rws = pfM.tile([P, NT], F32, tag="m", name="m6")
            nc.vector.reciprocal(rws[:], wsum[:])
            nc.vector.tensor_tensor(
                work[:], work[:], rws[:, :, None].to_broadcast([P, NT, E]),
                op=OP.mult)

            # combine back to expert-major [E, T] + local selection
            for i in range(NT):
                tp = psum_tile()
                nc.tensor.transpose(tp[:E, :P], work[:, i, :], ident[:])
                nc.vector.tensor_copy(c_fm[:, i * P:(i + 1) * P], tp[:E, :P])
            for t in range(TC):
                ps = psum_tile()
                nc.tensor.matmul(
                    ps[:ELOC, :], lhsT=ssel_sb[:], rhs=c_fm[:, ts(t, TW)],
                    start=True, stop=True)
                nc.vector.tensor_copy(c_loc[:, ts(t, TW)], ps[:ELOC, :])

        # ---- phase 6: MoE (4 routed dense + shared slice) ----
        with (
            tc.tile_pool(name="pw", bufs=8) as pw,
            tc.tile_pool(name="pcbc", bufs=1) as pcbc,
        ):
            za = pbig.tile([P, FC, T], F32R, tag="A")  # z (reuses x slot)
            for e in range(ELOC + 1):
                shared = e == ELOC
                if not shared:
                    cbc = pcbc.tile([P, T], F32, tag="cbc", name="cbc")
                    for t in range(TC):
                        ps = psum_tile()
                        nc.tensor.matmul(
                            ps[:], lhsT=sbc_sb[:, ts(e, P)],
                            rhs=c_fm[:, ts(t, TW)], start=True, stop=True)
                        nc.vector.tensor_copy(cbc[:, ts(t, TW)], ps[:])
                w1t = []
                for kc in range(KD):
                    wt = pw.tile([P, F], F32R, tag="w", name="w1t")
                    src = (ws1_d[ts(kc, P), :] if shared
                           else we1_d[e, ts(kc, P), :])
                    nc.sync.dma_start(wt[:], src)
                    w1t.append(wt)
                for fc in range(FC):
                    bias = (bs1_sb[:, fc:fc + 1] if shared
                            else be1_sb[:, e, fc:fc + 1])
                    for t in range(TC):
                        ps = psum_tile()
                        for kc in range(KD):
                            nc.tensor.matmul(
                                ps[:], lhsT=w1t[kc][:, ts(fc, P)],
                                rhs=tb[:, kc, ts(t, TW)],
                                start=(kc == 0), stop=(kc == KD - 1))
                        nc.scalar.activation(
                            za[:, fc, ts(t, TW)], ps[:], AF.Silu, bias=bias)
                        if not shared:
                            nc.vector.tensor_tensor(
                                za[:, fc, ts(t, TW)], za[:, fc, ts(t, TW)],
                                cbc[:, ts(t, TW)], op=OP.mult)
                w2t = []
                for fc in range(FC):
                    wt = pw.tile([P, D], F32R, tag="w", name="w2t")
                    src = (ws2_d[ts(fc, P), :] if shared
                           else we2_d[e, ts(fc, P), :])
                    nc.sync.dma_start(wt[:], src)
                    w2t.append(wt)
                for dc in range(KD):
                    for t in range(TC):
                        ps = psum_tile()
                        for fc in range(FC):
                            nc.tensor.matmul(
                                ps[:], lhsT=w2t[fc][:, ts(dc, P)],
                                rhs=za[:, fc, ts(t, TW)],
                                start=(fc == 0),
                                stop=(fc == FC - 1 and not shared))
                        if shared:
                            nc.tensor.matmul(
                                ps[:], lhsT=be2_sb[:, dc, :],
                                rhs=c_loc[:, ts(t, TW)],
                                start=False, stop=True)
                        stg = pstg.tile([P, TW], F32, tag="stg", name="stg")
                        nc.vector.tensor_copy(stg[:], ps[:])
                        nc.gpsimd.dma_start(
                            acc_t[t][:, dc, :], stg[:],
                            accum_op=(OP.bypass if e == 0 else OP.add))
        pmoec.release()

        # ---- phase 7: AllReduce; x2 = x1 + red + bs2sum; out projection ----
        with (
            tc.tile_pool(name="pxb", bufs=3) as pxb,
            tc.tile_pool(name="pwout", bufs=12) as pwout,
        ):
            x2 = pbig.tile([P, KD, T], F32R, tag="B")
            for t in range(TC):
                nc.gpsimd.collective_compute(
                    "AllReduce",
                    OP.add,
                    replica_groups=[list(range(NCORES))],
                    ins=[acc_t[t][:].opt()],
                    outs=[red_t[t][:].opt()],
                )
                for kc in range(KD):
                    xb = pxb.tile([P, TW], F32, tag="xb", name="xb")
                    nc.sync.dma_start(xb[:], x1_dram[:, kc, ts(t, TW)])
                    rb = pxb.tile([P, TW], F32, tag="rb", name="rb")
                    nc.sync.dma_start(rb[:], red_t[t][:, kc, :])
                    nc.vector.tensor_add(out=xb[:], in0=xb[:], in1=rb[:])
                    nc.vector.tensor_scalar_add(
                        x2[:, kc, ts(t, TW)], xb[:], bs2_sb[:, kc:kc + 1])
            for n in range(NCH):
                bout_sb = pxb.tile([P, TW], F32, tag="bout", name="bout")
                nc.sync.dma_start(bout_sb[:], bout_d[:, ts(n, TW)])
                wot = []
                for kc in range(KD):
                    wt = pwout.tile([P, TW], F32R, tag="wo", name="wo")
                    nc.sync.dma_start(wt[:], wout_d[n, ts(kc, P), :])
                    wot.append(wt)
                for m in range(NT):
                    ps = psum_tile()
                    for kc in range(KD):
                        nc.tensor.matmul(
                            ps[:], lhsT=x2[:, kc, ts(m, P)], rhs=wot[kc][:],
                            start=(kc == 0), stop=(kc == KD - 1))
                    stg = pstg.tile([P, TW], F32, tag="stg", name="stg")
                    nc.vector.tensor_add(out=stg[:], in0=ps[:], in1=bout_sb[:])
                    nc.sync.dma_start(logits_d[ts(m, P), ts(n, TW)], stg[:])

        for _pool in (pdram, pstg, ppsum, pbig, pconst):
            _pool.release()

    nc.compile()
    return nc


def _get_nc():
    if "nc" not in _NC_CACHE:
        _NC_CACHE["nc"] = _build_nc()
    return _NC_CACHE["nc"]


def _prep_in_maps(inputs):
    inp = {k: np.asarray(v) for k, v in inputs.items()}
    f32 = np.float32

    ids = np.ascontiguousarray(inp["input_ids"].reshape(T, 1).astype(np.int32))
    emb = np.ascontiguousarray(inp["emb"].astype(f32))
    WoS = np.ascontiguousarray(
        inp["Wo"].astype(f32).reshape(H, R, D).sum(0).astype(f32))
    g2 = inp["g2"].astype(f32)
    Wrg = np.ascontiguousarray((g2[:, None] * inp["Wr"].astype(f32)).astype(f32))
    breff = (inp["br"].astype(f32)
             + inp["beta2"].astype(f32) @ inp["Wr"].astype(f32))
    breff = np.ascontiguousarray(breff.reshape(E, 1).astype(f32))

    common = {
        "ids": ids, "emb": emb,
        "ones128": np.ones((P, P), f32),
        "Wv": np.ascontiguousarray(inp["Wv"].astype(f32)),
        "WoS": WoS, "Wrg": Wrg, "breff": breff,
        "g1v": inp["g1"].astype(f32), "b1v": inp["beta1"].astype(f32),
        "g2v": g2, "b2v": inp["beta2"].astype(f32),
        "bs2S": np.ascontiguousarray(inp["bs2"].astype(f32).sum(0)),
    }

    We1 = inp["We1"].astype(f32)
    be1 = inp["be1"].astype(f32)
    We2 = inp["We2"].astype(f32)
    be2 = inp["be2"].astype(f32)
    Ws1 = inp["Ws1"].astype(f32)
    bs1 = inp["bs1"].astype(f32)
    Ws2 = inp["Ws2"].astype(f32)
    Wout = inp["Wout"].astype(f32)
    bout = inp["bout"].astype(f32)

    in_maps = []
    for c in range(NCORES):
        el = list(range(ELOC * c, ELOC * (c + 1)))
        s, q = divmod(c, NCORES // NS)
        isl = slice(q * ILOC, (q + 1) * ILOC)
        Sbc = np.zeros((E, ELOC * P), f32)
        Ssel = np.zeros((E, ELOC), f32)
        for j, e in enumerate(el):
            Sbc[e, j * P:(j + 1) * P] = 1.0
            Ssel[e, j] = 1.0
        wout_pad = np.zeros((D, VPAD), f32)
        wout_pad[:, :VLOC] = Wout[:, VLOC * c:VLOC * (c + 1)]
        woutL = np.ascontiguousarray(
            wout_pad.reshape(D, NCH, TW).transpose(1, 0, 2))
        bout_pad = np.zeros((VPAD,), f32)
        bout_pad[:VLOC] = bout[VLOC * c:VLOC * (c + 1)]
        boutBC = np.ascontiguousarray(np.broadcast_to(bout_pad, (P, VPAD)))
        m = dict(common)
        m.update({
            "We1L": np.ascontiguousarray(We1[el]),
            "be1L": np.ascontiguousarray(be1[el]),
            "We2L": np.ascontiguousarray(We2[el]),
            "be2L": np.ascontiguousarray(be2[el]),
            "Ws1L": np.ascontiguousarray(Ws1[s][:, isl]),
            "bs1L": np.ascontiguousarray(bs1[s][isl]),
            "Ws2L": np.ascontiguousarray(Ws2[s][isl, :]),
            "Sbc": Sbc, "Ssel": Ssel,
            "WoutL": woutL, "boutBC": boutBC,
        })
        in_maps.append(m)
    return in_maps


def kernel(**inputs):
    in_maps = _prep_in_maps(inputs)
    nc = _get_nc()
    r = run_bass_kernel_spmd(nc, in_maps, list(range(NCORES)))
    logits = np.concatenate(
        [r.results[c]["logits"][:, :VLOC] for c in range(NCORES)], axis=1)
    return np.ascontiguousarray(logits.reshape(B, S, V).astype(np.float32))


if __name__ == "__main__":
    _build_nc()
    print("build + compile OK")


# revision 15
# speedup vs baseline: 1.0318x; 1.0318x over previous
"""Trainium2 Bass kernel for nn_BeyazKusAIEnhanced (moe_routing).

Model (T=2048 tokens, D=1024):
  x = emb[ids]
  h = LN1(x); attention collapses exactly to: ao = (h @ Wv) @ WoSum
    (softmax over a size-1 axis is exactly 1, so out = tile(v, 16 heads)
     and out @ Wo == v @ WoSum with WoSum[r,:] = sum_h Wo[h*64+r, :])
  x1 = x + ao
  t = LN2(x1); router probs = softmax(t @ Wr + br); top-8 -> combine [T,32]
  moe = sum_e combine[:,e] * (silu(t@We1[e]+be1[e]) @ We2[e] + be2[e])
  shared = sum_s silu(t@Ws1[s]+bs1[s]) @ Ws2[s] + bs2[s]
  out = (x1 + moe + shared) @ Wout + bout        [T, 32000]

Sharding (8 cores):
  - front part (gather/LN/attn/router) replicated on all cores
  - routed experts: 4 per core (dense compute; combine weights of
    non-selected experts are exactly 0, so dense == sparse w/ weights)
  - shared experts: inter dim (2*4096 = 8192) split 1024 per core;
    bs2 biases summed on host and added post-allreduce on every core
  - partial (moe+shared) accumulated in DRAM via accum-DMA, AllReduce'd
    across cores; x2 = x1 + reduced + bs2sum
  - output projection vocab-split: 4000 cols/core (padded to 4096)

Layout: activations feature-major [128 part, 8 kchunk, 2048 tok] in SBUF;
matmuls fp32r (full PE rate at moving free dim >= 256, ~1e-4 rel err).
LN stats via all-ones [128,128] matmul (partition-broadcast sums, no
explicit broadcast step); per-core expert selection via one-hot inputs.
Router runs in plain fp32 from x1 with LN folded (host folds g2 into Wr
and beta2@Wr into br) so top-8 selection is as close to the f32
reference as possible.
"""

import numpy as np

import concourse.bass as bass
import concourse.mybir as mybir
import concourse.tile as tile
from concourse import bacc
from concourse.bass import ts
from concourse.bass_utils import run_bass_kernel_spmd
from concourse.masks import make_identity

P = 128
B, S = 2, 1024
T = 2048          # tokens
D = 1024          # model dim
KD = D // P       # 8 k-chunks
H = 16            # heads
R = 64            # kv rank / head dim
E = 32            # routed experts
ELOC = 4          # experts per core
F = 1024          # moe inter dim
FC = F // P       # 8
NS = 2            # shared experts
ILOC = 1024       # shared inter slice per core
V = 32000
VLOC = 4000       # real vocab cols per core
VPAD = 4096       # padded to 8 x 512
NCH = VPAD // 512
TC = 4            # token chunks
TW = 512          # token chunk width
NT = T // P       # 16 token tiles
EPS = 1e-5
NCORES = 8

F32 = mybir.dt.float32
F32R = mybir.dt.float32r
I32 = mybir.dt.int32
AF = mybir.ActivationFunctionType
OP = mybir.AluOpType
AX = mybir.AxisListType

_NC_CACHE = {}


def _build_nc():
    nc = bacc.Bacc(None)

    ids_d = nc.declare_dram_parameter("ids", [T, 1], I32, isOutput=False)
    emb_d = nc.declare_dram_parameter("emb", [V, D], F32, isOutput=False)
    ones_d = nc.declare_dram_parameter("ones128", [P, P], F32R, isOutput=False)
    wv_d = nc.declare_dram_parameter("Wv", [D, R], F32, isOutput=False)
    wos_d = nc.declare_dram_parameter("WoS", [R, D], F32, isOutput=False)
    wrg_d = nc.declare_dram_parameter("Wrg", [D, E], F32, isOutput=False)
    breff_d = nc.declare_dram_parameter("breff", [E, 1], F32, isOutput=False)
    g1_d = nc.declare_dram_parameter("g1v", [D], F32, isOutput=False)
    b1_d = nc.declare_dram_parameter("b1v", [D], F32, isOutput=False)
    g2_d = nc.declare_dram_parameter("g2v", [D], F32, isOutput=False)
    b2_d = nc.declare_dram_parameter("b2v", [D], F32, isOutput=False)
    we1_d = nc.declare_dram_parameter("We1L", [ELOC, D, F], F32R, isOutput=False)
    be1_d = nc.declare_dram_parameter("be1L", [ELOC, F], F32, isOutput=False)
    we2_d = nc.declare_dram_parameter("We2L", [ELOC, F, D], F32R, isOutput=False)
    be2_d = nc.declare_dram_parameter("be2L", [ELOC, D], F32R, isOutput=False)
    ws1_d = nc.declare_dram_parameter("Ws1L", [D, ILOC], F32R, isOutput=False)
    bs1_d = nc.declare_dram_parameter("bs1L", [ILOC], F32, isOutput=False)
    ws2_d = nc.declare_dram_parameter("Ws2L", [ILOC, D], F32R, isOutput=False)
    bs2_d = nc.declare_dram_parameter("bs2S", [D], F32, isOutput=False)
    sbc_d = nc.declare_dram_parameter("Sbc", [E, ELOC * P], F32R, isOutput=False)
    ssel_d = nc.declare_dram_parameter("Ssel", [E, ELOC], F32R, isOutput=False)
    wout_d = nc.declare_dram_parameter("WoutL", [NCH, D, TW], F32R, isOutput=False)
    bout_d = nc.declare_dram_parameter("boutBC", [P, VPAD], F32, isOutput=False)
    logits_d = nc.declare_dram_parameter("logits", [T, VPAD], F32, isOutput=True)

    with tile.TileContext(nc) as tc:
        pconst = tc.alloc_tile_pool(name="pconst", bufs=1)
        pbig = tc.alloc_tile_pool(name="pbig", bufs=1)
        ppsum = tc.alloc_tile_pool(name="ppsum", bufs=6, space="PSUM")
        pstg = tc.alloc_tile_pool(name="pstg", bufs=3)
        pdram = tc.alloc_tile_pool(name="pdram", bufs=1, space="DRAM")

        def psum_tile():
            return ppsum.tile([P, TW], F32, tag="ps", name="ps", space="PSUM")

        # ---- small constants (~8.6 KB/partition) ----
        ident = pconst.tile([P, P], F32)
        make_identity(nc, ident[:])
        ones_sb = pconst.tile([P, P], F32R)
        nc.sync.dma_start(ones_sb[:], ones_d[:, :])
        wv_sb = pconst.tile([P, KD, R], F32)
        nc.sync.dma_start(wv_sb[:], wv_d.rearrange("(ko p) r -> p ko r", p=P))
        wos_sb = pconst.tile([R, KD, P], F32)
        nc.sync.dma_start(wos_sb[:], wos_d.rearrange("r (ko p) -> r ko p", p=P))
        wrg_sb = pconst.tile([P, KD, E], F32)
        nc.sync.dma_start(wrg_sb[:], wrg_d.rearrange("(ko p) e -> p ko e", p=P))
        breff_sb = pconst.tile([E, 1], F32)
        nc.sync.dma_start(breff_sb[:], breff_d[:, :])
        g1_sb = pconst.tile([P, KD], F32)
        nc.sync.dma_start(g1_sb[:], g1_d.rearrange("(ko p) -> p ko", p=P))
        b1_sb = pconst.tile([P, KD], F32)
        nc.sync.dma_start(b1_sb[:], b1_d.rearrange("(ko p) -> p ko", p=P))
        g2_sb = pconst.tile([P, KD], F32)
        nc.sync.dma_start(g2_sb[:], g2_d.rearrange("(ko p) -> p ko", p=P))
        b2_sb = pconst.tile([P, KD], F32)
        nc.sync.dma_start(b2_sb[:], b2_d.rearrange("(ko p) -> p ko", p=P))
        be1_sb = pconst.tile([P, ELOC, FC], F32)
        nc.sync.dma_start(be1_sb[:], be1_d.rearrange("e (ko p) -> p e ko", p=P))
        bs1_sb = pconst.tile([P, FC], F32)
        nc.sync.dma_start(bs1_sb[:], bs1_d.rearrange("(ko p) -> p ko", p=P))
        bs2_sb = pconst.tile([P, KD], F32)
        nc.sync.dma_start(bs2_sb[:], bs2_d.rearrange("(ko p) -> p ko", p=P))
        eps_sb = pconst.tile([P, 1], F32)
        nc.gpsimd.memset(eps_sb[:], EPS)

        # DRAM scratch
        x1_dram = pdram.tile([P, KD, T], F32, tag="x1d")
        acc_t = [pdram.tile([P, KD, TW], F32, tag=f"acc{t}", name=f"acc{t}")
                 for t in range(TC)]
        red_t = [pdram.tile([P, KD, TW], F32, tag=f"red{t}", name=f"red{t}",
                            addr_space="Shared")
                 for t in range(TC)]

        xa = pbig.tile([P, KD, T], F32, tag="A")  # x, then x1 (in place)
        hb = pbig.tile([P, KD, T], F32, tag="B")  # h (fp32, feeds attention)

        # ---- phase 1: embedding gather + PE transpose to feature-major ----
        with (
            tc.tile_pool(name="pgather", bufs=4) as pgather,
            tc.tile_pool(name="pidx", bufs=NT) as pidx,
        ):
            idxs = []
            for i in range(NT):
                idx_sb = pidx.tile([P, 1], I32, tag="idx", name="idx")
                nc.sync.dma_start(idx_sb[:], ids_d[i * P:(i + 1) * P, :])
                idxs.append(idx_sb)
            for i in range(NT):
                idx_sb = idxs[i]
                gx = pgather.tile([P, D], F32, tag="gx", name="gx")
                nc.gpsimd.indirect_dma_start(
                    out=gx[:],
                    out_offset=None,
                    in_=emb_d[:, :],
                    in_offset=bass.IndirectOffsetOnAxis(ap=idx_sb[:, :1], axis=0),
                )
                for kc in range(KD):
                    tp = psum_tile()
                    nc.tensor.transpose(tp[:, :P], gx[:, ts(kc, P)], ident[:])
                    nc.vector.tensor_copy(xa[:, kc, i * P:(i + 1) * P], tp[:, :P])

        # combine-weight tiles + MoE selection constants (outlive front pools)
        pmoec = tc.alloc_tile_pool(name="pmoec", bufs=1)
        c_fm = pmoec.tile([E, T], F32R, tag="cfm")
        c_loc = pmoec.tile([ELOC, T], F32R, tag="cloc")
        sbc_sb = pmoec.tile([E, ELOC * P], F32R, tag="sbc")
        nc.sync.dma_start(sbc_sb[:], sbc_d[:, :])
        ssel_sb = pmoec.tile([E, ELOC], F32R, tag="ssel")
        nc.sync.dma_start(ssel_sb[:], ssel_d[:, :])
        be2_sb = pmoec.tile([ELOC, KD, P], F32R, tag="be2")
        nc.sync.dma_start(be2_sb[:], be2_d.rearrange("e (ko p) -> e ko p", p=P))

        # ---- phases 2-5 (LN1, attention, LN2+router fused) ----
        with (
            tc.tile_pool(name="pfC", bufs=1) as pfC,
            tc.tile_pool(name="pfM", bufs=2) as pfM,
        ):
            pfA = tc.alloc_tile_pool(name="pfA", bufs=2)
            pfB = tc.alloc_tile_pool(name="pfB", bufs=2 * TC)

            def ln_stats(src, t):
                """LN stats for token chunk t -> (mu, rstd) tiles [P, TW]
                (every partition holds the same per-token row)."""
                ps_mu = psum_tile()
                ps_sq = psum_tile()
                for kc in range(KD):
                    xr = pfA.tile([P, TW], F32R, tag="sq", name="xr")
                    nc.vector.tensor_copy(xr[:], src[:, kc, ts(t, TW)])
                    nc.tensor.matmul(
                        ps_mu[:], lhsT=ones_sb[:], rhs=xr[:],
                        start=(kc == 0), stop=(kc == KD - 1))
                    sq = pfA.tile([P, TW], F32R, tag="sq", name="sq")
                    nc.scalar.activation(sq[:], src[:, kc, ts(t, TW)], AF.Square)
                    nc.tensor.matmul(
                        ps_sq[:], lhsT=ones_sb[:], rhs=sq[:],
                        start=(kc == 0), stop=(kc == KD - 1))
                mu = pfB.tile([P, TW], F32, tag="bc", name="mu")
                nc.vector.tensor_scalar_mul(mu[:], ps_mu[:], 1.0 / D)
                msq = pfA.tile([P, TW], F32, tag="lntmp", name="msq")
                nc.vector.tensor_scalar_mul(msq[:], ps_sq[:], 1.0 / D)
                mu2 = pfA.tile([P, TW], F32, tag="lntmp", name="mu2")
                nc.vector.tensor_mul(out=mu2[:], in0=mu[:], in1=mu[:])
                nc.vector.tensor_tensor(msq[:], msq[:], mu2[:], op=OP.subtract)
                nc.scalar.activation(msq[:], msq[:], AF.Sqrt, bias=eps_sb[:, 0:1])
                rstd = pfB.tile([P, TW], F32, tag="bc", name="rstd")
                nc.vector.reciprocal(rstd[:], msq[:])
                return mu, rstd

            def ln_apply(src, dst, t, mu, rstd, g_sb, b_sb):
                for kc in range(KD):
                    nc.vector.tensor_tensor(
                        dst[:, kc, ts(t, TW)], src[:, kc, ts(t, TW)], mu[:],
                        op=OP.subtract)
                    nc.vector.tensor_tensor(
                        dst[:, kc, ts(t, TW)], dst[:, kc, ts(t, TW)], rstd[:],
                        op=OP.mult)
                    nc.vector.tensor_scalar(
                        dst[:, kc, ts(t, TW)], dst[:, kc, ts(t, TW)],
                        g_sb[:, kc:kc + 1], b_sb[:, kc:kc + 1],
                        op0=OP.mult, op1=OP.add)

            # LN1 -> h (stats for all chunks first, applies after: keeps
            # the PE stats matmuls from waiting behind DVE apply chains)
            st1 = [ln_stats(xa, t) for t in range(TC)]
            for t in range(TC):
                ln_apply(xa, hb, t, st1[t][0], st1[t][1], g1_sb, b1_sb)

            # v = h @ Wv  [R, T]
            v_sb = pfC.tile([R, T], F32, tag="v")
            for t in range(TC):
                ps = psum_tile()
                for kc in range(KD):
                    nc.tensor.matmul(
                        ps[:R, :], lhsT=wv_sb[:, kc, :], rhs=hb[:, kc, ts(t, TW)],
                        start=(kc == 0), stop=(kc == KD - 1))
                nc.vector.tensor_copy(v_sb[:, ts(t, TW)], ps[:R, :])
            # x1 = x + v @ WoSum  (in place into xa)
            for dc in range(KD):
                for t in range(TC):
                    ps = psum_tile()
                    nc.tensor.matmul(
                        ps[:], lhsT=wos_sb[:, dc, :], rhs=v_sb[:, ts(t, TW)],
                        start=True, stop=True)
                    nc.vector.tensor_add(
                        out=xa[:, dc, ts(t, TW)], in0=xa[:, dc, ts(t, TW)],
                        in1=ps[:])
            nc.sync.dma_start(x1_dram[:], xa[:])

            # LN2 -> t (f32r, into slot B), fused with fp32 router matmul
            tb = pbig.tile([P, KD, T], F32R, tag="B", name="tb")
            r_fm = pfC.tile([E, T], F32, tag="v", name="r_fm")
            st2 = [ln_stats(xa, t) for t in range(TC)]
            for t in range(TC):
                mu, rstd = st2[t]
                ln_apply(xa, tb, t, mu, rstd, g2_sb, b2_sb)
                ps = psum_tile()
                for kc in range(KD):
                    rt = pfA.tile([P, TW], F32, tag="rt", name="rt")
                    nc.vector.tensor_tensor(
                        rt[:], xa[:, kc, ts(t, TW)], mu[:],
                        op=OP.subtract)
                    nc.tensor.matmul(
                        ps[:E, :], lhsT=wrg_sb[:, kc, :], rhs=rt[:],
                        start=(kc == 0), stop=(kc == KD - 1))
                nc.vector.tensor_tensor(
                    r_fm[:, ts(t, TW)], ps[:E, :], rstd[:E, :], op=OP.mult)
                nc.vector.tensor_scalar_add(
                    r_fm[:, ts(t, TW)], r_fm[:, ts(t, TW)], breff_sb[:E, 0:1])

            pfB.release()
            pfA.release()

            # softmax + top-8 in token-major
            r_tm = pfC.tile([P, NT, E], F32, tag="rtm")
            for i in range(NT):
                tp = psum_tile()
                nc.tensor.transpose(
                    tp[:, :E], r_fm[:, i * P:(i + 1) * P], ident[:E, :E])
                nc.vector.tensor_copy(r_tm[:, i, :], tp[:, :E])
            m_sb = pfM.tile([P, NT], F32, tag="m", name="m1")
            nc.vector.reduce_max(m_sb[:, :, None], r_tm[:], axis=AX.X)
            nc.vector.tensor_tensor(
                r_tm[:], r_tm[:], m_sb[:, :, None].to_broadcast([P, NT, E]),
                op=OP.subtract)
            nc.scalar.activation(r_tm[:], r_tm[:], AF.Exp)
            s_sb = pfM.tile([P, NT], F32, tag="m", name="m2")
            nc.vector.reduce_sum(s_sb[:, :, None], r_tm[:], axis=AX.X)
            rs_sb = pfM.tile([P, NT], F32, tag="m", name="m3")
            nc.vector.reciprocal(rs_sb[:], s_sb[:])
            nc.vector.tensor_tensor(
                r_tm[:], r_tm[:], rs_sb[:, :, None].to_broadcast([P, NT, E]),
                op=OP.mult)
            work = pfC.tile([P, NT, E], F32, tag="work")
            msk = pfC.tile([P, NT, E], F32, tag="msk")
            nc.vector.tensor_copy(work[:], r_tm[:])
            thr = pfM.tile([P, NT], F32, tag="m", name="m4")
            for it in range(8):
                nc.vector.reduce_max(thr[:, :, None], work[:], axis=AX.X)
                if it < 7:
                    nc.vector.tensor_tensor(
                        msk[:], work[:], thr[:, :, None].to_broadcast([P, NT, E]),
                        op=OP.is_lt)
                    nc.vector.tensor_tensor(work[:], work[:], msk[:], op=OP.mult)
            nc.vector.tensor_tensor(
                msk[:], r_tm[:], thr[:, :, None].to_broadcast([P, NT, E]),
                op=OP.is_ge)
            nc.vector.tensor_tensor(work[:], r_tm[:], msk[:], op=OP.mult)
            wsum = pfM.tile([P, NT], F32, tag="m", name="m5")
            nc.vector.reduce_sum(wsum[:, :, None], work[:], axis=AX.X)
            rws = pfM.tile([P, NT], F32, tag="m", name="m6")
            nc.vector.reciprocal(rws[:], wsum[:])
            nc.vector.tensor_tensor(
                work[:], work[:], rws[:, :, None].to_broadcast([P, NT, E]),
                op=OP.mult)

            # combine back to expert-major [E, T] + local selection
            for i in range(NT):
                tp = psum_tile()
                nc.tensor.transpose(tp[:E, :P], work[:, i, :], ident[:])
                nc.vector.tensor_copy(c_fm[:, i * P:(i + 1) * P], tp[:E, :P])
            for t in range(TC):
                ps = psum_tile()
                nc.tensor.matmul(
                    ps[:ELOC, :], lhsT=ssel_sb[:], rhs=c_fm[:, ts(t, TW)],
                    start=True, stop=True)
                nc.vector.tensor_copy(c_loc[:, ts(t, TW)], ps[:ELOC, :])

        # ---- phase 6: MoE (4 routed dense + shared slice) ----
        with (
            tc.tile_pool(name="pw", bufs=8) as pw,
            tc.tile_pool(name="pcbc", bufs=1) as pcbc,
        ):
            za = pbig.tile([P, FC, T], F32R, tag="A")  # z (reuses x slot)
            for e in range(ELOC + 1):
                shared = e == ELOC
                if not shared:
                    cbc = pcbc.tile([P, T], F32, tag="cbc", name="cbc")
                    for t in range(TC):
                        ps = psum_tile()
                        nc.tensor.matmul(
                            ps[:], lhsT=sbc_sb[:, ts(e, P)],
                            rhs=c_fm[:, ts(t, TW)], start=True, stop=True)
                        nc.vector.tensor_copy(cbc[:, ts(t, TW)], ps[:])
                w1t = []
                for kc in range(KD):
                    wt = pw.tile([P, F], F32R, tag="w", name="w1t")
                    src = (ws1_d[ts(kc, P), :] if shared
                           else we1_d[e, ts(kc, P), :])
                    nc.sync.dma_start(wt[:], src)
                    w1t.append(wt)
                for fc in range(FC):
                    bias = (bs1_sb[:, fc:fc + 1] if shared
                            else be1_sb[:, e, fc:fc + 1])
                    for t in range(TC):
                        ps = psum_tile()
                        for kc in range(KD):
                            nc.tensor.matmul(
                                ps[:], lhsT=w1t[kc][:, ts(fc, P)],
                                rhs=tb[:, kc, ts(t, TW)],
                                start=(kc == 0), stop=(kc == KD - 1))
                        nc.scalar.activation(
                            za[:, fc, ts(t, TW)], ps[:], AF.Silu, bias=bias)
                        if not shared:
                            nc.vector.tensor_tensor(
                                za[:, fc, ts(t, TW)], za[:, fc, ts(t, TW)],
                                cbc[:, ts(t, TW)], op=OP.mult)
                w2t = []
                for fc in range(FC):
                    wt = pw.tile([P, D], F32R, tag="w", name="w2t")
                    src = (ws2_d[ts(fc, P), :] if shared
                           else we2_d[e, ts(fc, P), :])
                    nc.sync.dma_start(wt[:], src)
                    w2t.append(wt)
                loop = ([(t, dc) for t in range(TC) for dc in range(KD)]
                        if shared else
                        [(t, dc) for dc in range(KD) for t in range(TC)])
                for t, dc in loop:
                    ps = psum_tile()
                    for fc in range(FC):
                        nc.tensor.matmul(
                            ps[:], lhsT=w2t[fc][:, ts(dc, P)],
                            rhs=za[:, fc, ts(t, TW)],
                            start=(fc == 0),
                            stop=(fc == FC - 1 and not shared))
                    if shared:
                        nc.tensor.matmul(
                            ps[:], lhsT=be2_sb[:, dc, :],
                            rhs=c_loc[:, ts(t, TW)],
                            start=False, stop=True)
                    stg = pstg.tile([P, TW], F32, tag="stg", name="stg")
                    nc.scalar.activation(stg[:], ps[:], AF.Copy)
                    nc.gpsimd.dma_start(
                        acc_t[t][:, dc, :], stg[:],
                        accum_op=(OP.bypass if e == 0 else OP.add))
                    if shared and dc == KD - 1:
                        # chunk t of the partial sum is complete on this
                        # core: all-reduce it while the rest of the MoE
                        # tail and the output projection proceed
                        nc.gpsimd.collective_compute(
                            "AllReduce",
                            OP.add,
                            replica_groups=[list(range(NCORES))],
                            ins=[acc_t[t][:].opt()],
                            outs=[red_t[t][:].opt()],
                        )
        pmoec.release()

        # ---- phase 7: AllReduce; x2 = x1 + red + bs2sum; out projection ----
        with (
            tc.tile_pool(name="pxb", bufs=3) as pxb,
            tc.tile_pool(name="pwout", bufs=12) as pwout,
        ):
            x2 = pbig.tile([P, KD, T], F32R, tag="B")
            for t in range(TC):
                for kc in range(KD):
                    xb = pxb.tile([P, TW], F32, tag="xb", name="xb")
                    nc.sync.dma_start(xb[:], x1_dram[:, kc, ts(t, TW)])
                    rb = pxb.tile([P, TW], F32, tag="rb", name="rb")
                    nc.sync.dma_start(rb[:], red_t[t][:, kc, :])
                    nc.vector.tensor_add(out=xb[:], in0=xb[:], in1=rb[:])
                    nc.vector.tensor_scalar_add(
                        x2[:, kc, ts(t, TW)], xb[:], bs2_sb[:, kc:kc + 1])
            # two half-passes over tokens: the first half only needs the
            # first two all-reduced chunks, so it overlaps the rest
            for half in range(2):
                for n in range(NCH):
                    bout_sb = pxb.tile([P, TW], F32, tag="bout", name="bout")
                    nc.sync.dma_start(bout_sb[:], bout_d[:, ts(n, TW)])
                    wot = []
                    for kc in range(KD):
                        wt = pwout.tile([P, TW], F32R, tag="wo", name="wo")
                        nc.sync.dma_start(wt[:], wout_d[n, ts(kc, P), :])
                        wot.append(wt)
                    for m in range(half * NT // 2, (half + 1) * NT // 2):
                        ps = psum_tile()
                        for kc in range(KD):
                            nc.tensor.matmul(
                                ps[:], lhsT=x2[:, kc, ts(m, P)], rhs=wot[kc][:],
                                start=(kc == 0), stop=(kc == KD - 1))
                        stg = pstg.tile([P, TW], F32, tag="stg", name="stg")
                        nc.vector.tensor_add(
                            out=stg[:], in0=ps[:], in1=bout_sb[:])
                        nc.sync.dma_start(logits_d[ts(m, P), ts(n, TW)], stg[:])

        for _pool in (pdram, pstg, ppsum, pbig, pconst):
            _pool.release()

    nc.compile()
    return nc


def _get_nc():
    if "nc" not in _NC_CACHE:
        _NC_CACHE["nc"] = _build_nc()
    return _NC_CACHE["nc"]


def _prep_in_maps(inputs):
    inp = {k: np.asarray(v) for k, v in inputs.items()}
    f32 = np.float32

    ids = np.ascontiguousarray(inp["input_ids"].reshape(T, 1).astype(np.int32))
    emb = np.ascontiguousarray(inp["emb"].astype(f32))
    WoS = np.ascontiguousarray(
        inp["Wo"].astype(f32).reshape(H, R, D).sum(0).astype(f32))
    g2 = inp["g2"].astype(f32)
    Wrg = np.ascontiguousarray((g2[:, None] * inp["Wr"].astype(f32)).astype(f32))
    breff = (inp["br"].astype(f32)
             + inp["beta2"].astype(f32) @ inp["Wr"].astype(f32))
    breff = np.ascontiguousarray(breff.reshape(E, 1).astype(f32))

    common = {
        "ids": ids, "emb": emb,
        "ones128": np.ones((P, P), f32),
        "Wv": np.ascontiguousarray(inp["Wv"].astype(f32)),
        "WoS": WoS, "Wrg": Wrg, "breff": breff,
        "g1v": inp["g1"].astype(f32), "b1v": inp["beta1"].astype(f32),
        "g2v": g2, "b2v": inp["beta2"].astype(f32),
        "bs2S": np.ascontiguousarray(inp["bs2"].astype(f32).sum(0)),
    }

    We1 = inp["We1"].astype(f32)
    be1 = inp["be1"].astype(f32)
    We2 = inp["We2"].astype(f32)
    be2 = inp["be2"].astype(f32)
    Ws1 = inp["Ws1"].astype(f32)
    bs1 = inp["bs1"].astype(f32)
    Ws2 = inp["Ws2"].astype(f32)
    Wout = inp["Wout"].astype(f32)
    bout = inp["bout"].astype(f32)

    in_maps = []
    for c in range(NCORES):
        el = list(range(ELOC * c, ELOC * (c + 1)))
        s, q = divmod(c, NCORES // NS)
        isl = slice(q * ILOC, (q + 1) * ILOC)
        Sbc = np.zeros((E, ELOC * P), f32)
        Ssel = np.zeros((E, ELOC), f32)
        for j, e in enumerate(el):
            Sbc[e, j * P:(j + 1) * P] = 1.0
            Ssel[e, j] = 1.0
        wout_pad = np.zeros((D, VPAD), f32)
        wout_pad[:, :VLOC] = Wout[:, VLOC * c:VLOC * (c + 1)]
        woutL = np.ascontiguousarray(
            wout_pad.reshape(D, NCH, TW).transpose(1, 0, 2))
        bout_pad = np.zeros((VPAD,), f32)
        bout_pad[:VLOC] = bout[VLOC * c:VLOC * (c + 1)]
        boutBC = np.ascontiguousarray(np.broadcast_to(bout_pad, (P, VPAD)))
        m = dict(common)
        m.update({
            "We1L": np.ascontiguousarray(We1[el]),
            "be1L": np.ascontiguousarray(be1[el]),
            "We2L": np.ascontiguousarray(We2[el]),
            "be2L": np.ascontiguousarray(be2[el]),
            "Ws1L": np.ascontiguousarray(Ws1[s][:, isl]),
            "bs1L": np.ascontiguousarray(bs1[s][isl]),
            "Ws2L": np.ascontiguousarray(Ws2[s][isl, :]),
            "Sbc": Sbc, "Ssel": Ssel,
            "WoutL": woutL, "boutBC": boutBC,
        })
        in_maps.append(m)
    return in_maps


def kernel(**inputs):
    in_maps = _prep_in_maps(inputs)
    nc = _get_nc()
    r = run_bass_kernel_spmd(nc, in_maps, list(range(NCORES)))
    logits = np.concatenate(
        [r.results[c]["logits"][:, :VLOC] for c in range(NCORES)], axis=1)
    return np.ascontiguousarray(logits.reshape(B, S, V).astype(np.float32))


if __name__ == "__main__":
    _build_nc()
    print("build + compile OK")


# revision 19
# speedup vs baseline: 1.0363x; 1.0044x over previous
"""Trainium2 Bass kernel for nn_BeyazKusAIEnhanced (moe_routing).

Model (T=2048 tokens, D=1024):
  x = emb[ids]
  h = LN1(x); attention collapses exactly to: ao = (h @ Wv) @ WoSum
    (softmax over a size-1 axis is exactly 1, so out = tile(v, 16 heads)
     and out @ Wo == v @ WoSum with WoSum[r,:] = sum_h Wo[h*64+r, :])
  x1 = x + ao
  t = LN2(x1); router probs = softmax(t @ Wr + br); top-8 -> combine [T,32]
  moe = sum_e combine[:,e] * (silu(t@We1[e]+be1[e]) @ We2[e] + be2[e])
  shared = sum_s silu(t@Ws1[s]+bs1[s]) @ Ws2[s] + bs2[s]
  out = (x1 + moe + shared) @ Wout + bout        [T, 32000]

Sharding (8 cores):
  - front part (gather/LN/attn/router) replicated on all cores
  - routed experts: 4 per core (dense compute; combine weights of
    non-selected experts are exactly 0, so dense == sparse w/ weights)
  - shared experts: inter dim (2*4096 = 8192) split 1024 per core;
    bs2 biases summed on host and added post-allreduce on every core
  - partial (moe+shared) accumulated in DRAM via accum-DMA, AllReduce'd
    across cores; x2 = x1 + reduced + bs2sum
  - output projection vocab-split: 4000 cols/core (padded to 4096)

Layout: activations feature-major [128 part, 8 kchunk, 2048 tok] in SBUF;
matmuls fp32r (full PE rate at moving free dim >= 256, ~1e-4 rel err).
LN stats via all-ones [128,128] matmul (partition-broadcast sums, no
explicit broadcast step); per-core expert selection via one-hot inputs.
Router runs in plain fp32 from x1 with LN folded (host folds g2 into Wr
and beta2@Wr into br) so top-8 selection is as close to the f32
reference as possible.
"""

import numpy as np

import concourse.bass as bass
import concourse.mybir as mybir
import concourse.tile as tile
from concourse import bacc
from concourse.bass import ts
from concourse.bass_utils import run_bass_kernel_spmd
from concourse.masks import make_identity

P = 128
B, S = 2, 1024
T = 2048          # tokens
D = 1024          # model dim
KD = D // P       # 8 k-chunks
H = 16            # heads
R = 64            # kv rank / head dim
E = 32            # routed experts
ELOC = 4          # experts per core
F = 1024          # moe inter dim
FC = F // P       # 8
NS = 2            # shared experts
ILOC = 1024       # shared inter slice per core
V = 32000
VLOC = 4000       # real vocab cols per core
VPAD = 4096       # padded to 8 x 512
NCH = VPAD // 512
TC = 4            # token chunks
TW = 512          # token chunk width
NT = T // P       # 16 token tiles
EPS = 1e-5
NCORES = 8

F32 = mybir.dt.float32
F32R = mybir.dt.float32r
I32 = mybir.dt.int32
AF = mybir.ActivationFunctionType
OP = mybir.AluOpType
AX = mybir.AxisListType

_NC_CACHE = {}


def _build_nc():
    nc = bacc.Bacc(None)

    ids_d = nc.declare_dram_parameter("ids", [T, 1], I32, isOutput=False)
    emb_d = nc.declare_dram_parameter("emb", [V, D], F32, isOutput=False)
    ones_d = nc.declare_dram_parameter("ones128", [P, P], F32R, isOutput=False)
    wv_d = nc.declare_dram_parameter("Wv", [D, R], F32, isOutput=False)
    wos_d = nc.declare_dram_parameter("WoS", [R, D], F32, isOutput=False)
    wrg_d = nc.declare_dram_parameter("Wrg", [D, E], F32, isOutput=False)
    breff_d = nc.declare_dram_parameter("breff", [E, 1], F32, isOutput=False)
    g1_d = nc.declare_dram_parameter("g1v", [D], F32, isOutput=False)
    b1_d = nc.declare_dram_parameter("b1v", [D], F32, isOutput=False)
    g2_d = nc.declare_dram_parameter("g2v", [D], F32, isOutput=False)
    b2_d = nc.declare_dram_parameter("b2v", [D], F32, isOutput=False)
    we1_d = nc.declare_dram_parameter("We1L", [ELOC, D, F], F32R, isOutput=False)
    be1_d = nc.declare_dram_parameter("be1L", [ELOC, F], F32, isOutput=False)
    we2_d = nc.declare_dram_parameter("We2L", [ELOC, F, D], F32R, isOutput=False)
    be2_d = nc.declare_dram_parameter("be2L", [ELOC, D], F32R, isOutput=False)
    ws1_d = nc.declare_dram_parameter("Ws1L", [D, ILOC], F32R, isOutput=False)
    bs1_d = nc.declare_dram_parameter("bs1L", [ILOC], F32, isOutput=False)
    ws2_d = nc.declare_dram_parameter("Ws2L", [ILOC, D], F32R, isOutput=False)
    bs2_d = nc.declare_dram_parameter("bs2S", [D], F32, isOutput=False)
    sbc_d = nc.declare_dram_parameter("Sbc", [E, ELOC * P], F32R, isOutput=False)
    ssel_d = nc.declare_dram_parameter("Ssel", [E, ELOC], F32R, isOutput=False)
    wout_d = nc.declare_dram_parameter("WoutL", [NCH, D, TW], F32R, isOutput=False)
    bout_d = nc.declare_dram_parameter("boutBC", [P, VPAD], F32, isOutput=False)
    logits_d = nc.declare_dram_parameter("logits", [T, VPAD], F32, isOutput=True)

    with tile.TileContext(nc) as tc:
        pconst = tc.alloc_tile_pool(name="pconst", bufs=1)
        pbig = tc.alloc_tile_pool(name="pbig", bufs=1)
        ppsum = tc.alloc_tile_pool(name="ppsum", bufs=6, space="PSUM")
        pstg = tc.alloc_tile_pool(name="pstg", bufs=3)
        pdram = tc.alloc_tile_pool(name="pdram", bufs=1, space="DRAM")

        def psum_tile():
            return ppsum.tile([P, TW], F32, tag="ps", name="ps", space="PSUM")

        # ---- small constants (~8.6 KB/partition) ----
        ident = pconst.tile([P, P], F32)
        make_identity(nc, ident[:])
        ones_sb = pconst.tile([P, P], F32R)
        nc.sync.dma_start(ones_sb[:], ones_d[:, :])
        wv_sb = pconst.tile([P, KD, R], F32)
        nc.sync.dma_start(wv_sb[:], wv_d.rearrange("(ko p) r -> p ko r", p=P))
        wos_sb = pconst.tile([R, KD, P], F32)
        nc.sync.dma_start(wos_sb[:], wos_d.rearrange("r (ko p) -> r ko p", p=P))
        wrg_sb = pconst.tile([P, KD, E], F32)
        nc.sync.dma_start(wrg_sb[:], wrg_d.rearrange("(ko p) e -> p ko e", p=P))
        breff_sb = pconst.tile([E, 1], F32)
        nc.sync.dma_start(breff_sb[:], breff_d[:, :])
        g1_sb = pconst.tile([P, KD], F32)
        nc.sync.dma_start(g1_sb[:], g1_d.rearrange("(ko p) -> p ko", p=P))
        b1_sb = pconst.tile([P, KD], F32)
        nc.sync.dma_start(b1_sb[:], b1_d.rearrange("(ko p) -> p ko", p=P))
        g2_sb = pconst.tile([P, KD], F32)
        nc.sync.dma_start(g2_sb[:], g2_d.rearrange("(ko p) -> p ko", p=P))
        b2_sb = pconst.tile([P, KD], F32)
        nc.sync.dma_start(b2_sb[:], b2_d.rearrange("(ko p) -> p ko", p=P))
        be1_sb = pconst.tile([P, ELOC, FC], F32)
        nc.sync.dma_start(be1_sb[:], be1_d.rearrange("e (ko p) -> p e ko", p=P))
        bs1_sb = pconst.tile([P, FC], F32)
        nc.sync.dma_start(bs1_sb[:], bs1_d.rearrange("(ko p) -> p ko", p=P))
        bs2_sb = pconst.tile([P, KD], F32)
        nc.sync.dma_start(bs2_sb[:], bs2_d.rearrange("(ko p) -> p ko", p=P))
        eps_sb = pconst.tile([P, 1], F32)
        nc.gpsimd.memset(eps_sb[:], EPS)

        # DRAM scratch
        x1_dram = pdram.tile([P, KD, T], F32, tag="x1d")
        acc_t = [pdram.tile([P, KD, TW], F32, tag=f"acc{t}", name=f"acc{t}")
                 for t in range(TC)]
        red_t = [pdram.tile([P, KD, TW], F32, tag=f"red{t}", name=f"red{t}",
                            addr_space="Shared")
                 for t in range(TC)]

        # combine-weight tiles + MoE selection constants (outlive front pools)
        pmoec = tc.alloc_tile_pool(name="pmoec", bufs=1)
        c_fm = pmoec.tile([E, T], F32R, tag="cfm")
        c_loc = pmoec.tile([ELOC, T], F32R, tag="cloc")
        sbc_sb = pmoec.tile([E, ELOC * P], F32R, tag="sbc")
        nc.sync.dma_start(sbc_sb[:], sbc_d[:, :])
        ssel_sb = pmoec.tile([E, ELOC], F32R, tag="ssel")
        nc.sync.dma_start(ssel_sb[:], ssel_d[:, :])
        be2_sb = pmoec.tile([ELOC, KD, P], F32R, tag="be2")
        nc.sync.dma_start(be2_sb[:], be2_d.rearrange("e (ko p) -> e ko p", p=P))

        pbigA = tc.alloc_tile_pool(name="pbigA", bufs=1)
        xa = pbigA.tile([P, KD, T], F32, tag="A")  # x, then x1 (in place)
        hb = pbig.tile([P, KD, T], F32, tag="B")  # h (fp32, feeds attention)

        # ---- phase 1: embedding gather + PE transpose to feature-major ----
        with (
            tc.tile_pool(name="pgather", bufs=4) as pgather,
            tc.tile_pool(name="pidx", bufs=NT) as pidx,
        ):
            idxs = []
            for i in range(NT):
                idx_sb = pidx.tile([P, 1], I32, tag="idx", name="idx")
                nc.sync.dma_start(idx_sb[:], ids_d[i * P:(i + 1) * P, :])
                idxs.append(idx_sb)
            for i in range(NT):
                idx_sb = idxs[i]
                gx = pgather.tile([P, D], F32, tag="gx", name="gx")
                nc.gpsimd.indirect_dma_start(
                    out=gx[:],
                    out_offset=None,
                    in_=emb_d[:, :],
                    in_offset=bass.IndirectOffsetOnAxis(ap=idx_sb[:, :1], axis=0),
                )
                for kc in range(KD):
                    tp = psum_tile()
                    nc.tensor.transpose(tp[:, :P], gx[:, ts(kc, P)], ident[:])
                    nc.vector.tensor_copy(xa[:, kc, i * P:(i + 1) * P], tp[:, :P])

        # ---- phases 2-5 (LN1, attention, LN2+router fused) ----
        with (
            tc.tile_pool(name="pfC", bufs=1) as pfC,
            tc.tile_pool(name="pfM", bufs=2) as pfM,
        ):
            pfA = tc.alloc_tile_pool(name="pfA", bufs=2)
            pfB = tc.alloc_tile_pool(name="pfB", bufs=2 * TC)

            def ln_stats(src, t):
                """LN stats for token chunk t -> (mu, rstd) tiles [P, TW]
                (every partition holds the same per-token row)."""
                ps_mu = psum_tile()
                ps_sq = psum_tile()
                for kc in range(KD):
                    xr = pfA.tile([P, TW], F32R, tag="sq", name="xr")
                    nc.vector.tensor_copy(xr[:], src[:, kc, ts(t, TW)])
                    nc.tensor.matmul(
                        ps_mu[:], lhsT=ones_sb[:], rhs=xr[:],
                        start=(kc == 0), stop=(kc == KD - 1))
                    sq = pfA.tile([P, TW], F32R, tag="sq", name="sq")
                    nc.scalar.activation(sq[:], src[:, kc, ts(t, TW)], AF.Square)
                    nc.tensor.matmul(
                        ps_sq[:], lhsT=ones_sb[:], rhs=sq[:],
                        start=(kc == 0), stop=(kc == KD - 1))
                mu = pfB.tile([P, TW], F32, tag="bc", name="mu")
                nc.vector.tensor_scalar_mul(mu[:], ps_mu[:], 1.0 / D)
                msq = pfA.tile([P, TW], F32, tag="lntmp", name="msq")
                nc.vector.tensor_scalar_mul(msq[:], ps_sq[:], 1.0 / D)
                mu2 = pfA.tile([P, TW], F32, tag="lntmp", name="mu2")
                nc.vector.tensor_mul(out=mu2[:], in0=mu[:], in1=mu[:])
                nc.vector.tensor_tensor(msq[:], msq[:], mu2[:], op=OP.subtract)
                nc.scalar.activation(msq[:], msq[:], AF.Sqrt, bias=eps_sb[:, 0:1])
                rstd = pfB.tile([P, TW], F32, tag="bc", name="rstd")
                nc.vector.reciprocal(rstd[:], msq[:])
                return mu, rstd

            def ln_apply(src, dst, t, mu, rstd, g_sb, b_sb):
                for kc in range(KD):
                    nc.vector.tensor_tensor(
                        dst[:, kc, ts(t, TW)], src[:, kc, ts(t, TW)], mu[:],
                        op=OP.subtract)
                    nc.vector.tensor_tensor(
                        dst[:, kc, ts(t, TW)], dst[:, kc, ts(t, TW)], rstd[:],
                        op=OP.mult)
                    nc.vector.tensor_scalar(
                        dst[:, kc, ts(t, TW)], dst[:, kc, ts(t, TW)],
                        g_sb[:, kc:kc + 1], b_sb[:, kc:kc + 1],
                        op0=OP.mult, op1=OP.add)

            # LN1 -> h (stats for all chunks first, applies after: keeps
            # the PE stats matmuls from waiting behind DVE apply chains)
            st1 = [ln_stats(xa, t) for t in range(TC)]
            for t in range(TC):
                ln_apply(xa, hb, t, st1[t][0], st1[t][1], g1_sb, b1_sb)

            # v = h @ Wv  [R, T]
            v_sb = pfC.tile([R, T], F32, tag="v")
            for t in range(TC):
                ps = psum_tile()
                for kc in range(KD):
                    nc.tensor.matmul(
                        ps[:R, :], lhsT=wv_sb[:, kc, :], rhs=hb[:, kc, ts(t, TW)],
                        start=(kc == 0), stop=(kc == KD - 1))
                nc.vector.tensor_copy(v_sb[:, ts(t, TW)], ps[:R, :])
            # x1 = x + v @ WoSum  (in place into xa)
            for dc in range(KD):
                for t in range(TC):
                    ps = psum_tile()
                    nc.tensor.matmul(
                        ps[:], lhsT=wos_sb[:, dc, :], rhs=v_sb[:, ts(t, TW)],
                        start=True, stop=True)
                    nc.vector.tensor_add(
                        out=xa[:, dc, ts(t, TW)], in0=xa[:, dc, ts(t, TW)],
                        in1=ps[:])
            nc.sync.dma_start(x1_dram[:], xa[:])

            # LN2 -> t (f32r, into slot B), fused with fp32 router matmul
            tb = pbig.tile([P, KD, T], F32R, tag="B", name="tb")
            r_fm = pfC.tile([E, T], F32, tag="v", name="r_fm")
            st2 = [ln_stats(xa, t) for t in range(TC)]
            for t in range(TC):
                mu, rstd = st2[t]
                ln_apply(xa, tb, t, mu, rstd, g2_sb, b2_sb)
                ps = psum_tile()
                for kc in range(KD):
                    rt = pfA.tile([P, TW], F32, tag="rt", name="rt")
                    nc.vector.tensor_tensor(
                        rt[:], xa[:, kc, ts(t, TW)], mu[:],
                        op=OP.subtract)
                    nc.tensor.matmul(
                        ps[:E, :], lhsT=wrg_sb[:, kc, :], rhs=rt[:],
                        start=(kc == 0), stop=(kc == KD - 1))
                nc.vector.tensor_tensor(
                    r_fm[:, ts(t, TW)], ps[:E, :], rstd[:E, :], op=OP.mult)
                nc.vector.tensor_scalar_add(
                    r_fm[:, ts(t, TW)], r_fm[:, ts(t, TW)], breff_sb[:E, 0:1])

            pfB.release()
            pfA.release()

            # softmax + top-8 in token-major
            r_tm = pfC.tile([P, NT, E], F32, tag="rtm")
            for i in range(NT):
                tp = psum_tile()
                nc.tensor.transpose(
                    tp[:, :E], r_fm[:, i * P:(i + 1) * P], ident[:E, :E])
                nc.vector.tensor_copy(r_tm[:, i, :], tp[:, :E])
            m_sb = pfM.tile([P, NT], F32, tag="m", name="m1")
            nc.vector.reduce_max(m_sb[:, :, None], r_tm[:], axis=AX.X)
            nc.vector.tensor_tensor(
                r_tm[:], r_tm[:], m_sb[:, :, None].to_broadcast([P, NT, E]),
                op=OP.subtract)
            nc.scalar.activation(r_tm[:], r_tm[:], AF.Exp)
            s_sb = pfM.tile([P, NT], F32, tag="m", name="m2")
            nc.vector.reduce_sum(s_sb[:, :, None], r_tm[:], axis=AX.X)
            rs_sb = pfM.tile([P, NT], F32, tag="m", name="m3")
            nc.vector.reciprocal(rs_sb[:], s_sb[:])
            nc.vector.tensor_tensor(
                r_tm[:], r_tm[:], rs_sb[:, :, None].to_broadcast([P, NT, E]),
                op=OP.mult)
            work = pmoec.tile([P, NT, E], F32, tag="work")
            msk = pfC.tile([P, NT, E], F32, tag="msk")
            nc.vector.tensor_copy(work[:], r_tm[:])
            thr = pfM.tile([P, NT], F32, tag="m", name="m4")
            for it in range(8):
                nc.vector.reduce_max(thr[:, :, None], work[:], axis=AX.X)
                if it < 7:
                    nc.vector.tensor_tensor(
                        msk[:], work[:], thr[:, :, None].to_broadcast([P, NT, E]),
                        op=OP.is_lt)
                    nc.vector.tensor_tensor(work[:], work[:], msk[:], op=OP.mult)
            nc.vector.tensor_tensor(
                msk[:], r_tm[:], thr[:, :, None].to_broadcast([P, NT, E]),
                op=OP.is_ge)
            nc.vector.tensor_tensor(work[:], r_tm[:], msk[:], op=OP.mult)
            wsum = pfM.tile([P, NT], F32, tag="m", name="m5")
            nc.vector.reduce_sum(wsum[:, :, None], work[:], axis=AX.X)
            rws = pfM.tile([P, NT], F32, tag="m", name="m6")
            nc.vector.reciprocal(rws[:], wsum[:])
            nc.vector.tensor_tensor(
                work[:], work[:], rws[:, :, None].to_broadcast([P, NT, E]),
                op=OP.mult)

        pbigA.release()

        T2 = T // 2

        # ---- phase 6: MoE (4 routed dense + shared slice) ----
        # z is split into two token-half buffers so expert e+1's first
        # matmuls can overwrite one half while expert e's second matmuls
        # still read the other: experts pipeline on the PE.
        with (
            tc.tile_pool(name="pw", bufs=8) as pw,
            tc.tile_pool(name="pcbc", bufs=1) as pcbc,
            tc.tile_pool(name="pz", bufs=2) as pz,
        ):
            def emit_cbc(e):
                cbc = pcbc.tile([P, T], F32, tag="cbc", name="cbc")
                for t in range(TC):
                    ps = psum_tile()
                    nc.tensor.matmul(
                        ps[:], lhsT=sbc_sb[:, ts(e, P)],
                        rhs=c_fm[:, ts(t, TW)], start=True, stop=True)
                    nc.vector.tensor_copy(cbc[:, ts(t, TW)], ps[:])
                return cbc

            for e in range(ELOC + 1):
                shared = e == ELOC
                cbc = emit_cbc(e) if (not shared and e > 0) else None
                w1t = []
                for kc in range(KD):
                    wt = pw.tile([P, F], F32R, tag="w", name="w1t")
                    src_ap = (ws1_d[ts(kc, P), :] if shared
                              else we1_d[e, ts(kc, P), :])
                    nc.sync.dma_start(wt[:], src_ap)
                    w1t.append(wt)
                zh = [pz.tile([P, FC, T2], F32R, tag="z", name="zh")
                      for _ in range(2)]
                for half in range(2):
                    for fc in range(FC):
                        bias = (bs1_sb[:, fc:fc + 1] if shared
                                else be1_sb[:, e, fc:fc + 1])
                        for t2 in range(TC // 2):
                            t = half * (TC // 2) + t2
                            ps = psum_tile()
                            for kc in range(KD):
                                nc.tensor.matmul(
                                    ps[:], lhsT=w1t[kc][:, ts(fc, P)],
                                    rhs=tb[:, kc, ts(t, TW)],
                                    start=(kc == 0), stop=(kc == KD - 1))
                            nc.scalar.activation(
                                zh[half][:, fc, ts(t2, TW)], ps[:], AF.Silu,
                                bias=bias)
                            if cbc is not None:
                                nc.vector.tensor_tensor(
                                    zh[half][:, fc, ts(t2, TW)],
                                    zh[half][:, fc, ts(t2, TW)],
                                    cbc[:, ts(t, TW)], op=OP.mult)
                if e == 0:
                    # combine weights: expert-major + per-core selection.
                    # Deferred here so expert 0's first matmuls don't wait
                    # on the top-k DVE chain.
                    for i in range(NT):
                        tp = psum_tile()
                        nc.tensor.transpose(tp[:E, :P], work[:, i, :], ident[:])
                        nc.vector.tensor_copy(
                            c_fm[:, i * P:(i + 1) * P], tp[:E, :P])
                    for t in range(TC):
                        ps = psum_tile()
                        nc.tensor.matmul(
                            ps[:ELOC, :], lhsT=ssel_sb[:],
                            rhs=c_fm[:, ts(t, TW)], start=True, stop=True)
                        nc.vector.tensor_copy(c_loc[:, ts(t, TW)], ps[:ELOC, :])
                    cbc = emit_cbc(0)
                    for half in range(2):
                        for t2 in range(TC // 2):
                            t = half * (TC // 2) + t2
                            for fc in range(FC):
                                nc.vector.tensor_tensor(
                                    zh[half][:, fc, ts(t2, TW)],
                                    zh[half][:, fc, ts(t2, TW)],
                                    cbc[:, ts(t, TW)], op=OP.mult)
                w2t = []
                for fc in range(FC):
                    wt = pw.tile([P, D], F32R, tag="w", name="w2t")
                    src_ap = (ws2_d[ts(fc, P), :] if shared
                              else we2_d[e, ts(fc, P), :])
                    nc.sync.dma_start(wt[:], src_ap)
                    w2t.append(wt)
                for t in range(TC):
                    half, t2 = divmod(t, TC // 2)
                    for dc in range(KD):
                        ps = psum_tile()
                        for fc in range(FC):
                            nc.tensor.matmul(
                                ps[:], lhsT=w2t[fc][:, ts(dc, P)],
                                rhs=zh[half][:, fc, ts(t2, TW)],
                                start=(fc == 0),
                                stop=(fc == FC - 1 and not shared))
                        if shared:
                            nc.tensor.matmul(
                                ps[:], lhsT=be2_sb[:, dc, :],
                                rhs=c_loc[:, ts(t, TW)],
                                start=False, stop=True)
                        stg = pstg.tile([P, TW], F32, tag="stg", name="stg")
                        nc.scalar.activation(stg[:], ps[:], AF.Copy)
                        nc.gpsimd.dma_start(
                            acc_t[t][:, dc, :], stg[:],
                            accum_op=(OP.bypass if e == 0 else OP.add))
                    if shared:
                        # chunk t of the partial sum is complete on this
                        # core: all-reduce it while the MoE tail and the
                        # output projection proceed
                        nc.gpsimd.collective_compute(
                            "AllReduce",
                            OP.add,
                            replica_groups=[list(range(NCORES))],
                            ins=[acc_t[t][:].opt()],
                            outs=[red_t[t][:].opt()],
                        )
        pmoec.release()

        # ---- phase 7: AllReduce; x2 = x1 + red + bs2sum; out projection ----
        with (
            tc.tile_pool(name="pxb", bufs=3) as pxb,
            tc.tile_pool(name="pwout", bufs=12) as pwout,
        ):
            x2 = pbig.tile([P, KD, T], F32R, tag="B")
            # two half-passes over tokens: the first half only needs the
            # first two all-reduced chunks, so its projection overlaps the
            # later all-reduces (and the engines stay in-order-clean)
            for half in range(2):
                for t in range(2 * half, 2 * half + 2):
                    for kc in range(KD):
                        xb = pxb.tile([P, TW], F32, tag="xb", name="xb")
                        nc.sync.dma_start(xb[:], x1_dram[:, kc, ts(t, TW)])
                        rb = pxb.tile([P, TW], F32, tag="rb", name="rb")
                        nc.sync.dma_start(rb[:], red_t[t][:, kc, :])
                        nc.vector.tensor_add(out=xb[:], in0=xb[:], in1=rb[:])
                        nc.vector.tensor_scalar_add(
                            x2[:, kc, ts(t, TW)], xb[:], bs2_sb[:, kc:kc + 1])
                for n in range(NCH):
                    bout_sb = pxb.tile([P, TW], F32, tag="bout", name="bout")
                    nc.sync.dma_start(bout_sb[:], bout_d[:, ts(n, TW)])
                    wot = []
                    for kc in range(KD):
                        wt = pwout.tile([P, TW], F32R, tag="wo", name="wo")
                        nc.sync.dma_start(wt[:], wout_d[n, ts(kc, P), :])
                        wot.append(wt)
                    for m in range(half * NT // 2, (half + 1) * NT // 2):
                        ps = psum_tile()
                        for kc in range(KD):
                            nc.tensor.matmul(
                                ps[:], lhsT=x2[:, kc, ts(m, P)], rhs=wot[kc][:],
                                start=(kc == 0), stop=(kc == KD - 1))
                        stg = pstg.tile([P, TW], F32, tag="stg", name="stg")
                        nc.vector.tensor_add(
                            out=stg[:], in0=ps[:], in1=bout_sb[:])
                        nc.sync.dma_start(logits_d[ts(m, P), ts(n, TW)], stg[:])

        for _pool in (pdram, pstg, ppsum, pbig, pconst):
            _pool.release()

    nc.compile()
    return nc


def _get_nc():
    if "nc" not in _NC_CACHE:
        _NC_CACHE["nc"] = _build_nc()
    return _NC_CACHE["nc"]


def _prep_in_maps(inputs):
    inp = {k: np.asarray(v) for k, v in inputs.items()}
    f32 = np.float32

    ids = np.ascontiguousarray(inp["input_ids"].reshape(T, 1).astype(np.int32))
    emb = np.ascontiguousarray(inp["emb"].astype(f32))
    WoS = np.ascontiguousarray(
        inp["Wo"].astype(f32).reshape(H, R, D).sum(0).astype(f32))
    g2 = inp["g2"].astype(f32)
    Wrg = np.ascontiguousarray((g2[:, None] * inp["Wr"].astype(f32)).astype(f32))
    breff = (inp["br"].astype(f32)
             + inp["beta2"].astype(f32) @ inp["Wr"].astype(f32))
    breff = np.ascontiguousarray(breff.reshape(E, 1).astype(f32))

    common = {
        "ids": ids, "emb": emb,
        "ones128": np.ones((P, P), f32),
        "Wv": np.ascontiguousarray(inp["Wv"].astype(f32)),
        "WoS": WoS, "Wrg": Wrg, "breff": breff,
        "g1v": inp["g1"].astype(f32), "b1v": inp["beta1"].astype(f32),
        "g2v": g2, "b2v": inp["beta2"].astype(f32),
        "bs2S": np.ascontiguousarray(inp["bs2"].astype(f32).sum(0)),
    }

    We1 = inp["We1"].astype(f32)
    be1 = inp["be1"].astype(f32)
    We2 = inp["We2"].astype(f32)
    be2 = inp["be2"].astype(f32)
    Ws1 = inp["Ws1"].astype(f32)
    bs1 = inp["bs1"].astype(f32)
    Ws2 = inp["Ws2"].astype(f32)
    Wout = inp["Wout"].astype(f32)
    bout = inp["bout"].astype(f32)

    in_maps = []
    for c in range(NCORES):
        el = list(range(ELOC * c, ELOC * (c + 1)))
        s, q = divmod(c, NCORES // NS)
        isl = slice(q * ILOC, (q + 1) * ILOC)
        Sbc = np.zeros((E, ELOC * P), f32)
        Ssel = np.zeros((E, ELOC), f32)
        for j, e in enumerate(el):
            Sbc[e, j * P:(j + 1) * P] = 1.0
            Ssel[e, j] = 1.0
        wout_pad = np.zeros((D, VPAD), f32)
        wout_pad[:, :VLOC] = Wout[:, VLOC * c:VLOC * (c + 1)]
        woutL = np.ascontiguousarray(
            wout_pad.reshape(D, NCH, TW).transpose(1, 0, 2))
        bout_pad = np.zeros((VPAD,), f32)
        bout_pad[:VLOC] = bout[VLOC * c:VLOC * (c + 1)]
        boutBC = np.ascontiguousarray(np.broadcast_to(bout_pad, (P, VPAD)))
        m = dict(common)
        m.update({
            "We1L": np.ascontiguousarray(We1[el]),
            "be1L": np.ascontiguousarray(be1[el]),
            "We2L": np.ascontiguousarray(We2[el]),
            "be2L": np.ascontiguousarray(be2[el]),
            "Ws1L": np.ascontiguousarray(Ws1[s][:, isl]),
            "bs1L": np.ascontiguousarray(bs1[s][isl]),
            "Ws2L": np.ascontiguousarray(Ws2[s][isl, :]),
            "Sbc": Sbc, "Ssel": Ssel,
            "WoutL": woutL, "boutBC": boutBC,
        })
        in_maps.append(m)
    return in_maps


def kernel(**inputs):
    in_maps = _prep_in_maps(inputs)
    nc = _get_nc()
    r = run_bass_kernel_spmd(nc, in_maps, list(range(NCORES)))
    logits = np.concatenate(
        [r.results[c]["logits"][:, :VLOC] for c in range(NCORES)], axis=1)
    return np.ascontiguousarray(logits.reshape(B, S, V).astype(np.float32))


if __name__ == "__main__":
    _build_nc()
    print("build + compile OK")


# revision 22
# speedup vs baseline: 1.0806x; 1.0427x over previous
"""Trainium2 Bass kernel for nn_BeyazKusAIEnhanced (moe_routing).

Model (T=2048 tokens, D=1024):
  x = emb[ids]
  h = LN1(x); attention collapses exactly to: ao = (h @ Wv) @ WoSum
    (softmax over a size-1 axis is exactly 1, so out = tile(v, 16 heads)
     and out @ Wo == v @ WoSum with WoSum[r,:] = sum_h Wo[h*64+r, :])
  x1 = x + ao
  t = LN2(x1); router probs = softmax(t @ Wr + br); top-8 -> combine [T,32]
  moe = sum_e combine[:,e] * (silu(t@We1[e]+be1[e]) @ We2[e] + be2[e])
  shared = sum_s silu(t@Ws1[s]+bs1[s]) @ Ws2[s] + bs2[s]
  out = (x1 + moe + shared) @ Wout + bout        [T, 32000]

Sharding (8 cores):
  - front part (gather/LN/attn/router) replicated on all cores
  - routed experts: 4 per core (dense compute; combine weights of
    non-selected experts are exactly 0, so dense == sparse w/ weights)
  - shared experts: inter dim (2*4096 = 8192) split 1024 per core;
    bs2 biases summed on host and added post-allreduce on every core
  - partial (moe+shared) accumulated in DRAM via accum-DMA, AllReduce'd
    across cores; x2 = x1 + reduced + bs2sum
  - output projection vocab-split: 4000 cols/core (padded to 4096)

Layout: activations feature-major [128 part, 8 kchunk, 2048 tok] in SBUF;
matmuls fp32r (full PE rate at moving free dim >= 256, ~1e-4 rel err).
LN stats via all-ones [128,128] matmul (partition-broadcast sums, no
explicit broadcast step); per-core expert selection via one-hot inputs.
Router runs in plain fp32 from x1 with LN folded (host folds g2 into Wr
and beta2@Wr into br) so top-8 selection is as close to the f32
reference as possible.
"""

import numpy as np

import concourse.bass as bass
import concourse.mybir as mybir
import concourse.tile as tile
from concourse import bacc
from concourse.bass import ts
from concourse.bass_utils import run_bass_kernel_spmd
from concourse.masks import make_identity

P = 128
B, S = 2, 1024
T = 2048          # tokens
D = 1024          # model dim
KD = D // P       # 8 k-chunks
H = 16            # heads
R = 64            # kv rank / head dim
E = 32            # routed experts
ELOC = 4          # experts per core
F = 1024          # moe inter dim
FC = F // P       # 8
NS = 2            # shared experts
ILOC = 1024       # shared inter slice per core
V = 32000
VLOC = 4000       # real vocab cols per core
VPAD = 4096       # padded to 8 x 512
NCH = VPAD // 512
TC = 4            # token chunks
TW = 512          # token chunk width
NT = T // P       # 16 token tiles
EPS = 1e-5
NCORES = 8

F32 = mybir.dt.float32
F32R = mybir.dt.float32r
I32 = mybir.dt.int32
AF = mybir.ActivationFunctionType
OP = mybir.AluOpType
AX = mybir.AxisListType

_NC_CACHE = {}


def _build_nc():
    nc = bacc.Bacc(None)

    ids_d = nc.declare_dram_parameter("ids", [T, 1], I32, isOutput=False)
    emb_d = nc.declare_dram_parameter("emb", [V, D], F32, isOutput=False)
    ones_d = nc.declare_dram_parameter("ones128", [P, P], F32R, isOutput=False)
    wv_d = nc.declare_dram_parameter("Wv", [D, R], F32, isOutput=False)
    wos_d = nc.declare_dram_parameter("WoS", [R, D], F32, isOutput=False)
    wrg_d = nc.declare_dram_parameter("Wrg", [D, E], F32, isOutput=False)
    breff_d = nc.declare_dram_parameter("breff", [E, 1], F32, isOutput=False)
    g1_d = nc.declare_dram_parameter("g1v", [D], F32, isOutput=False)
    b1_d = nc.declare_dram_parameter("b1v", [D], F32, isOutput=False)
    g2_d = nc.declare_dram_parameter("g2v", [D], F32, isOutput=False)
    b2_d = nc.declare_dram_parameter("b2v", [D], F32, isOutput=False)
    we1_d = nc.declare_dram_parameter("We1L", [ELOC, FC, D, P], F32R,
                                      isOutput=False)
    be1_d = nc.declare_dram_parameter("be1L", [ELOC, F], F32, isOutput=False)
    we2_d = nc.declare_dram_parameter("We2L", [ELOC, KD, F, P], F32R,
                                      isOutput=False)
    be2_d = nc.declare_dram_parameter("be2P", [E, D], F32R, isOutput=False)
    ws1_d = nc.declare_dram_parameter("Ws1L", [FC, D, P], F32R, isOutput=False)
    bs1_d = nc.declare_dram_parameter("bs1L", [ILOC], F32, isOutput=False)
    ws2_d = nc.declare_dram_parameter("Ws2L", [KD, ILOC, P], F32R, isOutput=False)
    bs2_d = nc.declare_dram_parameter("bs2S", [D], F32, isOutput=False)
    sbc_d = nc.declare_dram_parameter("Sbc", [E, ELOC * P], F32R, isOutput=False)
    wout_d = nc.declare_dram_parameter("WoutL", [NCH, D, TW], F32R, isOutput=False)
    bout_d = nc.declare_dram_parameter("boutBC", [P, VPAD], F32, isOutput=False)
    logits_d = nc.declare_dram_parameter("logits", [T, VPAD], F32, isOutput=True)

    with tile.TileContext(nc) as tc:
        pconst = tc.alloc_tile_pool(name="pconst", bufs=1)
        pbig = tc.alloc_tile_pool(name="pbig", bufs=1)
        ppsum = tc.alloc_tile_pool(name="ppsum", bufs=6, space="PSUM")
        pstg = tc.alloc_tile_pool(name="pstg", bufs=3)
        pdram = tc.alloc_tile_pool(name="pdram", bufs=1, space="DRAM")

        def psum_tile():
            return ppsum.tile([P, TW], F32, tag="ps", name="ps", space="PSUM")

        # ---- small constants (~8.6 KB/partition) ----
        ident = pconst.tile([P, P], F32)
        make_identity(nc, ident[:])
        ones_sb = pconst.tile([P, P], F32R)
        nc.sync.dma_start(ones_sb[:], ones_d[:, :])
        wv_sb = pconst.tile([P, KD, R], F32)
        nc.sync.dma_start(wv_sb[:], wv_d.rearrange("(ko p) r -> p ko r", p=P))
        wos_sb = pconst.tile([R, KD, P], F32)
        nc.sync.dma_start(wos_sb[:], wos_d.rearrange("r (ko p) -> r ko p", p=P))
        wrg_sb = pconst.tile([P, KD, E], F32)
        nc.sync.dma_start(wrg_sb[:], wrg_d.rearrange("(ko p) e -> p ko e", p=P))
        breff_sb = pconst.tile([E, 1], F32)
        nc.sync.dma_start(breff_sb[:], breff_d[:, :])
        g1_sb = pconst.tile([P, KD], F32)
        nc.sync.dma_start(g1_sb[:], g1_d.rearrange("(ko p) -> p ko", p=P))
        b1_sb = pconst.tile([P, KD], F32)
        nc.sync.dma_start(b1_sb[:], b1_d.rearrange("(ko p) -> p ko", p=P))
        g2_sb = pconst.tile([P, KD], F32)
        nc.sync.dma_start(g2_sb[:], g2_d.rearrange("(ko p) -> p ko", p=P))
        b2_sb = pconst.tile([P, KD], F32)
        nc.sync.dma_start(b2_sb[:], b2_d.rearrange("(ko p) -> p ko", p=P))
        be1_sb = pconst.tile([P, ELOC, FC], F32)
        nc.sync.dma_start(be1_sb[:], be1_d.rearrange("e (ko p) -> p e ko", p=P))
        bs1_sb = pconst.tile([P, FC], F32)
        nc.sync.dma_start(bs1_sb[:], bs1_d.rearrange("(ko p) -> p ko", p=P))
        bs2_sb = pconst.tile([P, KD], F32)
        nc.sync.dma_start(bs2_sb[:], bs2_d.rearrange("(ko p) -> p ko", p=P))
        eps_sb = pconst.tile([P, 1], F32)
        nc.gpsimd.memset(eps_sb[:], EPS)

        # DRAM scratch
        x1_dram = pdram.tile([P, KD, T], F32, tag="x1d")
        acc_h = [pdram.tile([P, KD, T // 2], F32, tag=f"acc{h}", name=f"acc{h}")
                 for h in range(2)]
        red_h = [pdram.tile([P, KD, T // 2], F32, tag=f"red{h}", name=f"red{h}",
                            addr_space="Shared")
                 for h in range(2)]

        # combine-weight tiles + MoE selection constants (outlive front pools)
        pmoec = tc.alloc_tile_pool(name="pmoec", bufs=1)
        c_fm = pmoec.tile([E, T], F32R, tag="cfm")
        sbc_sb = pmoec.tile([E, ELOC * P], F32R, tag="sbc")
        nc.sync.dma_start(sbc_sb[:], sbc_d[:, :])
        be2_sb = pmoec.tile([E, KD, P], F32R, tag="be2")
        nc.sync.dma_start(be2_sb[:], be2_d.rearrange("e (ko p) -> e ko p", p=P))

        pbigA = tc.alloc_tile_pool(name="pbigA", bufs=1)
        xa = pbigA.tile([P, KD, T], F32, tag="A")  # x, then x1 (in place)
        hb = pbig.tile([P, KD, T], F32, tag="B")  # h (fp32, feeds attention)

        # ---- phase 1: embedding gather + PE transpose to feature-major ----
        with (
            tc.tile_pool(name="pgather", bufs=4) as pgather,
            tc.tile_pool(name="pidx", bufs=NT) as pidx,
        ):
            idxs = []
            for i in range(NT):
                idx_sb = pidx.tile([P, 1], I32, tag="idx", name="idx")
                nc.sync.dma_start(idx_sb[:], ids_d[i * P:(i + 1) * P, :])
                idxs.append(idx_sb)
            for i in range(NT):
                idx_sb = idxs[i]
                gx = pgather.tile([P, D], F32, tag="gx", name="gx")
                nc.gpsimd.indirect_dma_start(
                    out=gx[:],
                    out_offset=None,
                    in_=emb_d[:, :],
                    in_offset=bass.IndirectOffsetOnAxis(ap=idx_sb[:, :1], axis=0),
                )
                for kc in range(KD):
                    tp = psum_tile()
                    nc.tensor.transpose(tp[:, :P], gx[:, ts(kc, P)], ident[:])
                    nc.vector.tensor_copy(xa[:, kc, i * P:(i + 1) * P], tp[:, :P])

        # ---- phases 2-5 (LN1, attention, LN2+router fused) ----
        with (
            tc.tile_pool(name="pfC", bufs=1) as pfC,
            tc.tile_pool(name="pfM", bufs=2) as pfM,
        ):
            pfA = tc.alloc_tile_pool(name="pfA", bufs=2)
            pfB = tc.alloc_tile_pool(name="pfB", bufs=2 * TC)

            def ln_stats(src, t):
                """LN stats for token chunk t -> (mu, rstd) tiles [P, TW]
                (every partition holds the same per-token row)."""
                ps_mu = psum_tile()
                ps_sq = psum_tile()
                for kc in range(KD):
                    xr = pfA.tile([P, TW], F32R, tag="sq", name="xr")
                    nc.vector.tensor_copy(xr[:], src[:, kc, ts(t, TW)])
                    nc.tensor.matmul(
                        ps_mu[:], lhsT=ones_sb[:], rhs=xr[:],
                        start=(kc == 0), stop=(kc == KD - 1))
                    sq = pfA.tile([P, TW], F32R, tag="sq", name="sq")
                    nc.scalar.activation(sq[:], src[:, kc, ts(t, TW)], AF.Square)
                    nc.tensor.matmul(
                        ps_sq[:], lhsT=ones_sb[:], rhs=sq[:],
                        start=(kc == 0), stop=(kc == KD - 1))
                mu = pfB.tile([P, TW], F32, tag="bc", name="mu")
                nc.vector.tensor_scalar_mul(mu[:], ps_mu[:], 1.0 / D)
                msq = pfA.tile([P, TW], F32, tag="lntmp", name="msq")
                nc.vector.tensor_scalar_mul(msq[:], ps_sq[:], 1.0 / D)
                mu2 = pfA.tile([P, TW], F32, tag="lntmp", name="mu2")
                nc.vector.tensor_mul(out=mu2[:], in0=mu[:], in1=mu[:])
                nc.vector.tensor_tensor(msq[:], msq[:], mu2[:], op=OP.subtract)
                nc.scalar.activation(msq[:], msq[:], AF.Sqrt, bias=eps_sb[:, 0:1])
                rstd = pfB.tile([P, TW], F32, tag="bc", name="rstd")
                nc.vector.reciprocal(rstd[:], msq[:])
                return mu, rstd

            def ln_apply(src, dst, t, mu, rstd, g_sb, b_sb):
                for kc in range(KD):
                    nc.vector.tensor_tensor(
                        dst[:, kc, ts(t, TW)], src[:, kc, ts(t, TW)], mu[:],
                        op=OP.subtract)
                    nc.vector.tensor_tensor(
                        dst[:, kc, ts(t, TW)], dst[:, kc, ts(t, TW)], rstd[:],
                        op=OP.mult)
                    nc.vector.tensor_scalar(
                        dst[:, kc, ts(t, TW)], dst[:, kc, ts(t, TW)],
                        g_sb[:, kc:kc + 1], b_sb[:, kc:kc + 1],
                        op0=OP.mult, op1=OP.add)

            # LN1 -> h (stats for all chunks first, applies after: keeps
            # the PE stats matmuls from waiting behind DVE apply chains)
            st1 = [ln_stats(xa, t) for t in range(TC)]
            for t in range(TC):
                ln_apply(xa, hb, t, st1[t][0], st1[t][1], g1_sb, b1_sb)

            # v = h @ Wv  [R, T]
            v_sb = pfC.tile([R, T], F32, tag="v")
            for t in range(TC):
                ps = psum_tile()
                for kc in range(KD):
                    nc.tensor.matmul(
                        ps[:R, :], lhsT=wv_sb[:, kc, :], rhs=hb[:, kc, ts(t, TW)],
                        start=(kc == 0), stop=(kc == KD - 1))
                nc.vector.tensor_copy(v_sb[:, ts(t, TW)], ps[:R, :])
            # x1 = x + v @ WoSum  (in place into xa)
            for dc in range(KD):
                for t in range(TC):
                    ps = psum_tile()
                    nc.tensor.matmul(
                        ps[:], lhsT=wos_sb[:, dc, :], rhs=v_sb[:, ts(t, TW)],
                        start=True, stop=True)
                    nc.vector.tensor_add(
                        out=xa[:, dc, ts(t, TW)], in0=xa[:, dc, ts(t, TW)],
                        in1=ps[:])
            nc.sync.dma_start(x1_dram[:], xa[:])

            # LN2 -> t (f32r, into slot B), fused with fp32 router matmul
            tb = pbig.tile([P, KD, T], F32R, tag="B", name="tb")
            r_fm = pfC.tile([E, T], F32, tag="v", name="r_fm")
            st2 = [ln_stats(xa, t) for t in range(TC)]
            for t in range(TC):
                mu, rstd = st2[t]
                ln_apply(xa, tb, t, mu, rstd, g2_sb, b2_sb)
                ps = psum_tile()
                for kc in range(KD):
                    rt = pfA.tile([P, TW], F32, tag="rt", name="rt")
                    nc.vector.tensor_tensor(
                        rt[:], xa[:, kc, ts(t, TW)], mu[:],
                        op=OP.subtract)
                    nc.tensor.matmul(
                        ps[:E, :], lhsT=wrg_sb[:, kc, :], rhs=rt[:],
                        start=(kc == 0), stop=(kc == KD - 1))
                nc.vector.tensor_tensor(
                    r_fm[:, ts(t, TW)], ps[:E, :], rstd[:E, :], op=OP.mult)
                nc.vector.tensor_scalar_add(
                    r_fm[:, ts(t, TW)], r_fm[:, ts(t, TW)], breff_sb[:E, 0:1])

            pfB.release()
            pfA.release()

            # softmax + top-8 in token-major
            r_tm = pfC.tile([P, NT, E], F32, tag="rtm")
            for i in range(NT):
                tp = psum_tile()
                nc.tensor.transpose(
                    tp[:, :E], r_fm[:, i * P:(i + 1) * P], ident[:E, :E])
                nc.vector.tensor_copy(r_tm[:, i, :], tp[:, :E])
            m_sb = pfM.tile([P, NT], F32, tag="m", name="m1")
            nc.vector.reduce_max(m_sb[:, :, None], r_tm[:], axis=AX.X)
            nc.vector.tensor_tensor(
                r_tm[:], r_tm[:], m_sb[:, :, None].to_broadcast([P, NT, E]),
                op=OP.subtract)
            nc.scalar.activation(r_tm[:], r_tm[:], AF.Exp)
            s_sb = pfM.tile([P, NT], F32, tag="m", name="m2")
            nc.vector.reduce_sum(s_sb[:, :, None], r_tm[:], axis=AX.X)
            rs_sb = pfM.tile([P, NT], F32, tag="m", name="m3")
            nc.vector.reciprocal(rs_sb[:], s_sb[:])
            nc.vector.tensor_tensor(
                r_tm[:], r_tm[:], rs_sb[:, :, None].to_broadcast([P, NT, E]),
                op=OP.mult)
            work = pmoec.tile([P, NT, E], F32, tag="work")
            msk = pfC.tile([P, NT, E], F32, tag="msk")
            nc.vector.tensor_copy(work[:], r_tm[:])
            thr = pfM.tile([P, NT], F32, tag="m", name="m4")
            for it in range(8):
                nc.vector.reduce_max(thr[:, :, None], work[:], axis=AX.X)
                if it < 7:
                    nc.vector.tensor_tensor(
                        msk[:], work[:], thr[:, :, None].to_broadcast([P, NT, E]),
                        op=OP.is_lt)
                    nc.vector.tensor_tensor(work[:], work[:], msk[:], op=OP.mult)
            nc.vector.tensor_tensor(
                msk[:], r_tm[:], thr[:, :, None].to_broadcast([P, NT, E]),
                op=OP.is_ge)
            nc.vector.tensor_tensor(work[:], r_tm[:], msk[:], op=OP.mult)
            wsum = pfM.tile([P, NT], F32, tag="m", name="m5")
            nc.vector.reduce_sum(wsum[:, :, None], work[:], axis=AX.X)
            rws = pfM.tile([P, NT], F32, tag="m", name="m6")
            nc.vector.reciprocal(rws[:], wsum[:])
            nc.vector.tensor_tensor(
                work[:], work[:], rws[:, :, None].to_broadcast([P, NT, E]),
                op=OP.mult)

        pbigA.release()

        T2 = T // 2

        # ---- phase 6: MoE (4 routed dense + shared slice) ----
        # Weights are host-pretiled fc-major (mm1) / dc-major (mm2) so only
        # the current +prefetch weight tile is live: experts pipeline on the
        # PE with the two z token-half buffers.
        with (
            tc.tile_pool(name="pw", bufs=10) as pw,
            tc.tile_pool(name="pcbc", bufs=1) as pcbc,
            tc.tile_pool(name="pz", bufs=2) as pz,
        ):
            def emit_cbc(e):
                cbc = pcbc.tile([P, T], F32, tag="cbc", name="cbc")
                for t in range(TC):
                    ps = psum_tile()
                    nc.tensor.matmul(
                        ps[:], lhsT=sbc_sb[:, ts(e, P)],
                        rhs=c_fm[:, ts(t, TW)], start=True, stop=True)
                    nc.vector.tensor_copy(cbc[:, ts(t, TW)], ps[:])
                return cbc

            TC2 = TC // 2
            for e in range(ELOC + 1):
                shared = e == ELOC
                cbc = emit_cbc(e) if (not shared and e > 0) else None
                zh = [pz.tile([P, FC, T2], F32R, tag="z", name="zh")
                      for _ in range(2)]
                for half in range(2):
                    for fc in range(FC):
                        w1f = pw.tile([P, KD, P], F32R, tag="w", name="w1f")
                        src_ap = (ws1_d[fc] if shared else we1_d[e, fc])
                        nc.sync.dma_start(
                            w1f[:], src_ap.rearrange("(ko p) m -> p ko m", p=P))
                        bias = (bs1_sb[:, fc:fc + 1] if shared
                                else be1_sb[:, e, fc:fc + 1])
                        for t2 in range(TC2):
                            t = half * TC2 + t2
                            ps = psum_tile()
                            for kc in range(KD):
                                nc.tensor.matmul(
                                    ps[:], lhsT=w1f[:, kc, :],
                                    rhs=tb[:, kc, ts(t, TW)],
                                    start=(kc == 0), stop=(kc == KD - 1))
                            nc.scalar.activation(
                                zh[half][:, fc, ts(t2, TW)], ps[:], AF.Silu,
                                bias=bias)
                            if cbc is not None:
                                nc.vector.tensor_tensor(
                                    zh[half][:, fc, ts(t2, TW)],
                                    zh[half][:, fc, ts(t2, TW)],
                                    cbc[:, ts(t, TW)], op=OP.mult)
                    if e == 0 and half == 0:
                        # combine weights -> expert-major, deferred so
                        # expert 0's first matmuls don't wait on the
                        # top-k DVE chain
                        for i in range(NT):
                            tp = psum_tile()
                            nc.tensor.transpose(
                                tp[:E, :P], work[:, i, :], ident[:])
                            nc.vector.tensor_copy(
                                c_fm[:, i * P:(i + 1) * P], tp[:E, :P])
                        cbc = emit_cbc(0)
                        for t2 in range(TC2):
                            for fc in range(FC):
                                nc.vector.tensor_tensor(
                                    zh[0][:, fc, ts(t2, TW)],
                                    zh[0][:, fc, ts(t2, TW)],
                                    cbc[:, ts(t2, TW)], op=OP.mult)
                # mm2
                if not shared:
                    for dc in range(KD):
                        w2d = pw.tile([P, FC, P], F32R, tag="w", name="w2d")
                        nc.sync.dma_start(
                            w2d[:],
                            we2_d[e, dc].rearrange("(fo p) m -> p fo m", p=P))
                        for t in range(TC):
                            half, t2 = divmod(t, TC2)
                            ps = psum_tile()
                            for fc in range(FC):
                                nc.tensor.matmul(
                                    ps[:], lhsT=w2d[:, fc, :],
                                    rhs=zh[half][:, fc, ts(t2, TW)],
                                    start=(fc == 0), stop=(fc == FC - 1))
                            stg = pstg.tile([P, TW], F32, tag="stg", name="stg")
                            nc.scalar.activation(stg[:], ps[:], AF.Copy)
                            nc.gpsimd.dma_start(
                                acc_h[half][:, dc, ts(t2, TW)], stg[:],
                                accum_op=(OP.bypass if e == 0 else OP.add))
                else:
                    # shared expert last: t-outer with resident w2 tiles so
                    # each all-reduce half fires as soon as it is complete
                    w2ds = []
                    for dc in range(KD):
                        w2d = pw.tile([P, FC, P], F32R, tag="w", name="w2ds")
                        nc.sync.dma_start(
                            w2d[:],
                            ws2_d[dc].rearrange("(fo p) m -> p fo m", p=P))
                        w2ds.append(w2d)
                    for t in range(TC):
                        half, t2 = divmod(t, TC2)
                        for dc in range(KD):
                            ps = psum_tile()
                            for fc in range(FC):
                                nc.tensor.matmul(
                                    ps[:], lhsT=w2ds[dc][:, fc, :],
                                    rhs=zh[half][:, fc, ts(t2, TW)],
                                    start=(fc == 0), stop=False)
                            nc.tensor.matmul(
                                ps[:], lhsT=be2_sb[:, dc, :],
                                rhs=c_fm[:, ts(t, TW)],
                                start=False, stop=True)
                            stg = pstg.tile([P, TW], F32, tag="stg", name="stg")
                            nc.scalar.activation(stg[:], ps[:], AF.Copy)
                            nc.gpsimd.dma_start(
                                acc_h[half][:, dc, ts(t2, TW)], stg[:],
                                accum_op=OP.add)
                        if t % TC2 == TC2 - 1:
                            nc.gpsimd.collective_compute(
                                "AllReduce",
                                OP.add,
                                replica_groups=[list(range(NCORES))],
                                ins=[acc_h[half][:].opt()],
                                outs=[red_h[half][:].opt()],
                            )
        pmoec.release()

        # ---- phase 7: AllReduce; x2 = x1 + red + bs2sum; out projection ----
        with (
            tc.tile_pool(name="pxb", bufs=3) as pxb,
            tc.tile_pool(name="pwout", bufs=12) as pwout,
        ):
            x2 = pbig.tile([P, KD, T], F32R, tag="B")
            # two half-passes over tokens: the first half only needs the
            # first two all-reduced chunks, so its projection overlaps the
            # later all-reduces (and the engines stay in-order-clean)
            for half in range(2):
                for t2 in range(TC // 2):
                    t = half * (TC // 2) + t2
                    for kc in range(KD):
                        xb = pxb.tile([P, TW], F32, tag="xb", name="xb")
                        nc.sync.dma_start(xb[:], x1_dram[:, kc, ts(t, TW)])
                        rb = pxb.tile([P, TW], F32, tag="rb", name="rb")
                        nc.sync.dma_start(rb[:], red_h[half][:, kc, ts(t2, TW)])
                        nc.vector.tensor_add(out=xb[:], in0=xb[:], in1=rb[:])
                        nc.vector.tensor_scalar_add(
                            x2[:, kc, ts(t, TW)], xb[:], bs2_sb[:, kc:kc + 1])
                for n in range(NCH):
                    bout_sb = pxb.tile([P, TW], F32, tag="bout", name="bout")
                    nc.sync.dma_start(bout_sb[:], bout_d[:, ts(n, TW)])
                    wot = []
                    for kc in range(KD):
                        wt = pwout.tile([P, TW], F32R, tag="wo", name="wo")
                        nc.sync.dma_start(wt[:], wout_d[n, ts(kc, P), :])
                        wot.append(wt)
                    for m in range(half * NT // 2, (half + 1) * NT // 2):
                        ps = psum_tile()
                        for kc in range(KD):
                            nc.tensor.matmul(
                                ps[:], lhsT=x2[:, kc, ts(m, P)], rhs=wot[kc][:],
                                start=(kc == 0), stop=(kc == KD - 1))
                        stg = pstg.tile([P, TW], F32, tag="stg", name="stg")
                        nc.vector.tensor_add(
                            out=stg[:], in0=ps[:], in1=bout_sb[:])
                        nc.sync.dma_start(logits_d[ts(m, P), ts(n, TW)], stg[:])

        for _pool in (pdram, pstg, ppsum, pbig, pconst):
            _pool.release()

    nc.compile()
    return nc


def _get_nc():
    if "nc" not in _NC_CACHE:
        _NC_CACHE["nc"] = _build_nc()
    return _NC_CACHE["nc"]


def _prep_in_maps(inputs):
    inp = {k: np.asarray(v) for k, v in inputs.items()}
    f32 = np.float32

    ids = np.ascontiguousarray(inp["input_ids"].reshape(T, 1).astype(np.int32))
    emb = np.ascontiguousarray(inp["emb"].astype(f32))
    WoS = np.ascontiguousarray(
        inp["Wo"].astype(f32).reshape(H, R, D).sum(0).astype(f32))
    g2 = inp["g2"].astype(f32)
    Wrg = np.ascontiguousarray((g2[:, None] * inp["Wr"].astype(f32)).astype(f32))
    breff = (inp["br"].astype(f32)
             + inp["beta2"].astype(f32) @ inp["Wr"].astype(f32))
    breff = np.ascontiguousarray(breff.reshape(E, 1).astype(f32))

    common = {
        "ids": ids, "emb": emb,
        "ones128": np.ones((P, P), f32),
        "Wv": np.ascontiguousarray(inp["Wv"].astype(f32)),
        "WoS": WoS, "Wrg": Wrg, "breff": breff,
        "g1v": inp["g1"].astype(f32), "b1v": inp["beta1"].astype(f32),
        "g2v": g2, "b2v": inp["beta2"].astype(f32),
        "bs2S": np.ascontiguousarray(inp["bs2"].astype(f32).sum(0)),
    }

    We1 = inp["We1"].astype(f32)
    be1 = inp["be1"].astype(f32)
    We2 = inp["We2"].astype(f32)
    be2 = inp["be2"].astype(f32)
    Ws1 = inp["Ws1"].astype(f32)
    bs1 = inp["bs1"].astype(f32)
    Ws2 = inp["Ws2"].astype(f32)
    Wout = inp["Wout"].astype(f32)
    bout = inp["bout"].astype(f32)

    in_maps = []
    for c in range(NCORES):
        el = list(range(ELOC * c, ELOC * (c + 1)))
        s, q = divmod(c, NCORES // NS)
        isl = slice(q * ILOC, (q + 1) * ILOC)
        Sbc = np.zeros((E, ELOC * P), f32)
        for j, e in enumerate(el):
            Sbc[e, j * P:(j + 1) * P] = 1.0
        wout_pad = np.zeros((D, VPAD), f32)
        wout_pad[:, :VLOC] = Wout[:, VLOC * c:VLOC * (c + 1)]
        woutL = np.ascontiguousarray(
            wout_pad.reshape(D, NCH, TW).transpose(1, 0, 2))
        bout_pad = np.zeros((VPAD,), f32)
        bout_pad[:VLOC] = bout[VLOC * c:VLOC * (c + 1)]
        boutBC = np.ascontiguousarray(np.broadcast_to(bout_pad, (P, VPAD)))
        m = dict(common)
        be2P = np.zeros((E, D), f32)
        be2P[el] = be2[el]
        m.update({
            "We1L": np.ascontiguousarray(
                We1[el].reshape(ELOC, D, FC, P).transpose(0, 2, 1, 3)),
            "be1L": np.ascontiguousarray(be1[el]),
            "We2L": np.ascontiguousarray(
                We2[el].reshape(ELOC, F, KD, P).transpose(0, 2, 1, 3)),
            "be2P": be2P,
            "Ws1L": np.ascontiguousarray(
                Ws1[s][:, isl].reshape(D, FC, P).transpose(1, 0, 2)),
            "bs1L": np.ascontiguousarray(bs1[s][isl]),
            "Ws2L": np.ascontiguousarray(
                Ws2[s][isl, :].reshape(ILOC, KD, P).transpose(1, 0, 2)),
            "Sbc": Sbc,
            "WoutL": woutL, "boutBC": boutBC,
        })
        in_maps.append(m)
    return in_maps


def kernel(**inputs):
    in_maps = _prep_in_maps(inputs)
    nc = _get_nc()
    r = run_bass_kernel_spmd(nc, in_maps, list(range(NCORES)))
    logits = np.concatenate(
        [r.results[c]["logits"][:, :VLOC] for c in range(NCORES)], axis=1)
    return np.ascontiguousarray(logits.reshape(B, S, V).astype(np.float32))


if __name__ == "__main__":
    _build_nc()
    print("build + compile OK")


# revision 23
# speedup vs baseline: 1.0978x; 1.0159x over previous
"""Trainium2 Bass kernel for nn_BeyazKusAIEnhanced (moe_routing).

Model (T=2048 tokens, D=1024):
  x = emb[ids]
  h = LN1(x); attention collapses exactly to: ao = (h @ Wv) @ WoSum
    (softmax over a size-1 axis is exactly 1, so out = tile(v, 16 heads)
     and out @ Wo == v @ WoSum with WoSum[r,:] = sum_h Wo[h*64+r, :])
  x1 = x + ao
  t = LN2(x1); router probs = softmax(t @ Wr + br); top-8 -> combine [T,32]
  moe = sum_e combine[:,e] * (silu(t@We1[e]+be1[e]) @ We2[e] + be2[e])
  shared = sum_s silu(t@Ws1[s]+bs1[s]) @ Ws2[s] + bs2[s]
  out = (x1 + moe + shared) @ Wout + bout        [T, 32000]

Sharding (8 cores):
  - front part (gather/LN/attn/router) replicated on all cores
  - routed experts: 4 per core (dense compute; combine weights of
    non-selected experts are exactly 0, so dense == sparse w/ weights)
  - shared experts: inter dim (2*4096 = 8192) split 1024 per core;
    bs2 biases summed on host and added post-allreduce on every core
  - partial (moe+shared) accumulated in DRAM via accum-DMA, AllReduce'd
    across cores; x2 = x1 + reduced + bs2sum
  - output projection vocab-split: 4000 cols/core (padded to 4096)

Layout: activations feature-major [128 part, 8 kchunk, 2048 tok] in SBUF;
matmuls fp32r (full PE rate at moving free dim >= 256, ~1e-4 rel err).
LN stats via all-ones [128,128] matmul (partition-broadcast sums, no
explicit broadcast step); per-core expert selection via one-hot inputs.
Router runs in plain fp32 from x1 with LN folded (host folds g2 into Wr
and beta2@Wr into br) so top-8 selection is as close to the f32
reference as possible.
"""

import numpy as np

import concourse.bass as bass
import concourse.mybir as mybir
import concourse.tile as tile
from concourse import bacc
from concourse.bass import ts
from concourse.bass_utils import run_bass_kernel_spmd
from concourse.masks import make_identity

P = 128
B, S = 2, 1024
T = 2048          # tokens
D = 1024          # model dim
KD = D // P       # 8 k-chunks
H = 16            # heads
R = 64            # kv rank / head dim
E = 32            # routed experts
ELOC = 4          # experts per core
F = 1024          # moe inter dim
FC = F // P       # 8
NS = 2            # shared experts
ILOC = 1024       # shared inter slice per core
V = 32000
VLOC = 4000       # real vocab cols per core
VPAD = 4096       # padded to 8 x 512
NCH = VPAD // 512
TC = 4            # token chunks
TW = 512          # token chunk width
NT = T // P       # 16 token tiles
EPS = 1e-5
NCORES = 8

F32 = mybir.dt.float32
F32R = mybir.dt.float32r
I32 = mybir.dt.int32
AF = mybir.ActivationFunctionType
OP = mybir.AluOpType
AX = mybir.AxisListType

_NC_CACHE = {}


def _build_nc():
    nc = bacc.Bacc(None)

    ids_d = nc.declare_dram_parameter("ids", [T, 1], I32, isOutput=False)
    emb_d = nc.declare_dram_parameter("emb", [V, D], F32, isOutput=False)
    ones_d = nc.declare_dram_parameter("ones128", [P, P], F32R, isOutput=False)
    wv_d = nc.declare_dram_parameter("Wv", [D, R], F32, isOutput=False)
    wos_d = nc.declare_dram_parameter("WoS", [R, D], F32, isOutput=False)
    wrg_d = nc.declare_dram_parameter("Wrg", [D, E], F32, isOutput=False)
    breff_d = nc.declare_dram_parameter("breff", [E, 1], F32, isOutput=False)
    g1_d = nc.declare_dram_parameter("g1v", [D], F32, isOutput=False)
    b1_d = nc.declare_dram_parameter("b1v", [D], F32, isOutput=False)
    g2_d = nc.declare_dram_parameter("g2v", [D], F32, isOutput=False)
    b2_d = nc.declare_dram_parameter("b2v", [D], F32, isOutput=False)
    we1_d = nc.declare_dram_parameter("We1L", [ELOC, FC, D, P], F32R,
                                      isOutput=False)
    be1_d = nc.declare_dram_parameter("be1L", [ELOC, F], F32, isOutput=False)
    we2_d = nc.declare_dram_parameter("We2L", [ELOC, KD, F, P], F32R,
                                      isOutput=False)
    be2_d = nc.declare_dram_parameter("be2P", [E, D], F32R, isOutput=False)
    ws1_d = nc.declare_dram_parameter("Ws1L", [FC, D, P], F32R, isOutput=False)
    bs1_d = nc.declare_dram_parameter("bs1L", [ILOC], F32, isOutput=False)
    ws2_d = nc.declare_dram_parameter("Ws2L", [KD, ILOC, P], F32R, isOutput=False)
    bs2_d = nc.declare_dram_parameter("bs2S", [D], F32, isOutput=False)
    sbc_d = nc.declare_dram_parameter("Sbc", [E, ELOC * P], F32R, isOutput=False)
    wout_d = nc.declare_dram_parameter("WoutL", [NCH, D, TW], F32R, isOutput=False)
    bout_d = nc.declare_dram_parameter("boutBC", [P, VPAD], F32, isOutput=False)
    logits_d = nc.declare_dram_parameter("logits", [T, VPAD], F32, isOutput=True)

    with tile.TileContext(nc) as tc:
        pconst = tc.alloc_tile_pool(name="pconst", bufs=1)
        pbig = tc.alloc_tile_pool(name="pbig", bufs=1)
        ppsum = tc.alloc_tile_pool(name="ppsum", bufs=6, space="PSUM")
        pstg = tc.alloc_tile_pool(name="pstg", bufs=3)
        pdram = tc.alloc_tile_pool(name="pdram", bufs=1, space="DRAM")

        def psum_tile():
            return ppsum.tile([P, TW], F32, tag="ps", name="ps", space="PSUM")

        # ---- small constants (~8.6 KB/partition) ----
        ident = pconst.tile([P, P], F32)
        make_identity(nc, ident[:])
        ones_sb = pconst.tile([P, P], F32R)
        nc.sync.dma_start(ones_sb[:], ones_d[:, :])
        wv_sb = pconst.tile([P, KD, R], F32)
        nc.sync.dma_start(wv_sb[:], wv_d.rearrange("(ko p) r -> p ko r", p=P))
        wos_sb = pconst.tile([R, KD, P], F32)
        nc.sync.dma_start(wos_sb[:], wos_d.rearrange("r (ko p) -> r ko p", p=P))
        wrg_sb = pconst.tile([P, KD, E], F32)
        nc.sync.dma_start(wrg_sb[:], wrg_d.rearrange("(ko p) e -> p ko e", p=P))
        breff_sb = pconst.tile([E, 1], F32)
        nc.sync.dma_start(breff_sb[:], breff_d[:, :])
        g1_sb = pconst.tile([P, KD], F32)
        nc.sync.dma_start(g1_sb[:], g1_d.rearrange("(ko p) -> p ko", p=P))
        b1_sb = pconst.tile([P, KD], F32)
        nc.sync.dma_start(b1_sb[:], b1_d.rearrange("(ko p) -> p ko", p=P))
        g2_sb = pconst.tile([P, KD], F32)
        nc.sync.dma_start(g2_sb[:], g2_d.rearrange("(ko p) -> p ko", p=P))
        b2_sb = pconst.tile([P, KD], F32)
        nc.sync.dma_start(b2_sb[:], b2_d.rearrange("(ko p) -> p ko", p=P))
        be1_sb = pconst.tile([P, ELOC, FC], F32)
        nc.sync.dma_start(be1_sb[:], be1_d.rearrange("e (ko p) -> p e ko", p=P))
        bs1_sb = pconst.tile([P, FC], F32)
        nc.sync.dma_start(bs1_sb[:], bs1_d.rearrange("(ko p) -> p ko", p=P))
        bs2_sb = pconst.tile([P, KD], F32)
        nc.sync.dma_start(bs2_sb[:], bs2_d.rearrange("(ko p) -> p ko", p=P))
        eps_sb = pconst.tile([P, 1], F32)
        nc.gpsimd.memset(eps_sb[:], EPS)

        # DRAM scratch
        x1_dram = pdram.tile([P, KD, T], F32, tag="x1d")
        acc_h = [pdram.tile([P, KD, T // 2], F32, tag=f"acc{h}", name=f"acc{h}")
                 for h in range(2)]
        red_h = [pdram.tile([P, KD, T // 2], F32, tag=f"red{h}", name=f"red{h}",
                            addr_space="Shared")
                 for h in range(2)]

        # combine-weight tiles + MoE selection constants (outlive front pools)
        pmoec = tc.alloc_tile_pool(name="pmoec", bufs=1)
        c_fm = pmoec.tile([E, T], F32R, tag="cfm")
        sbc_sb = pmoec.tile([E, ELOC * P], F32R, tag="sbc")
        nc.sync.dma_start(sbc_sb[:], sbc_d[:, :])
        be2_sb = pmoec.tile([E, KD, P], F32R, tag="be2")
        nc.sync.dma_start(be2_sb[:], be2_d.rearrange("e (ko p) -> e ko p", p=P))

        pbigA = tc.alloc_tile_pool(name="pbigA", bufs=1)
        xa = pbigA.tile([P, KD, T], F32, tag="A")  # x, then x1 (in place)
        hb = pbig.tile([P, KD, T], F32, tag="B")  # h (fp32, feeds attention)

        # ---- phase 1: embedding gather + PE transpose to feature-major ----
        with (
            tc.tile_pool(name="pgather", bufs=4) as pgather,
            tc.tile_pool(name="pidx", bufs=NT) as pidx,
        ):
            idxs = []
            for i in range(NT):
                idx_sb = pidx.tile([P, 1], I32, tag="idx", name="idx")
                nc.sync.dma_start(idx_sb[:], ids_d[i * P:(i + 1) * P, :])
                idxs.append(idx_sb)
            for i in range(NT):
                idx_sb = idxs[i]
                gx = pgather.tile([P, D], F32, tag="gx", name="gx")
                nc.gpsimd.indirect_dma_start(
                    out=gx[:],
                    out_offset=None,
                    in_=emb_d[:, :],
                    in_offset=bass.IndirectOffsetOnAxis(ap=idx_sb[:, :1], axis=0),
                )
                for kc in range(KD):
                    tp = psum_tile()
                    nc.tensor.transpose(tp[:, :P], gx[:, ts(kc, P)], ident[:])
                    nc.vector.tensor_copy(xa[:, kc, i * P:(i + 1) * P], tp[:, :P])

        # ---- phases 2-5 (LN1, attention, LN2+router fused) ----
        with (
            tc.tile_pool(name="pfC", bufs=1) as pfC,
            tc.tile_pool(name="pfM", bufs=2) as pfM,
        ):
            pfA = tc.alloc_tile_pool(name="pfA", bufs=2)
            pfB = tc.alloc_tile_pool(name="pfB", bufs=2 * TC)

            def ln_stats(src, t):
                """LN stats for token chunk t -> (mu, rstd) tiles [P, TW]
                (every partition holds the same per-token row)."""
                ps_mu = psum_tile()
                ps_sq = psum_tile()
                for kc in range(KD):
                    xr = pfA.tile([P, TW], F32R, tag="sq", name="xr")
                    nc.vector.tensor_copy(xr[:], src[:, kc, ts(t, TW)])
                    nc.tensor.matmul(
                        ps_mu[:], lhsT=ones_sb[:], rhs=xr[:],
                        start=(kc == 0), stop=(kc == KD - 1))
                    sq = pfA.tile([P, TW], F32R, tag="sq", name="sq")
                    nc.scalar.activation(sq[:], src[:, kc, ts(t, TW)], AF.Square)
                    nc.tensor.matmul(
                        ps_sq[:], lhsT=ones_sb[:], rhs=sq[:],
                        start=(kc == 0), stop=(kc == KD - 1))
                mu = pfB.tile([P, TW], F32, tag="bc", name="mu")
                nc.vector.tensor_scalar_mul(mu[:], ps_mu[:], 1.0 / D)
                msq = pfA.tile([P, TW], F32, tag="lntmp", name="msq")
                nc.vector.tensor_scalar_mul(msq[:], ps_sq[:], 1.0 / D)
                mu2 = pfA.tile([P, TW], F32, tag="lntmp", name="mu2")
                nc.vector.tensor_mul(out=mu2[:], in0=mu[:], in1=mu[:])
                nc.vector.tensor_tensor(msq[:], msq[:], mu2[:], op=OP.subtract)
                nc.scalar.activation(msq[:], msq[:], AF.Sqrt, bias=eps_sb[:, 0:1])
                rstd = pfB.tile([P, TW], F32, tag="bc", name="rstd")
                nc.vector.reciprocal(rstd[:], msq[:])
                return mu, rstd

            def ln_apply(src, dst, t, mu, rstd, g_sb, b_sb):
                for kc in range(KD):
                    nc.vector.tensor_tensor(
                        dst[:, kc, ts(t, TW)], src[:, kc, ts(t, TW)], mu[:],
                        op=OP.subtract)
                    nc.vector.tensor_tensor(
                        dst[:, kc, ts(t, TW)], dst[:, kc, ts(t, TW)], rstd[:],
                        op=OP.mult)
                    nc.vector.tensor_scalar(
                        dst[:, kc, ts(t, TW)], dst[:, kc, ts(t, TW)],
                        g_sb[:, kc:kc + 1], b_sb[:, kc:kc + 1],
                        op0=OP.mult, op1=OP.add)

            # LN1 -> h (stats for all chunks first, applies after: keeps
            # the PE stats matmuls from waiting behind DVE apply chains)
            st1 = [ln_stats(xa, t) for t in range(TC)]
            for t in range(TC):
                ln_apply(xa, hb, t, st1[t][0], st1[t][1], g1_sb, b1_sb)

            # v = h @ Wv  [R, T]
            v_sb = pfC.tile([R, T], F32, tag="v")
            for t in range(TC):
                ps = psum_tile()
                for kc in range(KD):
                    nc.tensor.matmul(
                        ps[:R, :], lhsT=wv_sb[:, kc, :], rhs=hb[:, kc, ts(t, TW)],
                        start=(kc == 0), stop=(kc == KD - 1))
                nc.vector.tensor_copy(v_sb[:, ts(t, TW)], ps[:R, :])
            # x1 = x + v @ WoSum  (in place into xa)
            for dc in range(KD):
                for t in range(TC):
                    ps = psum_tile()
                    nc.tensor.matmul(
                        ps[:], lhsT=wos_sb[:, dc, :], rhs=v_sb[:, ts(t, TW)],
                        start=True, stop=True)
                    nc.vector.tensor_add(
                        out=xa[:, dc, ts(t, TW)], in0=xa[:, dc, ts(t, TW)],
                        in1=ps[:])
            nc.sync.dma_start(x1_dram[:], xa[:])

            # LN2 -> t (f32r, into slot B), fused with fp32 router matmul
            tb = pbig.tile([P, KD, T], F32R, tag="B", name="tb")
            r_fm = pfC.tile([E, T], F32, tag="v", name="r_fm")
            st2 = [ln_stats(xa, t) for t in range(TC)]
            for t in range(TC):
                mu, rstd = st2[t]
                ln_apply(xa, tb, t, mu, rstd, g2_sb, b2_sb)
                ps = psum_tile()
                for kc in range(KD):
                    rt = pfA.tile([P, TW], F32, tag="rt", name="rt")
                    nc.vector.tensor_tensor(
                        rt[:], xa[:, kc, ts(t, TW)], mu[:],
                        op=OP.subtract)
                    nc.tensor.matmul(
                        ps[:E, :], lhsT=wrg_sb[:, kc, :], rhs=rt[:],
                        start=(kc == 0), stop=(kc == KD - 1))
                nc.vector.tensor_tensor(
                    r_fm[:, ts(t, TW)], ps[:E, :], rstd[:E, :], op=OP.mult)
                nc.vector.tensor_scalar_add(
                    r_fm[:, ts(t, TW)], r_fm[:, ts(t, TW)], breff_sb[:E, 0:1])

            pfB.release()
            pfA.release()

            # softmax + top-8 in token-major
            r_tm = pfC.tile([P, NT, E], F32, tag="rtm")
            for i in range(NT):
                tp = psum_tile()
                nc.tensor.transpose(
                    tp[:, :E], r_fm[:, i * P:(i + 1) * P], ident[:E, :E])
                nc.vector.tensor_copy(r_tm[:, i, :], tp[:, :E])
            m_sb = pfM.tile([P, NT], F32, tag="m", name="m1")
            nc.vector.reduce_max(m_sb[:, :, None], r_tm[:], axis=AX.X)
            nc.vector.tensor_tensor(
                r_tm[:], r_tm[:], m_sb[:, :, None].to_broadcast([P, NT, E]),
                op=OP.subtract)
            nc.scalar.activation(r_tm[:], r_tm[:], AF.Exp)
            s_sb = pfM.tile([P, NT], F32, tag="m", name="m2")
            nc.vector.reduce_sum(s_sb[:, :, None], r_tm[:], axis=AX.X)
            rs_sb = pfM.tile([P, NT], F32, tag="m", name="m3")
            nc.vector.reciprocal(rs_sb[:], s_sb[:])
            nc.vector.tensor_tensor(
                r_tm[:], r_tm[:], rs_sb[:, :, None].to_broadcast([P, NT, E]),
                op=OP.mult)
            work = pmoec.tile([P, NT, E], F32, tag="work")
            msk = pfC.tile([P, NT, E], F32, tag="msk")
            nc.vector.tensor_copy(work[:], r_tm[:])
            thr = pfM.tile([P, NT], F32, tag="m", name="m4")
            for it in range(8):
                nc.vector.reduce_max(thr[:, :, None], work[:], axis=AX.X)
                if it < 7:
                    nc.vector.tensor_tensor(
                        msk[:], work[:], thr[:, :, None].to_broadcast([P, NT, E]),
                        op=OP.is_lt)
                    nc.vector.tensor_tensor(work[:], work[:], msk[:], op=OP.mult)
            nc.vector.tensor_tensor(
                msk[:], r_tm[:], thr[:, :, None].to_broadcast([P, NT, E]),
                op=OP.is_ge)
            nc.vector.tensor_tensor(work[:], r_tm[:], msk[:], op=OP.mult)
            wsum = pfM.tile([P, NT], F32, tag="m", name="m5")
            nc.vector.reduce_sum(wsum[:, :, None], work[:], axis=AX.X)
            rws = pfM.tile([P, NT], F32, tag="m", name="m6")
            nc.vector.reciprocal(rws[:], wsum[:])
            nc.vector.tensor_tensor(
                work[:], work[:], rws[:, :, None].to_broadcast([P, NT, E]),
                op=OP.mult)

        pbigA.release()

        T2 = T // 2

        # ---- phase 6: MoE (4 routed dense + shared slice) ----
        # Token-half-outer over the whole expert set: the first half\'s
        # partial sum is complete mid-phase, so its all-reduce runs under
        # the second half\'s compute and the output projection starts at
        # phase end with no collective exposure.
        with (
            tc.tile_pool(name="pw", bufs=10) as pw,
            tc.tile_pool(name="pcbc", bufs=2) as pcbc,
            tc.tile_pool(name="pz", bufs=2) as pz,
        ):
            TC2 = TC // 2

            def emit_cbc(e, half):
                cbc = pcbc.tile([P, T2], F32, tag="cbc", name="cbc")
                for t2 in range(TC2):
                    t = half * TC2 + t2
                    ps = psum_tile()
                    nc.tensor.matmul(
                        ps[:], lhsT=sbc_sb[:, ts(e, P)],
                        rhs=c_fm[:, ts(t, TW)], start=True, stop=True)
                    nc.vector.tensor_copy(cbc[:, ts(t2, TW)], ps[:])
                return cbc

            for half in range(2):
                for e in range(ELOC + 1):
                    shared = e == ELOC
                    first = e == 0 and half == 0
                    cbc = (None if shared or first else emit_cbc(e, half))
                    zh = pz.tile([P, FC, T2], F32R, tag="z", name="zh")
                    for fc in range(FC):
                        w1f = pw.tile([P, KD, P], F32R, tag="w", name="w1f")
                        src_ap = (ws1_d[fc] if shared else we1_d[e, fc])
                        nc.sync.dma_start(
                            w1f[:], src_ap.rearrange("(ko p) m -> p ko m", p=P))
                        bias = (bs1_sb[:, fc:fc + 1] if shared
                                else be1_sb[:, e, fc:fc + 1])
                        for t2 in range(TC2):
                            t = half * TC2 + t2
                            ps = psum_tile()
                            for kc in range(KD):
                                nc.tensor.matmul(
                                    ps[:], lhsT=w1f[:, kc, :],
                                    rhs=tb[:, kc, ts(t, TW)],
                                    start=(kc == 0), stop=(kc == KD - 1))
                            nc.scalar.activation(
                                zh[:, fc, ts(t2, TW)], ps[:], AF.Silu,
                                bias=bias)
                            if cbc is not None:
                                nc.vector.tensor_tensor(
                                    zh[:, fc, ts(t2, TW)],
                                    zh[:, fc, ts(t2, TW)],
                                    cbc[:, ts(t2, TW)], op=OP.mult)
                    if first:
                        # combine weights -> expert-major, deferred so
                        # expert 0\'s first matmuls don\'t wait on the
                        # top-k DVE chain; then scale its z after the fact
                        for i in range(NT):
                            tp = psum_tile()
                            nc.tensor.transpose(
                                tp[:E, :P], work[:, i, :], ident[:])
                            nc.vector.tensor_copy(
                                c_fm[:, i * P:(i + 1) * P], tp[:E, :P])
                        cbc = emit_cbc(0, 0)
                        for t2 in range(TC2):
                            for fc in range(FC):
                                nc.vector.tensor_tensor(
                                    zh[:, fc, ts(t2, TW)],
                                    zh[:, fc, ts(t2, TW)],
                                    cbc[:, ts(t2, TW)], op=OP.mult)
                    # mm2 (dc-major streamed weights)
                    for dc in range(KD):
                        w2d = pw.tile([P, FC, P], F32R, tag="w", name="w2d")
                        src_ap = (ws2_d[dc] if shared else we2_d[e, dc])
                        nc.sync.dma_start(
                            w2d[:], src_ap.rearrange("(fo p) m -> p fo m", p=P))
                        for t2 in range(TC2):
                            t = half * TC2 + t2
                            ps = psum_tile()
                            for fc in range(FC):
                                nc.tensor.matmul(
                                    ps[:], lhsT=w2d[:, fc, :],
                                    rhs=zh[:, fc, ts(t2, TW)],
                                    start=(fc == 0),
                                    stop=(fc == FC - 1 and not shared))
                            if shared:
                                nc.tensor.matmul(
                                    ps[:], lhsT=be2_sb[:, dc, :],
                                    rhs=c_fm[:, ts(t, TW)],
                                    start=False, stop=True)
                            stg = pstg.tile([P, TW], F32, tag="stg", name="stg")
                            nc.scalar.activation(stg[:], ps[:], AF.Copy)
                            nc.gpsimd.dma_start(
                                acc_h[half][:, dc, ts(t2, TW)], stg[:],
                                accum_op=(OP.bypass if e == 0 else OP.add))
                # this token half\'s partial is complete on this core
                nc.gpsimd.collective_compute(
                    "AllReduce",
                    OP.add,
                    replica_groups=[list(range(NCORES))],
                    ins=[acc_h[half][:].opt()],
                    outs=[red_h[half][:].opt()],
                )
        pmoec.release()

        # ---- phase 7: AllReduce; x2 = x1 + red + bs2sum; out projection ----
        with (
            tc.tile_pool(name="pxb", bufs=3) as pxb,
            tc.tile_pool(name="pwout", bufs=12) as pwout,
        ):
            x2 = pbig.tile([P, KD, T], F32R, tag="B")
            # two half-passes over tokens: the first half only needs the
            # first two all-reduced chunks, so its projection overlaps the
            # later all-reduces (and the engines stay in-order-clean)
            for half in range(2):
                for t2 in range(TC // 2):
                    t = half * (TC // 2) + t2
                    for kc in range(KD):
                        xb = pxb.tile([P, TW], F32, tag="xb", name="xb")
                        nc.sync.dma_start(xb[:], x1_dram[:, kc, ts(t, TW)])
                        rb = pxb.tile([P, TW], F32, tag="rb", name="rb")
                        nc.sync.dma_start(rb[:], red_h[half][:, kc, ts(t2, TW)])
                        nc.vector.tensor_add(out=xb[:], in0=xb[:], in1=rb[:])
                        nc.vector.tensor_scalar_add(
                            x2[:, kc, ts(t, TW)], xb[:], bs2_sb[:, kc:kc + 1])
                for n in range(NCH):
                    bout_sb = pxb.tile([P, TW], F32, tag="bout", name="bout")
                    nc.sync.dma_start(bout_sb[:], bout_d[:, ts(n, TW)])
                    wot = []
                    for kc in range(KD):
                        wt = pwout.tile([P, TW], F32R, tag="wo", name="wo")
                        nc.sync.dma_start(wt[:], wout_d[n, ts(kc, P), :])
                        wot.append(wt)
                    for m in range(half * NT // 2, (half + 1) * NT // 2):
                        ps = psum_tile()
                        for kc in range(KD):
                            nc.tensor.matmul(
                                ps[:], lhsT=x2[:, kc, ts(m, P)], rhs=wot[kc][:],
                                start=(kc == 0), stop=(kc == KD - 1))
                        stg = pstg.tile([P, TW], F32, tag="stg", name="stg")
                        nc.vector.tensor_add(
                            out=stg[:], in0=ps[:], in1=bout_sb[:])
                        nc.sync.dma_start(logits_d[ts(m, P), ts(n, TW)], stg[:])

        for _pool in (pdram, pstg, ppsum, pbig, pconst):
            _pool.release()

    nc.compile()
    return nc


def _get_nc():
    if "nc" not in _NC_CACHE:
        _NC_CACHE["nc"] = _build_nc()
    return _NC_CACHE["nc"]


def _prep_in_maps(inputs):
    inp = {k: np.asarray(v) for k, v in inputs.items()}
    f32 = np.float32

    ids = np.ascontiguousarray(inp["input_ids"].reshape(T, 1).astype(np.int32))
    emb = np.ascontiguousarray(inp["emb"].astype(f32))
    WoS = np.ascontiguousarray(
        inp["Wo"].astype(f32).reshape(H, R, D).sum(0).astype(f32))
    g2 = inp["g2"].astype(f32)
    Wrg = np.ascontiguousarray((g2[:, None] * inp["Wr"].astype(f32)).astype(f32))
    breff = (inp["br"].astype(f32)
             + inp["beta2"].astype(f32) @ inp["Wr"].astype(f32))
    breff = np.ascontiguousarray(breff.reshape(E, 1).astype(f32))

    common = {
        "ids": ids, "emb": emb,
        "ones128": np.ones((P, P), f32),
        "Wv": np.ascontiguousarray(inp["Wv"].astype(f32)),
        "WoS": WoS, "Wrg": Wrg, "breff": breff,
        "g1v": inp["g1"].astype(f32), "b1v": inp["beta1"].astype(f32),
        "g2v": g2, "b2v": inp["beta2"].astype(f32),
        "bs2S": np.ascontiguousarray(inp["bs2"].astype(f32).sum(0)),
    }

    We1 = inp["We1"].astype(f32)
    be1 = inp["be1"].astype(f32)
    We2 = inp["We2"].astype(f32)
    be2 = inp["be2"].astype(f32)
    Ws1 = inp["Ws1"].astype(f32)
    bs1 = inp["bs1"].astype(f32)
    Ws2 = inp["Ws2"].astype(f32)
    Wout = inp["Wout"].astype(f32)
    bout = inp["bout"].astype(f32)

    in_maps = []
    for c in range(NCORES):
        el = list(range(ELOC * c, ELOC * (c + 1)))
        s, q = divmod(c, NCORES // NS)
        isl = slice(q * ILOC, (q + 1) * ILOC)
        Sbc = np.zeros((E, ELOC * P), f32)
        for j, e in enumerate(el):
            Sbc[e, j * P:(j + 1) * P] = 1.0
        wout_pad = np.zeros((D, VPAD), f32)
        wout_pad[:, :VLOC] = Wout[:, VLOC * c:VLOC * (c + 1)]
        woutL = np.ascontiguousarray(
            wout_pad.reshape(D, NCH, TW).transpose(1, 0, 2))
        bout_pad = np.zeros((VPAD,), f32)
        bout_pad[:VLOC] = bout[VLOC * c:VLOC * (c + 1)]
        boutBC = np.ascontiguousarray(np.broadcast_to(bout_pad, (P, VPAD)))
        m = dict(common)
        be2P = np.zeros((E, D), f32)
        be2P[el] = be2[el]
        m.update({
            "We1L": np.ascontiguousarray(
                We1[el].reshape(ELOC, D, FC, P).transpose(0, 2, 1, 3)),
            "be1L": np.ascontiguousarray(be1[el]),
            "We2L": np.ascontiguousarray(
                We2[el].reshape(ELOC, F, KD, P).transpose(0, 2, 1, 3)),
            "be2P": be2P,
            "Ws1L": np.ascontiguousarray(
                Ws1[s][:, isl].reshape(D, FC, P).transpose(1, 0, 2)),
            "bs1L": np.ascontiguousarray(bs1[s][isl]),
            "Ws2L": np.ascontiguousarray(
                Ws2[s][isl, :].reshape(ILOC, KD, P).transpose(1, 0, 2)),
            "Sbc": Sbc,
            "WoutL": woutL, "boutBC": boutBC,
        })
        in_maps.append(m)
    return in_maps


def kernel(**inputs):
    in_maps = _prep_in_maps(inputs)
    nc = _get_nc()
    r = run_bass_kernel_spmd(nc, in_maps, list(range(NCORES)))
    logits = np.concatenate(
        [r.results[c]["logits"][:, :VLOC] for c in range(NCORES)], axis=1)
    return np.ascontiguousarray(logits.reshape(B, S, V).astype(np.float32))


if __name__ == "__main__":
    _build_nc()
    print("build + compile OK")


# revision 25
# speedup vs baseline: 1.1040x; 1.0057x over previous
"""Trainium2 Bass kernel for nn_BeyazKusAIEnhanced (moe_routing).

Model (T=2048 tokens, D=1024):
  x = emb[ids]
  h = LN1(x); attention collapses exactly to: ao = (h @ Wv) @ WoSum
    (softmax over a size-1 axis is exactly 1, so out = tile(v, 16 heads)
     and out @ Wo == v @ WoSum with WoSum[r,:] = sum_h Wo[h*64+r, :])
  x1 = x + ao
  t = LN2(x1); router probs = softmax(t @ Wr + br); top-8 -> combine [T,32]
  moe = sum_e combine[:,e] * (silu(t@We1[e]+be1[e]) @ We2[e] + be2[e])
  shared = sum_s silu(t@Ws1[s]+bs1[s]) @ Ws2[s] + bs2[s]
  out = (x1 + moe + shared) @ Wout + bout        [T, 32000]

Sharding (8 cores):
  - front part (gather/LN/attn/router) replicated on all cores
  - routed experts: 4 per core (dense compute; combine weights of
    non-selected experts are exactly 0, so dense == sparse w/ weights)
  - shared experts: inter dim (2*4096 = 8192) split 1024 per core;
    bs2 biases summed on host and added post-allreduce on every core
  - partial (moe+shared) accumulated in DRAM via accum-DMA, AllReduce'd
    across cores; x2 = x1 + reduced + bs2sum
  - output projection vocab-split: 4000 cols/core (padded to 4096)

Layout: activations feature-major [128 part, 8 kchunk, 2048 tok] in SBUF;
matmuls fp32r (full PE rate at moving free dim >= 256, ~1e-4 rel err).
LN stats via all-ones [128,128] matmul (partition-broadcast sums, no
explicit broadcast step); per-core expert selection via one-hot inputs.
Router runs in plain fp32 from x1 with LN folded (host folds g2 into Wr
and beta2@Wr into br) so top-8 selection is as close to the f32
reference as possible.
"""

import numpy as np

import concourse.bass as bass
import concourse.mybir as mybir
import concourse.tile as tile
from concourse import bacc
from concourse.bass import ts
from concourse.bass_utils import run_bass_kernel_spmd
from concourse.masks import make_identity

P = 128
B, S = 2, 1024
T = 2048          # tokens
D = 1024          # model dim
KD = D // P       # 8 k-chunks
H = 16            # heads
R = 64            # kv rank / head dim
E = 32            # routed experts
ELOC = 4          # experts per core
F = 1024          # moe inter dim
FC = F // P       # 8
NS = 2            # shared experts
ILOC = 1024       # shared inter slice per core
V = 32000
VLOC = 4000       # real vocab cols per core
VPAD = 4096       # padded to 8 x 512
NCH = VPAD // 512
TC = 4            # token chunks
TW = 512          # token chunk width
NT = T // P       # 16 token tiles
EPS = 1e-5
NCORES = 8

F32 = mybir.dt.float32
F32R = mybir.dt.float32r
I32 = mybir.dt.int32
AF = mybir.ActivationFunctionType
OP = mybir.AluOpType
AX = mybir.AxisListType

_NC_CACHE = {}


def _build_nc():
    nc = bacc.Bacc(None)

    ids_d = nc.declare_dram_parameter("ids", [T, 1], I32, isOutput=False)
    emb_d = nc.declare_dram_parameter("emb", [V, D], F32, isOutput=False)
    ones_d = nc.declare_dram_parameter("ones128", [P, P], F32R, isOutput=False)
    wv_d = nc.declare_dram_parameter("Wv", [D, R], F32, isOutput=False)
    wos_d = nc.declare_dram_parameter("WoS", [R, D], F32, isOutput=False)
    wrg_d = nc.declare_dram_parameter("Wrg", [D, E], F32, isOutput=False)
    breff_d = nc.declare_dram_parameter("breff", [E, 1], F32, isOutput=False)
    g1_d = nc.declare_dram_parameter("g1v", [D], F32, isOutput=False)
    b1_d = nc.declare_dram_parameter("b1v", [D], F32, isOutput=False)
    g2_d = nc.declare_dram_parameter("g2v", [D], F32, isOutput=False)
    b2_d = nc.declare_dram_parameter("b2v", [D], F32, isOutput=False)
    we1_d = nc.declare_dram_parameter("We1L", [ELOC, FC, D, P], F32R,
                                      isOutput=False)
    be1_d = nc.declare_dram_parameter("be1L", [ELOC, F], F32, isOutput=False)
    we2_d = nc.declare_dram_parameter("We2L", [ELOC, KD, F, P], F32R,
                                      isOutput=False)
    be2_d = nc.declare_dram_parameter("be2P", [E, D], F32R, isOutput=False)
    ws1_d = nc.declare_dram_parameter("Ws1L", [FC, D, P], F32R, isOutput=False)
    bs1_d = nc.declare_dram_parameter("bs1L", [ILOC], F32, isOutput=False)
    ws2_d = nc.declare_dram_parameter("Ws2L", [KD, ILOC, P], F32R, isOutput=False)
    bs2_d = nc.declare_dram_parameter("bs2S", [D], F32, isOutput=False)
    sbc_d = nc.declare_dram_parameter("Sbc", [E, ELOC * P], F32R, isOutput=False)
    wout_d = nc.declare_dram_parameter("WoutL", [NCH, D, TW], F32R, isOutput=False)
    logits_d = nc.declare_dram_parameter("logits", [T, VPAD], F32, isOutput=True)

    with tile.TileContext(nc) as tc:
        pconst = tc.alloc_tile_pool(name="pconst", bufs=1)
        pbig = tc.alloc_tile_pool(name="pbig", bufs=1)
        ppsum = tc.alloc_tile_pool(name="ppsum", bufs=6, space="PSUM")
        pstg = tc.alloc_tile_pool(name="pstg", bufs=3)
        pdram = tc.alloc_tile_pool(name="pdram", bufs=1, space="DRAM")

        def psum_tile():
            return ppsum.tile([P, TW], F32, tag="ps", name="ps", space="PSUM")

        # ---- small constants (~8.6 KB/partition) ----
        ident = pconst.tile([P, P], F32)
        make_identity(nc, ident[:])
        ones_sb = pconst.tile([P, P], F32R)
        nc.sync.dma_start(ones_sb[:], ones_d[:, :])
        wv_sb = pconst.tile([P, KD, R], F32)
        nc.sync.dma_start(wv_sb[:], wv_d.rearrange("(ko p) r -> p ko r", p=P))
        wos_sb = pconst.tile([R, KD, P], F32)
        nc.sync.dma_start(wos_sb[:], wos_d.rearrange("r (ko p) -> r ko p", p=P))
        wrg_sb = pconst.tile([P, KD, E], F32)
        nc.sync.dma_start(wrg_sb[:], wrg_d.rearrange("(ko p) e -> p ko e", p=P))
        breff_sb = pconst.tile([E, 1], F32)
        nc.sync.dma_start(breff_sb[:], breff_d[:, :])
        g1_sb = pconst.tile([P, KD], F32)
        nc.sync.dma_start(g1_sb[:], g1_d.rearrange("(ko p) -> p ko", p=P))
        b1_sb = pconst.tile([P, KD], F32)
        nc.sync.dma_start(b1_sb[:], b1_d.rearrange("(ko p) -> p ko", p=P))
        g2_sb = pconst.tile([P, KD], F32)
        nc.sync.dma_start(g2_sb[:], g2_d.rearrange("(ko p) -> p ko", p=P))
        b2_sb = pconst.tile([P, KD], F32)
        nc.sync.dma_start(b2_sb[:], b2_d.rearrange("(ko p) -> p ko", p=P))
        be1_sb = pconst.tile([P, ELOC, FC], F32)
        nc.sync.dma_start(be1_sb[:], be1_d.rearrange("e (ko p) -> p e ko", p=P))
        bs1_sb = pconst.tile([P, FC], F32)
        nc.sync.dma_start(bs1_sb[:], bs1_d.rearrange("(ko p) -> p ko", p=P))
        bs2_sb = pconst.tile([P, KD], F32)
        nc.sync.dma_start(bs2_sb[:], bs2_d.rearrange("(ko p) -> p ko", p=P))
        eps_sb = pconst.tile([P, 1], F32)
        nc.gpsimd.memset(eps_sb[:], EPS)

        # DRAM scratch
        x1_dram = pdram.tile([P, KD, T], F32, tag="x1d")
        acc_h = [pdram.tile([P, KD, T // 2], F32, tag=f"acc{h}", name=f"acc{h}")
                 for h in range(2)]
        red_h = [pdram.tile([P, KD, T // 2], F32, tag=f"red{h}", name=f"red{h}",
                            addr_space="Shared")
                 for h in range(2)]

        # combine-weight tiles + MoE selection constants (outlive front pools)
        pmoec = tc.alloc_tile_pool(name="pmoec", bufs=1)
        c_fm = pmoec.tile([E, T], F32R, tag="cfm")
        sbc_sb = pmoec.tile([E, ELOC * P], F32R, tag="sbc")
        nc.sync.dma_start(sbc_sb[:], sbc_d[:, :])
        be2_sb = pmoec.tile([E, KD, P], F32R, tag="be2")
        nc.sync.dma_start(be2_sb[:], be2_d.rearrange("e (ko p) -> e ko p", p=P))

        pbigA = tc.alloc_tile_pool(name="pbigA", bufs=1)
        xa = pbigA.tile([P, KD, T], F32, tag="A")  # x, then x1 (in place)
        hb = pbig.tile([P, KD, T], F32, tag="B")  # h (fp32, feeds attention)

        # ---- phase 1: embedding gather + PE transpose to feature-major ----
        with (
            tc.tile_pool(name="pgather", bufs=4) as pgather,
            tc.tile_pool(name="pidx", bufs=NT) as pidx,
        ):
            idxs = []
            for i in range(NT):
                idx_sb = pidx.tile([P, 1], I32, tag="idx", name="idx")
                nc.sync.dma_start(idx_sb[:], ids_d[i * P:(i + 1) * P, :])
                idxs.append(idx_sb)
            for i in range(NT):
                idx_sb = idxs[i]
                gx = pgather.tile([P, D], F32, tag="gx", name="gx")
                nc.gpsimd.indirect_dma_start(
                    out=gx[:],
                    out_offset=None,
                    in_=emb_d[:, :],
                    in_offset=bass.IndirectOffsetOnAxis(ap=idx_sb[:, :1], axis=0),
                )
                for kc in range(KD):
                    tp = psum_tile()
                    nc.tensor.transpose(tp[:, :P], gx[:, ts(kc, P)], ident[:])
                    nc.vector.tensor_copy(xa[:, kc, i * P:(i + 1) * P], tp[:, :P])

        # ---- phases 2-5 (LN1, attention, LN2+router fused) ----
        with (
            tc.tile_pool(name="pfC", bufs=1) as pfC,
            tc.tile_pool(name="pfM", bufs=2) as pfM,
        ):
            pfA = tc.alloc_tile_pool(name="pfA", bufs=2)
            pfB = tc.alloc_tile_pool(name="pfB", bufs=2 * TC)

            def ln_stats(src, t):
                """LN stats for token chunk t -> (mu, rstd) tiles [P, TW]
                (every partition holds the same per-token row)."""
                ps_mu = psum_tile()
                ps_sq = psum_tile()
                for kc in range(KD):
                    xr = pfA.tile([P, TW], F32R, tag="sq", name="xr")
                    nc.vector.tensor_copy(xr[:], src[:, kc, ts(t, TW)])
                    nc.tensor.matmul(
                        ps_mu[:], lhsT=ones_sb[:], rhs=xr[:],
                        start=(kc == 0), stop=(kc == KD - 1))
                    sq = pfA.tile([P, TW], F32R, tag="sq", name="sq")
                    nc.scalar.activation(sq[:], src[:, kc, ts(t, TW)], AF.Square)
                    nc.tensor.matmul(
                        ps_sq[:], lhsT=ones_sb[:], rhs=sq[:],
                        start=(kc == 0), stop=(kc == KD - 1))
                mu = pfB.tile([P, TW], F32, tag="bc", name="mu")
                nc.vector.tensor_scalar_mul(mu[:], ps_mu[:], 1.0 / D)
                msq = pfA.tile([P, TW], F32, tag="lntmp", name="msq")
                nc.vector.tensor_scalar_mul(msq[:], ps_sq[:], 1.0 / D)
                mu2 = pfA.tile([P, TW], F32, tag="lntmp", name="mu2")
                nc.vector.tensor_mul(out=mu2[:], in0=mu[:], in1=mu[:])
                nc.vector.tensor_tensor(msq[:], msq[:], mu2[:], op=OP.subtract)
                nc.scalar.activation(msq[:], msq[:], AF.Sqrt, bias=eps_sb[:, 0:1])
                rstd = pfB.tile([P, TW], F32, tag="bc", name="rstd")
                nc.vector.reciprocal(rstd[:], msq[:])
                return mu, rstd

            def ln_apply(src, dst, t, mu, rstd, g_sb, b_sb):
                for kc in range(KD):
                    eng = nc.vector if kc % 2 == 0 else nc.gpsimd
                    eng.tensor_tensor(
                        dst[:, kc, ts(t, TW)], src[:, kc, ts(t, TW)], mu[:],
                        op=OP.subtract)
                    eng.tensor_tensor(
                        dst[:, kc, ts(t, TW)], dst[:, kc, ts(t, TW)], rstd[:],
                        op=OP.mult)
                    eng.tensor_scalar(
                        dst[:, kc, ts(t, TW)], dst[:, kc, ts(t, TW)],
                        g_sb[:, kc:kc + 1], b_sb[:, kc:kc + 1],
                        op0=OP.mult, op1=OP.add)

            # LN1 -> h (stats for all chunks first, applies after: keeps
            # the PE stats matmuls from waiting behind DVE apply chains)
            st1 = [ln_stats(xa, t) for t in range(TC)]
            for t in range(TC):
                ln_apply(xa, hb, t, st1[t][0], st1[t][1], g1_sb, b1_sb)

            # v = h @ Wv  [R, T]
            v_sb = pfC.tile([R, T], F32, tag="v")
            for t in range(TC):
                ps = psum_tile()
                for kc in range(KD):
                    nc.tensor.matmul(
                        ps[:R, :], lhsT=wv_sb[:, kc, :], rhs=hb[:, kc, ts(t, TW)],
                        start=(kc == 0), stop=(kc == KD - 1))
                nc.vector.tensor_copy(v_sb[:, ts(t, TW)], ps[:R, :])
            # x1 = x + v @ WoSum  (in place into xa)
            for dc in range(KD):
                for t in range(TC):
                    ps = psum_tile()
                    nc.tensor.matmul(
                        ps[:], lhsT=wos_sb[:, dc, :], rhs=v_sb[:, ts(t, TW)],
                        start=True, stop=True)
                    nc.vector.tensor_add(
                        out=xa[:, dc, ts(t, TW)], in0=xa[:, dc, ts(t, TW)],
                        in1=ps[:])
            nc.sync.dma_start(x1_dram[:], xa[:])

            # LN2 -> t (f32r, into slot B), fused with fp32 router matmul
            tb = pbig.tile([P, KD, T], F32R, tag="B", name="tb")
            r_fm = pfC.tile([E, T], F32, tag="v", name="r_fm")
            st2 = [ln_stats(xa, t) for t in range(TC)]
            for t in range(TC):
                mu, rstd = st2[t]
                ln_apply(xa, tb, t, mu, rstd, g2_sb, b2_sb)
                ps = psum_tile()
                for kc in range(KD):
                    rt = pfA.tile([P, TW], F32, tag="rt", name="rt")
                    nc.vector.tensor_tensor(
                        rt[:], xa[:, kc, ts(t, TW)], mu[:],
                        op=OP.subtract)
                    nc.tensor.matmul(
                        ps[:E, :], lhsT=wrg_sb[:, kc, :], rhs=rt[:],
                        start=(kc == 0), stop=(kc == KD - 1))
                nc.vector.tensor_tensor(
                    r_fm[:, ts(t, TW)], ps[:E, :], rstd[:E, :], op=OP.mult)
                nc.vector.tensor_scalar_add(
                    r_fm[:, ts(t, TW)], r_fm[:, ts(t, TW)], breff_sb[:E, 0:1])

            pfB.release()
            pfA.release()

            # softmax + top-8 in token-major
            r_tm = pfC.tile([P, NT, E], F32, tag="rtm")
            for i in range(NT):
                tp = psum_tile()
                nc.tensor.transpose(
                    tp[:, :E], r_fm[:, i * P:(i + 1) * P], ident[:E, :E])
                nc.vector.tensor_copy(r_tm[:, i, :], tp[:, :E])
            m_sb = pfM.tile([P, NT], F32, tag="m", name="m1")
            nc.vector.reduce_max(m_sb[:, :, None], r_tm[:], axis=AX.X)
            nc.vector.tensor_tensor(
                r_tm[:], r_tm[:], m_sb[:, :, None].to_broadcast([P, NT, E]),
                op=OP.subtract)
            nc.scalar.activation(r_tm[:], r_tm[:], AF.Exp)
            s_sb = pfM.tile([P, NT], F32, tag="m", name="m2")
            nc.vector.reduce_sum(s_sb[:, :, None], r_tm[:], axis=AX.X)
            rs_sb = pfM.tile([P, NT], F32, tag="m", name="m3")
            nc.vector.reciprocal(rs_sb[:], s_sb[:])
            nc.vector.tensor_tensor(
                r_tm[:], r_tm[:], rs_sb[:, :, None].to_broadcast([P, NT, E]),
                op=OP.mult)
            work = pmoec.tile([P, NT, E], F32, tag="work")
            msk = pfC.tile([P, NT, E], F32, tag="msk")
            nc.vector.tensor_copy(work[:], r_tm[:])
            thr = pfM.tile([P, NT], F32, tag="m", name="m4")
            for it in range(8):
                nc.vector.reduce_max(thr[:, :, None], work[:], axis=AX.X)
                if it < 7:
                    nc.vector.tensor_tensor(
                        msk[:], work[:], thr[:, :, None].to_broadcast([P, NT, E]),
                        op=OP.is_lt)
                    nc.vector.tensor_tensor(work[:], work[:], msk[:], op=OP.mult)
            nc.vector.tensor_tensor(
                msk[:], r_tm[:], thr[:, :, None].to_broadcast([P, NT, E]),
                op=OP.is_ge)
            nc.vector.tensor_tensor(work[:], r_tm[:], msk[:], op=OP.mult)
            wsum = pfM.tile([P, NT], F32, tag="m", name="m5")
            nc.vector.reduce_sum(wsum[:, :, None], work[:], axis=AX.X)
            rws = pfM.tile([P, NT], F32, tag="m", name="m6")
            nc.vector.reciprocal(rws[:], wsum[:])
            nc.vector.tensor_tensor(
                work[:], work[:], rws[:, :, None].to_broadcast([P, NT, E]),
                op=OP.mult)

        pbigA.release()

        T2 = T // 2

        # ---- phase 6: MoE (4 routed dense + shared slice) ----
        # Token-half-outer over the whole expert set: the first half\'s
        # partial sum is complete mid-phase, so its all-reduce runs under
        # the second half\'s compute and the output projection starts at
        # phase end with no collective exposure.
        with (
            tc.tile_pool(name="pw", bufs=10) as pw,
            tc.tile_pool(name="pcbc", bufs=2) as pcbc,
            tc.tile_pool(name="pz", bufs=2) as pz,
        ):
            TC2 = TC // 2

            def emit_cbc(e, half):
                cbc = pcbc.tile([P, T2], F32, tag="cbc", name="cbc")
                for t2 in range(TC2):
                    t = half * TC2 + t2
                    ps = psum_tile()
                    nc.tensor.matmul(
                        ps[:], lhsT=sbc_sb[:, ts(e, P)],
                        rhs=c_fm[:, ts(t, TW)], start=True, stop=True)
                    nc.vector.tensor_copy(cbc[:, ts(t2, TW)], ps[:])
                return cbc

            for half in range(2):
                for e in range(ELOC + 1):
                    shared = e == ELOC
                    first = e == 0 and half == 0
                    cbc = (None if shared or first else emit_cbc(e, half))
                    zh = pz.tile([P, FC, T2], F32R, tag="z", name="zh")
                    for fc in range(FC):
                        w1f = pw.tile([P, KD, P], F32R, tag="w", name="w1f")
                        src_ap = (ws1_d[fc] if shared else we1_d[e, fc])
                        nc.sync.dma_start(
                            w1f[:], src_ap.rearrange("(ko p) m -> p ko m", p=P))
                        bias = (bs1_sb[:, fc:fc + 1] if shared
                                else be1_sb[:, e, fc:fc + 1])
                        for t2 in range(TC2):
                            t = half * TC2 + t2
                            ps = psum_tile()
                            for kc in range(KD):
                                nc.tensor.matmul(
                                    ps[:], lhsT=w1f[:, kc, :],
                                    rhs=tb[:, kc, ts(t, TW)],
                                    start=(kc == 0), stop=(kc == KD - 1))
                            nc.scalar.activation(
                                zh[:, fc, ts(t2, TW)], ps[:], AF.Silu,
                                bias=bias)
                            if cbc is not None:
                                nc.vector.tensor_tensor(
                                    zh[:, fc, ts(t2, TW)],
                                    zh[:, fc, ts(t2, TW)],
                                    cbc[:, ts(t2, TW)], op=OP.mult)
                    if first:
                        # combine weights -> expert-major, deferred so
                        # expert 0\'s first matmuls don\'t wait on the
                        # top-k DVE chain; then scale its z after the fact
                        for i in range(NT):
                            tp = psum_tile()
                            nc.tensor.transpose(
                                tp[:E, :P], work[:, i, :], ident[:])
                            nc.vector.tensor_copy(
                                c_fm[:, i * P:(i + 1) * P], tp[:E, :P])
                        cbc = emit_cbc(0, 0)
                        for t2 in range(TC2):
                            for fc in range(FC):
                                nc.vector.tensor_tensor(
                                    zh[:, fc, ts(t2, TW)],
                                    zh[:, fc, ts(t2, TW)],
                                    cbc[:, ts(t2, TW)], op=OP.mult)
                    # mm2 (dc-major streamed weights)
                    for dc in range(KD):
                        w2d = pw.tile([P, FC, P], F32R, tag="w", name="w2d")
                        src_ap = (ws2_d[dc] if shared else we2_d[e, dc])
                        nc.sync.dma_start(
                            w2d[:], src_ap.rearrange("(fo p) m -> p fo m", p=P))
                        for t2 in range(TC2):
                            t = half * TC2 + t2
                            ps = psum_tile()
                            for fc in range(FC):
                                nc.tensor.matmul(
                                    ps[:], lhsT=w2d[:, fc, :],
                                    rhs=zh[:, fc, ts(t2, TW)],
                                    start=(fc == 0),
                                    stop=(fc == FC - 1 and not shared))
                            if shared:
                                nc.tensor.matmul(
                                    ps[:], lhsT=be2_sb[:, dc, :],
                                    rhs=c_fm[:, ts(t, TW)],
                                    start=False, stop=True)
                            stg = pstg.tile([P, TW], F32, tag="stg", name="stg")
                            nc.scalar.activation(stg[:], ps[:], AF.Copy)
                            nc.gpsimd.dma_start(
                                acc_h[half][:, dc, ts(t2, TW)], stg[:],
                                accum_op=(OP.bypass if e == 0 else OP.add))
                # this token half\'s partial is complete on this core
                nc.gpsimd.collective_compute(
                    "AllReduce",
                    OP.add,
                    replica_groups=[list(range(NCORES))],
                    ins=[acc_h[half][:].opt()],
                    outs=[red_h[half][:].opt()],
                )
        pmoec.release()

        # ---- phase 7: AllReduce; x2 = x1 + red + bs2sum; out projection ----
        with (
            tc.tile_pool(name="pxb", bufs=3) as pxb,
            tc.tile_pool(name="pwout", bufs=12) as pwout,
        ):
            x2 = pbig.tile([P, KD, T], F32R, tag="B")
            # two half-passes over tokens: the first half only needs the
            # first two all-reduced chunks, so its projection overlaps the
            # later all-reduces (and the engines stay in-order-clean)
            for half in range(2):
                for t2 in range(TC // 2):
                    t = half * (TC // 2) + t2
                    for kc in range(KD):
                        xb = pxb.tile([P, TW], F32, tag="xb", name="xb")
                        nc.sync.dma_start(xb[:], x1_dram[:, kc, ts(t, TW)])
                        rb = pxb.tile([P, TW], F32, tag="rb", name="rb")
                        nc.sync.dma_start(rb[:], red_h[half][:, kc, ts(t2, TW)])
                        nc.vector.tensor_add(out=xb[:], in0=xb[:], in1=rb[:])
                        nc.vector.tensor_scalar_add(
                            x2[:, kc, ts(t, TW)], xb[:], bs2_sb[:, kc:kc + 1])
                for n in range(NCH):
                    wot = []
                    for kc in range(KD):
                        wt = pwout.tile([P, TW], F32R, tag="wo", name="wo")
                        nc.sync.dma_start(wt[:], wout_d[n, ts(kc, P), :])
                        wot.append(wt)
                    for m in range(half * NT // 2, (half + 1) * NT // 2):
                        ps = psum_tile()
                        for kc in range(KD):
                            nc.tensor.matmul(
                                ps[:], lhsT=x2[:, kc, ts(m, P)], rhs=wot[kc][:],
                                start=(kc == 0), stop=(kc == KD - 1))
                        stg = pstg.tile([P, TW], F32, tag="stg", name="stg")
                        nc.scalar.activation(stg[:], ps[:], AF.Copy)
                        nc.sync.dma_start(logits_d[ts(m, P), ts(n, TW)], stg[:])

        for _pool in (pdram, pstg, ppsum, pbig, pconst):
            _pool.release()

    nc.compile()
    return nc


def _get_nc():
    if "nc" not in _NC_CACHE:
        _NC_CACHE["nc"] = _build_nc()
    return _NC_CACHE["nc"]


def _prep_in_maps(inputs):
    inp = {k: np.asarray(v) for k, v in inputs.items()}
    f32 = np.float32

    ids = np.ascontiguousarray(inp["input_ids"].reshape(T, 1).astype(np.int32))
    emb = np.ascontiguousarray(inp["emb"].astype(f32))
    WoS = np.ascontiguousarray(
        inp["Wo"].astype(f32).reshape(H, R, D).sum(0).astype(f32))
    g2 = inp["g2"].astype(f32)
    Wrg = np.ascontiguousarray((g2[:, None] * inp["Wr"].astype(f32)).astype(f32))
    breff = (inp["br"].astype(f32)
             + inp["beta2"].astype(f32) @ inp["Wr"].astype(f32))
    breff = np.ascontiguousarray(breff.reshape(E, 1).astype(f32))

    common = {
        "ids": ids, "emb": emb,
        "ones128": np.ones((P, P), f32),
        "Wv": np.ascontiguousarray(inp["Wv"].astype(f32)),
        "WoS": WoS, "Wrg": Wrg, "breff": breff,
        "g1v": inp["g1"].astype(f32), "b1v": inp["beta1"].astype(f32),
        "g2v": g2, "b2v": inp["beta2"].astype(f32),
        "bs2S": np.ascontiguousarray(inp["bs2"].astype(f32).sum(0)),
    }

    We1 = inp["We1"].astype(f32)
    be1 = inp["be1"].astype(f32)
    We2 = inp["We2"].astype(f32)
    be2 = inp["be2"].astype(f32)
    Ws1 = inp["Ws1"].astype(f32)
    bs1 = inp["bs1"].astype(f32)
    Ws2 = inp["Ws2"].astype(f32)
    Wout = inp["Wout"].astype(f32)
    bout = inp["bout"].astype(f32)

    in_maps = []
    for c in range(NCORES):
        el = list(range(ELOC * c, ELOC * (c + 1)))
        s, q = divmod(c, NCORES // NS)
        isl = slice(q * ILOC, (q + 1) * ILOC)
        Sbc = np.zeros((E, ELOC * P), f32)
        for j, e in enumerate(el):
            Sbc[e, j * P:(j + 1) * P] = 1.0
        wout_pad = np.zeros((D, VPAD), f32)
        wout_pad[:, :VLOC] = Wout[:, VLOC * c:VLOC * (c + 1)]
        woutL = np.ascontiguousarray(
            wout_pad.reshape(D, NCH, TW).transpose(1, 0, 2))
        m = dict(common)
        be2P = np.zeros((E, D), f32)
        be2P[el] = be2[el]
        m.update({
            "We1L": np.ascontiguousarray(
                We1[el].reshape(ELOC, D, FC, P).transpose(0, 2, 1, 3)),
            "be1L": np.ascontiguousarray(be1[el]),
            "We2L": np.ascontiguousarray(
                We2[el].reshape(ELOC, F, KD, P).transpose(0, 2, 1, 3)),
            "be2P": be2P,
            "Ws1L": np.ascontiguousarray(
                Ws1[s][:, isl].reshape(D, FC, P).transpose(1, 0, 2)),
            "bs1L": np.ascontiguousarray(bs1[s][isl]),
            "Ws2L": np.ascontiguousarray(
                Ws2[s][isl, :].reshape(ILOC, KD, P).transpose(1, 0, 2)),
            "Sbc": Sbc,
            "WoutL": woutL,
        })
        in_maps.append(m)
    return in_maps


def kernel(**inputs):
    in_maps = _prep_in_maps(inputs)
    nc = _get_nc()
    r = run_bass_kernel_spmd(nc, in_maps, list(range(NCORES)))
    logits = np.concatenate(
        [r.results[c]["logits"][:, :VLOC] for c in range(NCORES)], axis=1)
    bout = np.asarray(inputs["bout"]).astype(np.float32)
    if np.any(bout):
        logits = logits + bout[None, :]
    return np.ascontiguousarray(logits.reshape(B, S, V).astype(np.float32))


if __name__ == "__main__":
    _build_nc()
    print("build + compile OK")


# revision 26
# speedup vs baseline: 1.1302x; 1.0237x over previous
"""Trainium2 Bass kernel for nn_BeyazKusAIEnhanced (moe_routing).

Model (T=2048 tokens, D=1024):
  x = emb[ids]
  h = LN1(x); attention collapses exactly to: ao = (h @ Wv) @ WoSum
    (softmax over a size-1 axis is exactly 1, so out = tile(v, 16 heads)
     and out @ Wo == v @ WoSum with WoSum[r,:] = sum_h Wo[h*64+r, :])
  x1 = x + ao
  t = LN2(x1); router probs = softmax(t @ Wr + br); top-8 -> combine [T,32]
  moe = sum_e combine[:,e] * (silu(t@We1[e]+be1[e]) @ We2[e] + be2[e])
  shared = sum_s silu(t@Ws1[s]+bs1[s]) @ Ws2[s] + bs2[s]
  out = (x1 + moe + shared) @ Wout + bout        [T, 32000]

Sharding (8 cores):
  - front part (gather/LN/attn/router) replicated on all cores
  - routed experts: 4 per core (dense compute; combine weights of
    non-selected experts are exactly 0, so dense == sparse w/ weights)
  - shared experts: inter dim (2*4096 = 8192) split 1024 per core;
    bs2 biases summed on host and added post-allreduce on every core
  - partial (moe+shared) accumulated in DRAM via accum-DMA, AllReduce'd
    across cores; x2 = x1 + reduced + bs2sum
  - output projection vocab-split: 4000 cols/core (padded to 4096)

Layout: activations feature-major [128 part, 8 kchunk, 2048 tok] in SBUF;
matmuls fp32r (full PE rate at moving free dim >= 256, ~1e-4 rel err).
LN stats via all-ones [128,128] matmul (partition-broadcast sums, no
explicit broadcast step); per-core expert selection via one-hot inputs.
Router runs in plain fp32 from x1 with LN folded (host folds g2 into Wr
and beta2@Wr into br) so top-8 selection is as close to the f32
reference as possible.
"""

import numpy as np

import concourse.bass as bass
import concourse.mybir as mybir
import concourse.tile as tile
from concourse import bacc
from concourse.bass import ts
from concourse.bass_utils import run_bass_kernel_spmd
from concourse.masks import make_identity

P = 128
B, S = 2, 1024
T = 2048          # tokens
D = 1024          # model dim
KD = D // P       # 8 k-chunks
H = 16            # heads
R = 64            # kv rank / head dim
E = 32            # routed experts
ELOC = 4          # experts per core
F = 1024          # moe inter dim
FC = F // P       # 8
NS = 2            # shared experts
ILOC = 1024       # shared inter slice per core
V = 32000
VLOC = 4000       # real vocab cols per core
VPAD = 4096       # padded to 8 x 512
NCH = VPAD // 512
TC = 4            # token chunks
TW = 512          # token chunk width
NT = T // P       # 16 token tiles
EPS = 1e-5
NCORES = 8

F32 = mybir.dt.float32
F32R = mybir.dt.float32r
I32 = mybir.dt.int32
AF = mybir.ActivationFunctionType
OP = mybir.AluOpType
AX = mybir.AxisListType

_NC_CACHE = {}


def _build_nc():
    nc = bacc.Bacc(None)

    ids_d = nc.declare_dram_parameter("ids", [T, 1], I32, isOutput=False)
    emb_d = nc.declare_dram_parameter("emb", [V, D], F32, isOutput=False)
    ones_d = nc.declare_dram_parameter("ones128", [P, P], F32R, isOutput=False)
    wv_d = nc.declare_dram_parameter("Wv", [D, R], F32, isOutput=False)
    wos_d = nc.declare_dram_parameter("WoS", [R, D], F32, isOutput=False)
    wrg_d = nc.declare_dram_parameter("Wrg", [D, E], F32, isOutput=False)
    breff_d = nc.declare_dram_parameter("breff", [E, 1], F32, isOutput=False)
    g1_d = nc.declare_dram_parameter("g1v", [D], F32, isOutput=False)
    b1_d = nc.declare_dram_parameter("b1v", [D], F32, isOutput=False)
    g2_d = nc.declare_dram_parameter("g2v", [D], F32, isOutput=False)
    b2_d = nc.declare_dram_parameter("b2v", [D], F32, isOutput=False)
    we1_d = nc.declare_dram_parameter("We1L", [ELOC, FC, D, P], F32R,
                                      isOutput=False)
    be1_d = nc.declare_dram_parameter("be1L", [ELOC, F], F32, isOutput=False)
    we2_d = nc.declare_dram_parameter("We2L", [ELOC, KD, F, P], F32R,
                                      isOutput=False)
    be2_d = nc.declare_dram_parameter("be2P", [E, D], F32R, isOutput=False)
    ws1_d = nc.declare_dram_parameter("Ws1L", [FC, D, P], F32R, isOutput=False)
    bs1_d = nc.declare_dram_parameter("bs1L", [ILOC], F32, isOutput=False)
    ws2_d = nc.declare_dram_parameter("Ws2L", [KD, ILOC, P], F32R, isOutput=False)
    bs2_d = nc.declare_dram_parameter("bs2S", [D], F32, isOutput=False)
    sbc_d = nc.declare_dram_parameter("Sbc", [E, ELOC * P], F32R, isOutput=False)
    wout_d = nc.declare_dram_parameter("WoutL", [NCH, D, TW], F32R, isOutput=False)
    logits_d = nc.declare_dram_parameter("logits", [T, VPAD], F32, isOutput=True)

    with tile.TileContext(nc) as tc:
        pconst = tc.alloc_tile_pool(name="pconst", bufs=1)
        pbig = tc.alloc_tile_pool(name="pbig", bufs=1)
        ppsum = tc.alloc_tile_pool(name="ppsum", bufs=6, space="PSUM")
        pstg = tc.alloc_tile_pool(name="pstg", bufs=3)
        pdram = tc.alloc_tile_pool(name="pdram", bufs=1, space="DRAM")

        def psum_tile():
            return ppsum.tile([P, TW], F32, tag="ps", name="ps", space="PSUM")

        # ---- small constants (~8.6 KB/partition) ----
        ident = pconst.tile([P, P], F32)
        make_identity(nc, ident[:])
        ones_sb = pconst.tile([P, P], F32R)
        nc.sync.dma_start(ones_sb[:], ones_d[:, :])
        wv_sb = pconst.tile([P, KD, R], F32)
        nc.sync.dma_start(wv_sb[:], wv_d.rearrange("(ko p) r -> p ko r", p=P))
        wos_sb = pconst.tile([R, KD, P], F32)
        nc.sync.dma_start(wos_sb[:], wos_d.rearrange("r (ko p) -> r ko p", p=P))
        wrg_sb = pconst.tile([P, KD, E], F32)
        nc.sync.dma_start(wrg_sb[:], wrg_d.rearrange("(ko p) e -> p ko e", p=P))
        breff_sb = pconst.tile([E, 1], F32)
        nc.sync.dma_start(breff_sb[:], breff_d[:, :])
        g1_sb = pconst.tile([P, KD], F32)
        nc.sync.dma_start(g1_sb[:], g1_d.rearrange("(ko p) -> p ko", p=P))
        b1_sb = pconst.tile([P, KD], F32)
        nc.sync.dma_start(b1_sb[:], b1_d.rearrange("(ko p) -> p ko", p=P))
        g2_sb = pconst.tile([P, KD], F32)
        nc.sync.dma_start(g2_sb[:], g2_d.rearrange("(ko p) -> p ko", p=P))
        b2_sb = pconst.tile([P, KD], F32)
        nc.sync.dma_start(b2_sb[:], b2_d.rearrange("(ko p) -> p ko", p=P))
        be1_sb = pconst.tile([P, ELOC, FC], F32)
        nc.sync.dma_start(be1_sb[:], be1_d.rearrange("e (ko p) -> p e ko", p=P))
        bs1_sb = pconst.tile([P, FC], F32)
        nc.sync.dma_start(bs1_sb[:], bs1_d.rearrange("(ko p) -> p ko", p=P))
        bs2_sb = pconst.tile([P, KD], F32)
        nc.sync.dma_start(bs2_sb[:], bs2_d.rearrange("(ko p) -> p ko", p=P))
        eps_sb = pconst.tile([P, 1], F32)
        nc.gpsimd.memset(eps_sb[:], EPS)

        # DRAM scratch
        x1_dram = pdram.tile([P, KD, T], F32, tag="x1d")
        acc_h = [pdram.tile([P, KD, T // 2], F32, tag=f"acc{h}", name=f"acc{h}")
                 for h in range(2)]
        red_h = [pdram.tile([P, KD, T // 2], F32, tag=f"red{h}", name=f"red{h}",
                            addr_space="Shared")
                 for h in range(2)]

        # combine-weight tiles + MoE selection constants (outlive front pools)
        pmoec = tc.alloc_tile_pool(name="pmoec", bufs=1)
        c_fm = pmoec.tile([E, T], F32R, tag="cfm")
        sbc_sb = pmoec.tile([E, ELOC * P], F32R, tag="sbc")
        nc.sync.dma_start(sbc_sb[:], sbc_d[:, :])
        be2_sb = pmoec.tile([E, KD, P], F32R, tag="be2")
        nc.sync.dma_start(be2_sb[:], be2_d.rearrange("e (ko p) -> e ko p", p=P))

        pbigA = tc.alloc_tile_pool(name="pbigA", bufs=1)
        xa = pbigA.tile([P, KD, T], F32, tag="A")  # x, then x1 (in place)
        hb = pbig.tile([P, KD, T], F32, tag="B")  # h (fp32, feeds attention)

        # ---- phase 1: embedding gather + PE transpose to feature-major ----
        with (
            tc.tile_pool(name="pgather", bufs=4) as pgather,
            tc.tile_pool(name="pidx", bufs=NT) as pidx,
        ):
            idxs = []
            for i in range(NT):
                idx_sb = pidx.tile([P, 1], I32, tag="idx", name="idx")
                nc.sync.dma_start(idx_sb[:], ids_d[i * P:(i + 1) * P, :])
                idxs.append(idx_sb)
            for i in range(NT):
                idx_sb = idxs[i]
                gx = pgather.tile([P, D], F32, tag="gx", name="gx")
                nc.gpsimd.indirect_dma_start(
                    out=gx[:],
                    out_offset=None,
                    in_=emb_d[:, :],
                    in_offset=bass.IndirectOffsetOnAxis(ap=idx_sb[:, :1], axis=0),
                )
                for kc in range(KD):
                    tp = psum_tile()
                    nc.tensor.transpose(tp[:, :P], gx[:, ts(kc, P)], ident[:])
                    nc.vector.tensor_copy(xa[:, kc, i * P:(i + 1) * P], tp[:, :P])

        # ---- phases 2-5 (LN1, attention, LN2+router fused) ----
        with (
            tc.tile_pool(name="pfC", bufs=1) as pfC,
            tc.tile_pool(name="pfM", bufs=2) as pfM,
        ):
            pfA = tc.alloc_tile_pool(name="pfA", bufs=2)
            pfB = tc.alloc_tile_pool(name="pfB", bufs=2 * TC)

            def ln_stats(src, t):
                """LN stats for token chunk t -> (mu, rstd) tiles [P, TW]
                (every partition holds the same per-token row)."""
                ps_mu = psum_tile()
                ps_sq = psum_tile()
                for kc in range(KD):
                    xr = pfA.tile([P, TW], F32R, tag="sq", name="xr")
                    nc.vector.tensor_copy(xr[:], src[:, kc, ts(t, TW)])
                    nc.tensor.matmul(
                        ps_mu[:], lhsT=ones_sb[:], rhs=xr[:],
                        start=(kc == 0), stop=(kc == KD - 1))
                    sq = pfA.tile([P, TW], F32R, tag="sq", name="sq")
                    nc.scalar.activation(sq[:], src[:, kc, ts(t, TW)], AF.Square)
                    nc.tensor.matmul(
                        ps_sq[:], lhsT=ones_sb[:], rhs=sq[:],
                        start=(kc == 0), stop=(kc == KD - 1))
                mu = pfB.tile([P, TW], F32, tag="bc", name="mu")
                nc.vector.tensor_scalar_mul(mu[:], ps_mu[:], 1.0 / D)
                msq = pfA.tile([P, TW], F32, tag="lntmp", name="msq")
                nc.vector.tensor_scalar_mul(msq[:], ps_sq[:], 1.0 / D)
                mu2 = pfA.tile([P, TW], F32, tag="lntmp", name="mu2")
                nc.vector.tensor_mul(out=mu2[:], in0=mu[:], in1=mu[:])
                nc.vector.tensor_tensor(msq[:], msq[:], mu2[:], op=OP.subtract)
                nc.scalar.activation(msq[:], msq[:], AF.Sqrt, bias=eps_sb[:, 0:1])
                rstd = pfB.tile([P, TW], F32, tag="bc", name="rstd")
                nc.vector.reciprocal(rstd[:], msq[:])
                return mu, rstd

            def ln_apply(src, dst, t, mu, rstd, g_sb, b_sb):
                for kc in range(KD):
                    eng = nc.vector if kc % 2 == 0 else nc.gpsimd
                    eng.tensor_tensor(
                        dst[:, kc, ts(t, TW)], src[:, kc, ts(t, TW)], mu[:],
                        op=OP.subtract)
                    eng.tensor_tensor(
                        dst[:, kc, ts(t, TW)], dst[:, kc, ts(t, TW)], rstd[:],
                        op=OP.mult)
                    eng.tensor_scalar(
                        dst[:, kc, ts(t, TW)], dst[:, kc, ts(t, TW)],
                        g_sb[:, kc:kc + 1], b_sb[:, kc:kc + 1],
                        op0=OP.mult, op1=OP.add)

            # LN1 -> h (stats for all chunks first, applies after: keeps
            # the PE stats matmuls from waiting behind DVE apply chains)
            st1 = [ln_stats(xa, t) for t in range(TC)]
            for t in range(TC):
                ln_apply(xa, hb, t, st1[t][0], st1[t][1], g1_sb, b1_sb)

            # v = h @ Wv  [R, T]
            v_sb = pfC.tile([R, T], F32, tag="v")
            for t in range(TC):
                ps = psum_tile()
                for kc in range(KD):
                    nc.tensor.matmul(
                        ps[:R, :], lhsT=wv_sb[:, kc, :], rhs=hb[:, kc, ts(t, TW)],
                        start=(kc == 0), stop=(kc == KD - 1))
                nc.vector.tensor_copy(v_sb[:, ts(t, TW)], ps[:R, :])
            # x1 = x + v @ WoSum  (in place into xa)
            for dc in range(KD):
                for t in range(TC):
                    ps = psum_tile()
                    nc.tensor.matmul(
                        ps[:], lhsT=wos_sb[:, dc, :], rhs=v_sb[:, ts(t, TW)],
                        start=True, stop=True)
                    nc.vector.tensor_add(
                        out=xa[:, dc, ts(t, TW)], in0=xa[:, dc, ts(t, TW)],
                        in1=ps[:])
            nc.sync.dma_start(x1_dram[:], xa[:])

            # LN2 -> t (f32r, into slot B), fused with fp32 router matmul
            tb = pbig.tile([P, KD, T], F32R, tag="B", name="tb")
            r_fm = pfC.tile([E, T], F32, tag="v", name="r_fm")
            st2 = [ln_stats(xa, t) for t in range(TC)]
            for t in range(TC):
                mu, rstd = st2[t]
                ln_apply(xa, tb, t, mu, rstd, g2_sb, b2_sb)
                ps = psum_tile()
                for kc in range(KD):
                    rt = pfA.tile([P, TW], F32, tag="rt", name="rt")
                    nc.vector.tensor_tensor(
                        rt[:], xa[:, kc, ts(t, TW)], mu[:],
                        op=OP.subtract)
                    nc.tensor.matmul(
                        ps[:E, :], lhsT=wrg_sb[:, kc, :], rhs=rt[:],
                        start=(kc == 0), stop=(kc == KD - 1))
                nc.vector.tensor_tensor(
                    r_fm[:, ts(t, TW)], ps[:E, :], rstd[:E, :], op=OP.mult)
                nc.vector.tensor_scalar_add(
                    r_fm[:, ts(t, TW)], r_fm[:, ts(t, TW)], breff_sb[:E, 0:1])

            pfB.release()
            pfA.release()

            # softmax + top-8 in token-major
            r_tm = pfC.tile([P, NT, E], F32, tag="rtm")
            for i in range(NT):
                tp = psum_tile()
                nc.tensor.transpose(
                    tp[:, :E], r_fm[:, i * P:(i + 1) * P], ident[:E, :E])
                nc.vector.tensor_copy(r_tm[:, i, :], tp[:, :E])
            m_sb = pfM.tile([P, NT], F32, tag="m", name="m1")
            nc.vector.reduce_max(m_sb[:, :, None], r_tm[:], axis=AX.X)
            nc.vector.tensor_tensor(
                r_tm[:], r_tm[:], m_sb[:, :, None].to_broadcast([P, NT, E]),
                op=OP.subtract)
            nc.scalar.activation(r_tm[:], r_tm[:], AF.Exp)
            s_sb = pfM.tile([P, NT], F32, tag="m", name="m2")
            nc.vector.reduce_sum(s_sb[:, :, None], r_tm[:], axis=AX.X)
            rs_sb = pfM.tile([P, NT], F32, tag="m", name="m3")
            nc.vector.reciprocal(rs_sb[:], s_sb[:])
            nc.vector.tensor_tensor(
                r_tm[:], r_tm[:], rs_sb[:, :, None].to_broadcast([P, NT, E]),
                op=OP.mult)
            work = pmoec.tile([P, NT, E], F32, tag="work")
            msk = pfC.tile([P, NT, E], F32, tag="msk")
            nc.vector.tensor_copy(work[:], r_tm[:])
            thr = pfM.tile([P, NT], F32, tag="m", name="m4")
            for it in range(8):
                nc.vector.reduce_max(thr[:, :, None], work[:], axis=AX.X)
                if it < 7:
                    nc.vector.tensor_tensor(
                        msk[:], work[:], thr[:, :, None].to_broadcast([P, NT, E]),
                        op=OP.is_lt)
                    nc.vector.tensor_tensor(work[:], work[:], msk[:], op=OP.mult)
            nc.vector.tensor_tensor(
                msk[:], r_tm[:], thr[:, :, None].to_broadcast([P, NT, E]),
                op=OP.is_ge)
            nc.vector.tensor_tensor(work[:], r_tm[:], msk[:], op=OP.mult)
            wsum = pfM.tile([P, NT], F32, tag="m", name="m5")
            nc.vector.reduce_sum(wsum[:, :, None], work[:], axis=AX.X)
            rws = pfM.tile([P, NT], F32, tag="m", name="m6")
            nc.vector.reciprocal(rws[:], wsum[:])
            nc.vector.tensor_tensor(
                work[:], work[:], rws[:, :, None].to_broadcast([P, NT, E]),
                op=OP.mult)

        pbigA.release()

        T2 = T // 2

        # ---- phase 6: MoE (4 routed dense + shared slice) ----
        # Token-half-outer over the whole expert set: the first half\'s
        # partial sum is complete mid-phase, so its all-reduce runs under
        # the second half\'s compute and the output projection starts at
        # phase end with no collective exposure.
        with (
            tc.tile_pool(name="pw", bufs=10) as pw,
            tc.tile_pool(name="pcbc", bufs=2) as pcbc,
            tc.tile_pool(name="pz", bufs=2) as pz,
        ):
            TC2 = TC // 2

            def emit_cbc(e, half):
                cbc = pcbc.tile([P, T2], F32, tag="cbc", name="cbc")
                for t2 in range(TC2):
                    t = half * TC2 + t2
                    ps = psum_tile()
                    nc.tensor.matmul(
                        ps[:], lhsT=sbc_sb[:, ts(e, P)],
                        rhs=c_fm[:, ts(t, TW)], start=True, stop=True)
                    nc.vector.tensor_copy(cbc[:, ts(t2, TW)], ps[:])
                return cbc

            for half in range(2):
                for e in range(ELOC + 1):
                    shared = e == ELOC
                    first = e == 0 and half == 0
                    cbc = (None if shared or first else emit_cbc(e, half))
                    zh = pz.tile([P, FC, T2], F32R, tag="z", name="zh")
                    for fc in range(FC):
                        w1f = pw.tile([P, KD, P], F32R, tag="w", name="w1f")
                        src_ap = (ws1_d[fc] if shared else we1_d[e, fc])
                        nc.sync.dma_start(
                            w1f[:], src_ap.rearrange("(ko p) m -> p ko m", p=P))
                        bias = (bs1_sb[:, fc:fc + 1] if shared
                                else be1_sb[:, e, fc:fc + 1])
                        for t2 in range(TC2):
                            t = half * TC2 + t2
                            ps = psum_tile()
                            for kc in range(KD):
                                nc.tensor.matmul(
                                    ps[:], lhsT=w1f[:, kc, :],
                                    rhs=tb[:, kc, ts(t, TW)],
                                    start=(kc == 0), stop=(kc == KD - 1))
                            nc.scalar.activation(
                                zh[:, fc, ts(t2, TW)], ps[:], AF.Silu,
                                bias=bias)
                            if cbc is not None:
                                nc.vector.tensor_tensor(
                                    zh[:, fc, ts(t2, TW)],
                                    zh[:, fc, ts(t2, TW)],
                                    cbc[:, ts(t2, TW)], op=OP.mult)
                    if first:
                        # combine weights -> expert-major, deferred so
                        # expert 0\'s first matmuls don\'t wait on the
                        # top-k DVE chain; then scale its z after the fact
                        for i in range(NT):
                            tp = psum_tile()
                            nc.tensor.transpose(
                                tp[:E, :P], work[:, i, :], ident[:])
                            nc.vector.tensor_copy(
                                c_fm[:, i * P:(i + 1) * P], tp[:E, :P])
                        cbc = emit_cbc(0, 0)
                        for t2 in range(TC2):
                            for fc in range(FC):
                                nc.vector.tensor_tensor(
                                    zh[:, fc, ts(t2, TW)],
                                    zh[:, fc, ts(t2, TW)],
                                    cbc[:, ts(t2, TW)], op=OP.mult)
                    # mm2 (dc-major streamed weights)
                    for dc in range(KD):
                        w2d = pw.tile([P, FC, P], F32R, tag="w", name="w2d")
                        src_ap = (ws2_d[dc] if shared else we2_d[e, dc])
                        nc.sync.dma_start(
                            w2d[:], src_ap.rearrange("(fo p) m -> p fo m", p=P))
                        for t2 in range(TC2):
                            t = half * TC2 + t2
                            ps = psum_tile()
                            for fc in range(FC):
                                nc.tensor.matmul(
                                    ps[:], lhsT=w2d[:, fc, :],
                                    rhs=zh[:, fc, ts(t2, TW)],
                                    start=(fc == 0),
                                    stop=(fc == FC - 1 and not shared))
                            if shared:
                                nc.tensor.matmul(
                                    ps[:], lhsT=be2_sb[:, dc, :],
                                    rhs=c_fm[:, ts(t, TW)],
                                    start=False, stop=True)
                            stg = pstg.tile([P, TW], F32, tag="stg", name="stg")
                            nc.scalar.activation(stg[:], ps[:], AF.Copy)
                            nc.gpsimd.dma_start(
                                acc_h[half][:, dc, ts(t2, TW)], stg[:],
                                accum_op=(OP.bypass if e == 0 else OP.add))
                # this token half\'s partial is complete on this core
                nc.gpsimd.collective_compute(
                    "AllReduce",
                    OP.add,
                    replica_groups=[list(range(NCORES))],
                    ins=[acc_h[half][:].opt()],
                    outs=[red_h[half][:].opt()],
                )
        pmoec.release()

        # ---- phase 7: AllReduce; x2 = x1 + red + bs2sum; out projection ----
        with (
            tc.tile_pool(name="pxb", bufs=3) as pxb,
            tc.tile_pool(name="pwout", bufs=20) as pwout,
        ):
            x2 = pbig.tile([P, KD, T], F32R, tag="B")
            # two half-passes over tokens: the first half only needs the
            # first two all-reduced chunks, so its projection overlaps the
            # later all-reduces (and the engines stay in-order-clean)
            def load_wot(n):
                wot = []
                for kc in range(KD):
                    wt = pwout.tile([P, TW], F32R, tag="wo", name="wo")
                    nc.sync.dma_start(wt[:], wout_d[n, ts(kc, P), :])
                    wot.append(wt)
                return wot

            for half in range(2):
                prefetched = {}
                for t2 in range(TC // 2):
                    t = half * (TC // 2) + t2
                    for kc in range(KD):
                        xb = pxb.tile([P, TW], F32, tag="xb", name="xb")
                        nc.sync.dma_start(xb[:], x1_dram[:, kc, ts(t, TW)])
                        rb = pxb.tile([P, TW], F32, tag="rb", name="rb")
                        nc.sync.dma_start(rb[:], red_h[half][:, kc, ts(t2, TW)])
                        nc.vector.tensor_add(out=xb[:], in0=xb[:], in1=rb[:])
                        nc.vector.tensor_scalar_add(
                            x2[:, kc, ts(t, TW)], xb[:], bs2_sb[:, kc:kc + 1])
                    # slot the first weight chunks between the x2 loads so
                    # the projection isn't stuck behind a 16MB DMA burst
                    prefetched[t2] = load_wot(t2)
                for n in range(NCH):
                    wot = prefetched.get(n) or load_wot(n)
                    for m in range(half * NT // 2, (half + 1) * NT // 2):
                        ps = psum_tile()
                        for kc in range(KD):
                            nc.tensor.matmul(
                                ps[:], lhsT=x2[:, kc, ts(m, P)], rhs=wot[kc][:],
                                start=(kc == 0), stop=(kc == KD - 1))
                        stg = pstg.tile([P, TW], F32, tag="stg", name="stg")
                        nc.scalar.activation(stg[:], ps[:], AF.Copy)
                        nc.sync.dma_start(logits_d[ts(m, P), ts(n, TW)], stg[:])

        for _pool in (pdram, pstg, ppsum, pbig, pconst):
            _pool.release()

    nc.compile()
    return nc


def _get_nc():
    if "nc" not in _NC_CACHE:
        _NC_CACHE["nc"] = _build_nc()
    return _NC_CACHE["nc"]


def _prep_in_maps(inputs):
    inp = {k: np.asarray(v) for k, v in inputs.items()}
    f32 = np.float32

    ids = np.ascontiguousarray(inp["input_ids"].reshape(T, 1).astype(np.int32))
    emb = np.ascontiguousarray(inp["emb"].astype(f32))
    WoS = np.ascontiguousarray(
        inp["Wo"].astype(f32).reshape(H, R, D).sum(0).astype(f32))
    g2 = inp["g2"].astype(f32)
    Wrg = np.ascontiguousarray((g2[:, None] * inp["Wr"].astype(f32)).astype(f32))
    breff = (inp["br"].astype(f32)
             + inp["beta2"].astype(f32) @ inp["Wr"].astype(f32))
    breff = np.ascontiguousarray(breff.reshape(E, 1).astype(f32))

    common = {
        "ids": ids, "emb": emb,
        "ones128": np.ones((P, P), f32),
        "Wv": np.ascontiguousarray(inp["Wv"].astype(f32)),
        "WoS": WoS, "Wrg": Wrg, "breff": breff,
        "g1v": inp["g1"].astype(f32), "b1v": inp["beta1"].astype(f32),
        "g2v": g2, "b2v": inp["beta2"].astype(f32),
        "bs2S": np.ascontiguousarray(inp["bs2"].astype(f32).sum(0)),
    }

    We1 = inp["We1"].astype(f32)
    be1 = inp["be1"].astype(f32)
    We2 = inp["We2"].astype(f32)
    be2 = inp["be2"].astype(f32)
    Ws1 = inp["Ws1"].astype(f32)
    bs1 = inp["bs1"].astype(f32)
    Ws2 = inp["Ws2"].astype(f32)
    Wout = inp["Wout"].astype(f32)
    bout = inp["bout"].astype(f32)

    in_maps = []
    for c in range(NCORES):
        el = list(range(ELOC * c, ELOC * (c + 1)))
        s, q = divmod(c, NCORES // NS)
        isl = slice(q * ILOC, (q + 1) * ILOC)
        Sbc = np.zeros((E, ELOC * P), f32)
        for j, e in enumerate(el):
            Sbc[e, j * P:(j + 1) * P] = 1.0
        wout_pad = np.zeros((D, VPAD), f32)
        wout_pad[:, :VLOC] = Wout[:, VLOC * c:VLOC * (c + 1)]
        woutL = np.ascontiguousarray(
            wout_pad.reshape(D, NCH, TW).transpose(1, 0, 2))
        m = dict(common)
        be2P = np.zeros((E, D), f32)
        be2P[el] = be2[el]
        m.update({
            "We1L": np.ascontiguousarray(
                We1[el].reshape(ELOC, D, FC, P).transpose(0, 2, 1, 3)),
            "be1L": np.ascontiguousarray(be1[el]),
            "We2L": np.ascontiguousarray(
                We2[el].reshape(ELOC, F, KD, P).transpose(0, 2, 1, 3)),
            "be2P": be2P,
            "Ws1L": np.ascontiguousarray(
                Ws1[s][:, isl].reshape(D, FC, P).transpose(1, 0, 2)),
            "bs1L": np.ascontiguousarray(bs1[s][isl]),
            "Ws2L": np.ascontiguousarray(
                Ws2[s][isl, :].reshape(ILOC, KD, P).transpose(1, 0, 2)),
            "Sbc": Sbc,
            "WoutL": woutL,
        })
        in_maps.append(m)
    return in_maps


def kernel(**inputs):
    in_maps = _prep_in_maps(inputs)
    nc = _get_nc()
    r = run_bass_kernel_spmd(nc, in_maps, list(range(NCORES)))
    logits = np.concatenate(
        [r.results[c]["logits"][:, :VLOC] for c in range(NCORES)], axis=1)
    bout = np.asarray(inputs["bout"]).astype(np.float32)
    if np.any(bout):
        logits = logits + bout[None, :]
    return np.ascontiguousarray(logits.reshape(B, S, V).astype(np.float32))


if __name__ == "__main__":
    _build_nc()
    print("build + compile OK")


# revision 31
# speedup vs baseline: 1.1536x; 1.0207x over previous
"""Trainium2 Bass kernel for nn_BeyazKusAIEnhanced (moe_routing).

Model (T=2048 tokens, D=1024):
  x = emb[ids]
  h = LN1(x); attention collapses exactly to: ao = (h @ Wv) @ WoSum
    (softmax over a size-1 axis is exactly 1, so out = tile(v, 16 heads)
     and out @ Wo == v @ WoSum with WoSum[r,:] = sum_h Wo[h*64+r, :])
  x1 = x + ao
  t = LN2(x1); router probs = softmax(t @ Wr + br); top-8 -> combine [T,32]
  moe = sum_e combine[:,e] * (silu(t@We1[e]+be1[e]) @ We2[e] + be2[e])
  shared = sum_s silu(t@Ws1[s]+bs1[s]) @ Ws2[s] + bs2[s]
  out = (x1 + moe + shared) @ Wout + bout        [T, 32000]

Sharding (8 cores):
  - front part (gather/LN/attn/router) replicated on all cores
  - routed experts: 4 per core (dense compute; combine weights of
    non-selected experts are exactly 0, so dense == sparse w/ weights)
  - shared experts: inter dim (2*4096 = 8192) split 1024 per core;
    bs2 biases summed on host and added post-allreduce on every core
  - partial (moe+shared) accumulated in DRAM via accum-DMA, AllReduce'd
    across cores; x2 = x1 + reduced + bs2sum
  - output projection vocab-split: 4000 cols/core (padded to 4096)

Layout: activations feature-major [128 part, 8 kchunk, 2048 tok] in SBUF;
matmuls fp32r (full PE rate at moving free dim >= 256, ~1e-4 rel err).
LN stats via all-ones [128,128] matmul (partition-broadcast sums, no
explicit broadcast step); per-core expert selection via one-hot inputs.
Router runs in plain fp32 from x1 with LN folded (host folds g2 into Wr
and beta2@Wr into br) so top-8 selection is as close to the f32
reference as possible.
"""

import numpy as np

import concourse.bass as bass
import concourse.mybir as mybir
import concourse.tile as tile
from concourse import bacc
from concourse.bass import ts
from concourse.bass_utils import run_bass_kernel_spmd
from concourse.masks import make_identity

P = 128
B, S = 2, 1024
T = 2048          # tokens
D = 1024          # model dim
KD = D // P       # 8 k-chunks
H = 16            # heads
R = 64            # kv rank / head dim
E = 32            # routed experts
ELOC = 4          # experts per core
F = 1024          # moe inter dim
FC = F // P       # 8
NS = 2            # shared experts
ILOC = 1024       # shared inter slice per core
V = 32000
VLOC = 4000       # real vocab cols per core
VPAD = 4096       # padded to 8 x 512
NCH = VPAD // 512
TC = 4            # token chunks
TW = 512          # token chunk width
NT = T // P       # 16 token tiles
EPS = 1e-5
NCORES = 8

F32 = mybir.dt.float32
F32R = mybir.dt.float32r
I32 = mybir.dt.int32
AF = mybir.ActivationFunctionType
OP = mybir.AluOpType
AX = mybir.AxisListType

_NC_CACHE = {}


def _build_nc():
    nc = bacc.Bacc(None)

    ids_d = nc.declare_dram_parameter("ids", [T, 1], I32, isOutput=False)
    emb_d = nc.declare_dram_parameter("emb", [V, D], F32, isOutput=False)
    ones_d = nc.declare_dram_parameter("ones128", [P, P], F32R, isOutput=False)
    wv_d = nc.declare_dram_parameter("Wv", [D, R], F32, isOutput=False)
    wos_d = nc.declare_dram_parameter("WoS", [R, D], F32, isOutput=False)
    wrg_d = nc.declare_dram_parameter("Wrg", [D, E], F32, isOutput=False)
    breff_d = nc.declare_dram_parameter("breff", [E, 1], F32, isOutput=False)
    g1_d = nc.declare_dram_parameter("g1v", [D], F32, isOutput=False)
    b1_d = nc.declare_dram_parameter("b1v", [D], F32, isOutput=False)
    g2_d = nc.declare_dram_parameter("g2v", [D], F32, isOutput=False)
    b2_d = nc.declare_dram_parameter("b2v", [D], F32, isOutput=False)
    we1_d = nc.declare_dram_parameter("We1L", [ELOC, FC, D, P], F32R,
                                      isOutput=False)
    be1_d = nc.declare_dram_parameter("be1L", [ELOC, F], F32, isOutput=False)
    we2_d = nc.declare_dram_parameter("We2L", [ELOC, KD, F, P], F32R,
                                      isOutput=False)
    be2_d = nc.declare_dram_parameter("be2P", [E, D], F32R, isOutput=False)
    ws1_d = nc.declare_dram_parameter("Ws1L", [FC, D, P], F32R, isOutput=False)
    bs1_d = nc.declare_dram_parameter("bs1L", [ILOC], F32, isOutput=False)
    ws2_d = nc.declare_dram_parameter("Ws2L", [KD, ILOC, P], F32R, isOutput=False)
    bs2_d = nc.declare_dram_parameter("bs2S", [D], F32, isOutput=False)
    sbc_d = nc.declare_dram_parameter("Sbc", [E, ELOC * P], F32R, isOutput=False)
    wout_d = nc.declare_dram_parameter("WoutL", [NCH, D, TW], F32R, isOutput=False)
    logits_d = nc.declare_dram_parameter("logits", [T, VPAD], F32, isOutput=True)

    with tile.TileContext(nc) as tc:
        pconst = tc.alloc_tile_pool(name="pconst", bufs=1)
        pbig = tc.alloc_tile_pool(name="pbig", bufs=1)
        ppsum = tc.alloc_tile_pool(name="ppsum", bufs=7, space="PSUM")
        pstg = tc.alloc_tile_pool(name="pstg", bufs=4)
        pdram = tc.alloc_tile_pool(name="pdram", bufs=1, space="DRAM")

        def psum_tile():
            return ppsum.tile([P, TW], F32, tag="ps", name="ps", space="PSUM")

        # ---- small constants (~8.6 KB/partition) ----
        ident = pconst.tile([P, P], F32)
        make_identity(nc, ident[:])
        ones_sb = pconst.tile([P, P], F32R)
        nc.sync.dma_start(ones_sb[:], ones_d[:, :])
        wv_sb = pconst.tile([P, KD, R], F32)
        nc.sync.dma_start(wv_sb[:], wv_d.rearrange("(ko p) r -> p ko r", p=P))
        wos_sb = pconst.tile([R, KD, P], F32)
        nc.sync.dma_start(wos_sb[:], wos_d.rearrange("r (ko p) -> r ko p", p=P))
        wrg_sb = pconst.tile([P, KD, E], F32)
        nc.sync.dma_start(wrg_sb[:], wrg_d.rearrange("(ko p) e -> p ko e", p=P))
        breff_sb = pconst.tile([E, 1], F32)
        nc.sync.dma_start(breff_sb[:], breff_d[:, :])
        g1_sb = pconst.tile([P, KD], F32)
        nc.sync.dma_start(g1_sb[:], g1_d.rearrange("(ko p) -> p ko", p=P))
        b1_sb = pconst.tile([P, KD], F32)
        nc.sync.dma_start(b1_sb[:], b1_d.rearrange("(ko p) -> p ko", p=P))
        g2_sb = pconst.tile([P, KD], F32)
        nc.sync.dma_start(g2_sb[:], g2_d.rearrange("(ko p) -> p ko", p=P))
        b2_sb = pconst.tile([P, KD], F32)
        nc.sync.dma_start(b2_sb[:], b2_d.rearrange("(ko p) -> p ko", p=P))
        be1_sb = pconst.tile([P, ELOC, FC], F32)
        nc.sync.dma_start(be1_sb[:], be1_d.rearrange("e (ko p) -> p e ko", p=P))
        bs1_sb = pconst.tile([P, FC], F32)
        nc.sync.dma_start(bs1_sb[:], bs1_d.rearrange("(ko p) -> p ko", p=P))
        bs2_sb = pconst.tile([P, KD], F32)
        nc.sync.dma_start(bs2_sb[:], bs2_d.rearrange("(ko p) -> p ko", p=P))
        eps_sb = pconst.tile([P, 1], F32)
        nc.gpsimd.memset(eps_sb[:], EPS)

        # DRAM scratch
        x1_dram = pdram.tile([P, KD, T], F32, tag="x1d")
        acc_h = [pdram.tile([P, KD, T // 2], F32, tag=f"acc{h}", name=f"acc{h}")
                 for h in range(2)]
        red_h = [pdram.tile([P, KD, T // 2], F32, tag=f"red{h}", name=f"red{h}",
                            addr_space="Shared")
                 for h in range(2)]

        # combine-weight tiles + MoE selection constants (outlive front pools)
        pmoec = tc.alloc_tile_pool(name="pmoec", bufs=1)
        c_fm = pmoec.tile([E, T], F32R, tag="cfm")
        sbc_sb = pmoec.tile([E, ELOC * P], F32R, tag="sbc")
        nc.sync.dma_start(sbc_sb[:], sbc_d[:, :])
        be2_sb = pmoec.tile([E, KD, P], F32R, tag="be2")
        nc.sync.dma_start(be2_sb[:], be2_d.rearrange("e (ko p) -> e ko p", p=P))

        pbigA = tc.alloc_tile_pool(name="pbigA", bufs=1)
        xa = pbigA.tile([P, KD, T], F32, tag="A")  # x, then x1 (in place)
        hb = pbig.tile([P, KD, T], F32, tag="B")  # h (fp32, feeds attention)

        # ---- phases 1-5 (gather, LN1, attention, LN2+router fused) ----
        with (
            tc.tile_pool(name="pfC", bufs=1) as pfC,
            tc.tile_pool(name="pfM", bufs=2) as pfM,
        ):
            pfA = tc.alloc_tile_pool(name="pfA", bufs=2)
            pfB = tc.alloc_tile_pool(name="pfB", bufs=2 * TC)

            def ln_stats(src, t):
                """LN stats for token chunk t -> (mu, rstd) tiles [P, TW]
                (every partition holds the same per-token row)."""
                ps_mu = psum_tile()
                ps_sq = psum_tile()
                for kc in range(KD):
                    xr = pfA.tile([P, TW], F32R, tag="sq", name="xr")
                    nc.vector.tensor_copy(xr[:], src[:, kc, ts(t, TW)])
                    nc.tensor.matmul(
                        ps_mu[:], lhsT=ones_sb[:], rhs=xr[:],
                        start=(kc == 0), stop=(kc == KD - 1))
                    sq = pfA.tile([P, TW], F32R, tag="sq", name="sq")
                    nc.scalar.activation(sq[:], src[:, kc, ts(t, TW)], AF.Square)
                    nc.tensor.matmul(
                        ps_sq[:], lhsT=ones_sb[:], rhs=sq[:],
                        start=(kc == 0), stop=(kc == KD - 1))
                mu = pfB.tile([P, TW], F32, tag="bc", name="mu")
                nc.vector.tensor_scalar_mul(mu[:], ps_mu[:], 1.0 / D)
                msq = pfA.tile([P, TW], F32, tag="lntmp", name="msq")
                nc.vector.tensor_scalar_mul(msq[:], ps_sq[:], 1.0 / D)
                mu2 = pfA.tile([P, TW], F32, tag="lntmp", name="mu2")
                nc.vector.tensor_mul(out=mu2[:], in0=mu[:], in1=mu[:])
                nc.vector.tensor_tensor(msq[:], msq[:], mu2[:], op=OP.subtract)
                nc.scalar.activation(msq[:], msq[:], AF.Sqrt, bias=eps_sb[:, 0:1])
                rstd = pfB.tile([P, TW], F32, tag="bc", name="rstd")
                nc.vector.reciprocal(rstd[:], msq[:])
                return mu, rstd

            def ln_apply(src, dst, t, mu, rstd, g_sb, b_sb):
                for kc in range(KD):
                    eng = nc.vector if kc % 2 == 0 else nc.gpsimd
                    eng.tensor_tensor(
                        dst[:, kc, ts(t, TW)], src[:, kc, ts(t, TW)], mu[:],
                        op=OP.subtract)
                    eng.tensor_tensor(
                        dst[:, kc, ts(t, TW)], dst[:, kc, ts(t, TW)], rstd[:],
                        op=OP.mult)
                    eng.tensor_scalar(
                        dst[:, kc, ts(t, TW)], dst[:, kc, ts(t, TW)],
                        g_sb[:, kc:kc + 1], b_sb[:, kc:kc + 1],
                        op0=OP.mult, op1=OP.add)

            # embedding gather + PE transpose to feature-major, with each
            # token chunk's LN1 stats emitted as soon as its tiles land
            st1 = []
            with (
                tc.tile_pool(name="pgather", bufs=2) as pgather,
                tc.tile_pool(name="pidx", bufs=NT) as pidx,
            ):
                idxs = []
                for i in range(NT):
                    idx_sb = pidx.tile([P, 1], I32, tag="idx", name="idx")
                    nc.sync.dma_start(idx_sb[:], ids_d[i * P:(i + 1) * P, :])
                    idxs.append(idx_sb)
                for i in range(NT):
                    gx = pgather.tile([P, D], F32, tag="gx", name="gx")
                    nc.gpsimd.indirect_dma_start(
                        out=gx[:],
                        out_offset=None,
                        in_=emb_d[:, :],
                        in_offset=bass.IndirectOffsetOnAxis(
                            ap=idxs[i][:, :1], axis=0),
                    )
                    for kc in range(KD):
                        tp = psum_tile()
                        nc.tensor.transpose(tp[:, :P], gx[:, ts(kc, P)], ident[:])
                        nc.vector.tensor_copy(
                            xa[:, kc, i * P:(i + 1) * P], tp[:, :P])
                    if i % (NT // TC) == NT // TC - 1:
                        st1.append(ln_stats(xa, i // (NT // TC)))

            # LN1 -> h
            for t in range(TC):
                ln_apply(xa, hb, t, st1[t][0], st1[t][1], g1_sb, b1_sb)

            # v = h @ Wv  [R, T]
            v_sb = pfC.tile([R, T], F32, tag="v")
            for t in range(TC):
                ps = psum_tile()
                for kc in range(KD):
                    nc.tensor.matmul(
                        ps[:R, :], lhsT=wv_sb[:, kc, :], rhs=hb[:, kc, ts(t, TW)],
                        start=(kc == 0), stop=(kc == KD - 1))
                nc.vector.tensor_copy(v_sb[:, ts(t, TW)], ps[:R, :])
            # x1 = x + v @ WoSum  (in place into xa)
            for dc in range(KD):
                for t in range(TC):
                    ps = psum_tile()
                    nc.tensor.matmul(
                        ps[:], lhsT=wos_sb[:, dc, :], rhs=v_sb[:, ts(t, TW)],
                        start=True, stop=True)
                    nc.vector.tensor_add(
                        out=xa[:, dc, ts(t, TW)], in0=xa[:, dc, ts(t, TW)],
                        in1=ps[:])
            nc.sync.dma_start(x1_dram[:], xa[:])

            # LN2 -> t (f32r, into slot B), fused with fp32 router matmul
            tb = pbig.tile([P, KD, T], F32R, tag="B", name="tb")
            r_fm = pfC.tile([E, T], F32, tag="v", name="r_fm")
            st2 = [ln_stats(xa, t) for t in range(TC)]
            for t in range(TC):
                mu, rstd = st2[t]
                ln_apply(xa, tb, t, mu, rstd, g2_sb, b2_sb)
                ps = psum_tile()
                for kc in range(KD):
                    rt = pfA.tile([P, TW], F32, tag="lntmp", name="rt")
                    nc.vector.tensor_tensor(
                        rt[:], xa[:, kc, ts(t, TW)], mu[:],
                        op=OP.subtract)
                    nc.tensor.matmul(
                        ps[:E, :], lhsT=wrg_sb[:, kc, :], rhs=rt[:],
                        start=(kc == 0), stop=(kc == KD - 1))
                nc.vector.tensor_tensor(
                    r_fm[:, ts(t, TW)], ps[:E, :], rstd[:E, :], op=OP.mult)
                nc.vector.tensor_scalar_add(
                    r_fm[:, ts(t, TW)], r_fm[:, ts(t, TW)], breff_sb[:E, 0:1])

            pfB.release()
            pfA.release()

            # softmax + top-8 in token-major
            r_tm = pfC.tile([P, NT, E], F32, tag="rtm")
            for i in range(NT):
                tp = psum_tile()
                nc.tensor.transpose(
                    tp[:, :E], r_fm[:, i * P:(i + 1) * P], ident[:E, :E])
                nc.vector.tensor_copy(r_tm[:, i, :], tp[:, :E])
            m_sb = pfM.tile([P, NT], F32, tag="m", name="m1")
            nc.vector.reduce_max(m_sb[:, :, None], r_tm[:], axis=AX.X)
            nc.vector.tensor_tensor(
                r_tm[:], r_tm[:], m_sb[:, :, None].to_broadcast([P, NT, E]),
                op=OP.subtract)
            nc.scalar.activation(r_tm[:], r_tm[:], AF.Exp)
            s_sb = pfM.tile([P, NT], F32, tag="m", name="m2")
            nc.vector.reduce_sum(s_sb[:, :, None], r_tm[:], axis=AX.X)
            rs_sb = pfM.tile([P, NT], F32, tag="m", name="m3")
            nc.vector.reciprocal(rs_sb[:], s_sb[:])
            nc.vector.tensor_tensor(
                r_tm[:], r_tm[:], rs_sb[:, :, None].to_broadcast([P, NT, E]),
                op=OP.mult)
            work = pmoec.tile([P, NT, E], F32, tag="work")
            msk = pfC.tile([P, NT, E], F32, tag="msk")
            nc.vector.tensor_copy(work[:], r_tm[:])
            thr = pfM.tile([P, NT], F32, tag="m", name="m4")
            for it in range(8):
                nc.vector.reduce_max(thr[:, :, None], work[:], axis=AX.X)
                if it < 7:
                    nc.vector.tensor_tensor(
                        msk[:], work[:], thr[:, :, None].to_broadcast([P, NT, E]),
                        op=OP.is_lt)
                    nc.vector.tensor_tensor(work[:], work[:], msk[:], op=OP.mult)
            nc.vector.tensor_tensor(
                msk[:], r_tm[:], thr[:, :, None].to_broadcast([P, NT, E]),
                op=OP.is_ge)
            nc.vector.tensor_tensor(work[:], r_tm[:], msk[:], op=OP.mult)
            wsum = pfM.tile([P, NT], F32, tag="m", name="m5")
            nc.vector.reduce_sum(wsum[:, :, None], work[:], axis=AX.X)
            rws = pfM.tile([P, NT], F32, tag="m", name="m6")
            nc.vector.reciprocal(rws[:], wsum[:])
            nc.vector.tensor_tensor(
                work[:], work[:], rws[:, :, None].to_broadcast([P, NT, E]),
                op=OP.mult)

        pbigA.release()

        T2 = T // 2

        # ---- phase 6: MoE (4 routed dense + shared slice) ----
        # Token-half-outer over the whole expert set: the first half\'s
        # partial sum is complete mid-phase, so its all-reduce runs under
        # the second half\'s compute and the output projection starts at
        # phase end with no collective exposure.
        with (
            tc.tile_pool(name="pw", bufs=8) as pw,
            tc.tile_pool(name="pcbc", bufs=2) as pcbc,
            tc.tile_pool(name="pz", bufs=2) as pz,
        ):
            TC2 = TC // 2
            sbc_sb = pw.tile([E, ELOC * P], F32R, tag="sbc", name="sbc", bufs=1)
            nc.sync.dma_start(sbc_sb[:], sbc_d[:, :])
            be2_sb = pw.tile([E, KD, P], F32R, tag="be2", name="be2", bufs=1)
            nc.sync.dma_start(
                be2_sb[:], be2_d.rearrange("e (ko p) -> e ko p", p=P))

            def emit_cbc(e, half):
                cbc = pcbc.tile([P, T2], F32, tag="cbc", name="cbc")
                for t2 in range(TC2):
                    t = half * TC2 + t2
                    ps = psum_tile()
                    nc.tensor.matmul(
                        ps[:], lhsT=sbc_sb[:, ts(e, P)],
                        rhs=c_fm[:, ts(t, TW)], start=True, stop=True)
                    nc.vector.tensor_copy(cbc[:, ts(t2, TW)], ps[:])
                return cbc

            for half in range(2):
                for e in range(ELOC + 1):
                    shared = e == ELOC
                    first = e == 0 and half == 0
                    cbc = (None if shared or first else emit_cbc(e, half))
                    zh = pz.tile([P, FC, T2], F32R, tag="z", name="zh")
                    for fc in range(FC):
                        w1f = pw.tile([P, KD, P], F32R, tag="w", name="w1f")
                        src_ap = (ws1_d[fc] if shared else we1_d[e, fc])
                        nc.sync.dma_start(
                            w1f[:], src_ap.rearrange("(ko p) m -> p ko m", p=P))
                        bias = (bs1_sb[:, fc:fc + 1] if shared
                                else be1_sb[:, e, fc:fc + 1])
                        for t2 in range(TC2):
                            t = half * TC2 + t2
                            ps = psum_tile()
                            for kc in range(KD):
                                nc.tensor.matmul(
                                    ps[:], lhsT=w1f[:, kc, :],
                                    rhs=tb[:, kc, ts(t, TW)],
                                    start=(kc == 0), stop=(kc == KD - 1))
                            nc.scalar.activation(
                                zh[:, fc, ts(t2, TW)], ps[:], AF.Silu,
                                bias=bias)
                            if cbc is not None:
                                nc.vector.tensor_tensor(
                                    zh[:, fc, ts(t2, TW)],
                                    zh[:, fc, ts(t2, TW)],
                                    cbc[:, ts(t2, TW)], op=OP.mult)
                    if first:
                        # combine weights -> expert-major, deferred so
                        # expert 0\'s first matmuls don\'t wait on the
                        # top-k DVE chain; then scale its z after the fact
                        for i in range(NT):
                            tp = psum_tile()
                            nc.tensor.transpose(
                                tp[:E, :P], work[:, i, :], ident[:])
                            nc.vector.tensor_copy(
                                c_fm[:, i * P:(i + 1) * P], tp[:E, :P])
                        cbc = emit_cbc(0, 0)
                        for t2 in range(TC2):
                            for fc in range(FC):
                                nc.vector.tensor_tensor(
                                    zh[:, fc, ts(t2, TW)],
                                    zh[:, fc, ts(t2, TW)],
                                    cbc[:, ts(t2, TW)], op=OP.mult)
                    # mm2 (dc-major streamed weights)
                    for dc in range(KD):
                        w2d = pw.tile([P, FC, P], F32R, tag="w", name="w2d")
                        src_ap = (ws2_d[dc] if shared else we2_d[e, dc])
                        nc.sync.dma_start(
                            w2d[:], src_ap.rearrange("(fo p) m -> p fo m", p=P))
                        for t2 in range(TC2):
                            t = half * TC2 + t2
                            ps = psum_tile()
                            for fc in range(FC):
                                nc.tensor.matmul(
                                    ps[:], lhsT=w2d[:, fc, :],
                                    rhs=zh[:, fc, ts(t2, TW)],
                                    start=(fc == 0),
                                    stop=(fc == FC - 1 and not shared))
                            if shared:
                                nc.tensor.matmul(
                                    ps[:], lhsT=be2_sb[:, dc, :],
                                    rhs=c_fm[:, ts(t, TW)],
                                    start=False, stop=True)
                            stg = pstg.tile([P, TW], F32, tag="stg", name="stg")
                            nc.scalar.activation(stg[:], ps[:], AF.Copy)
                            nc.gpsimd.dma_start(
                                acc_h[half][:, dc, ts(t2, TW)], stg[:],
                                accum_op=(OP.bypass if e == 0 else OP.add))
                # this token half\'s partial is complete on this core
                nc.gpsimd.collective_compute(
                    "AllReduce",
                    OP.add,
                    replica_groups=[list(range(NCORES))],
                    ins=[acc_h[half][:].opt()],
                    outs=[red_h[half][:].opt()],
                )
        pmoec.release()

        # ---- phase 7: AllReduce; x2 = x1 + red + bs2sum; out projection ----
        with (
            tc.tile_pool(name="pxb", bufs=3) as pxb,
            tc.tile_pool(name="pwout", bufs=20) as pwout,
        ):
            x2 = pbig.tile([P, KD, T], F32R, tag="B")
            # two half-passes over tokens: the first half only needs the
            # first two all-reduced chunks, so its projection overlaps the
            # later all-reduces (and the engines stay in-order-clean)
            def load_wot(n):
                wot = []
                for kc in range(KD):
                    wt = pwout.tile([P, TW], F32R, tag="wo", name="wo")
                    nc.sync.dma_start(wt[:], wout_d[n, ts(kc, P), :])
                    wot.append(wt)
                return wot

            for half in range(2):
                prefetched = {}
                for t2 in range(TC // 2):
                    t = half * (TC // 2) + t2
                    for kc in range(KD):
                        xb = pxb.tile([P, TW], F32, tag="xb", name="xb")
                        nc.sync.dma_start(xb[:], x1_dram[:, kc, ts(t, TW)])
                        rb = pxb.tile([P, TW], F32, tag="rb", name="rb")
                        nc.sync.dma_start(rb[:], red_h[half][:, kc, ts(t2, TW)])
                        nc.vector.tensor_add(out=xb[:], in0=xb[:], in1=rb[:])
                        nc.vector.tensor_scalar_add(
                            x2[:, kc, ts(t, TW)], xb[:], bs2_sb[:, kc:kc + 1])
                    # slot the first weight chunks between the x2 loads so
                    # the projection isn't stuck behind a 16MB DMA burst
                    prefetched[t2] = load_wot(t2)
                for n in range(NCH):
                    wot = prefetched.get(n) or load_wot(n)
                    for m in range(half * NT // 2, (half + 1) * NT // 2):
                        ps = psum_tile()
                        for kc in range(KD):
                            nc.tensor.matmul(
                                ps[:], lhsT=x2[:, kc, ts(m, P)], rhs=wot[kc][:],
                                start=(kc == 0), stop=(kc == KD - 1))
                        stg = pstg.tile([P, TW], F32, tag="stg", name="stg")
                        nc.scalar.activation(stg[:], ps[:], AF.Copy)
                        nc.sync.dma_start(logits_d[ts(m, P), ts(n, TW)], stg[:])

        for _pool in (pdram, pstg, ppsum, pbig, pconst):
            _pool.release()

    nc.compile()
    return nc


def _get_nc():
    if "nc" not in _NC_CACHE:
        _NC_CACHE["nc"] = _build_nc()
    return _NC_CACHE["nc"]


def _prep_in_maps(inputs):
    inp = {k: np.asarray(v) for k, v in inputs.items()}
    f32 = np.float32

    ids = np.ascontiguousarray(inp["input_ids"].reshape(T, 1).astype(np.int32))
    emb = np.ascontiguousarray(inp["emb"].astype(f32))
    WoS = np.ascontiguousarray(
        inp["Wo"].astype(f32).reshape(H, R, D).sum(0).astype(f32))
    g2 = inp["g2"].astype(f32)
    Wrg = np.ascontiguousarray((g2[:, None] * inp["Wr"].astype(f32)).astype(f32))
    breff = (inp["br"].astype(f32)
             + inp["beta2"].astype(f32) @ inp["Wr"].astype(f32))
    breff = np.ascontiguousarray(breff.reshape(E, 1).astype(f32))

    common = {
        "ids": ids, "emb": emb,
        "ones128": np.ones((P, P), f32),
        "Wv": np.ascontiguousarray(inp["Wv"].astype(f32)),
        "WoS": WoS, "Wrg": Wrg, "breff": breff,
        "g1v": inp["g1"].astype(f32), "b1v": inp["beta1"].astype(f32),
        "g2v": g2, "b2v": inp["beta2"].astype(f32),
        "bs2S": np.ascontiguousarray(inp["bs2"].astype(f32).sum(0)),
    }

    We1 = inp["We1"].astype(f32)
    be1 = inp["be1"].astype(f32)
    We2 = inp["We2"].astype(f32)
    be2 = inp["be2"].astype(f32)
    Ws1 = inp["Ws1"].astype(f32)
    bs1 = inp["bs1"].astype(f32)
    Ws2 = inp["Ws2"].astype(f32)
    Wout = inp["Wout"].astype(f32)
    bout = inp["bout"].astype(f32)

    in_maps = []
    for c in range(NCORES):
        el = list(range(ELOC * c, ELOC * (c + 1)))
        s, q = divmod(c, NCORES // NS)
        isl = slice(q * ILOC, (q + 1) * ILOC)
        Sbc = np.zeros((E, ELOC * P), f32)
        for j, e in enumerate(el):
            Sbc[e, j * P:(j + 1) * P] = 1.0
        wout_pad = np.zeros((D, VPAD), f32)
        wout_pad[:, :VLOC] = Wout[:, VLOC * c:VLOC * (c + 1)]
        woutL = np.ascontiguousarray(
            wout_pad.reshape(D, NCH, TW).transpose(1, 0, 2))
        m = dict(common)
        be2P = np.zeros((E, D), f32)
        be2P[el] = be2[el]
        m.update({
            "We1L": np.ascontiguousarray(
                We1[el].reshape(ELOC, D, FC, P).transpose(0, 2, 1, 3)),
            "be1L": np.ascontiguousarray(be1[el]),
            "We2L": np.ascontiguousarray(
                We2[el].reshape(ELOC, F, KD, P).transpose(0, 2, 1, 3)),
            "be2P": be2P,
            "Ws1L": np.ascontiguousarray(
                Ws1[s][:, isl].reshape(D, FC, P).transpose(1, 0, 2)),
            "bs1L": np.ascontiguousarray(bs1[s][isl]),
            "Ws2L": np.ascontiguousarray(
                Ws2[s][isl, :].reshape(ILOC, KD, P).transpose(1, 0, 2)),
            "Sbc": Sbc,
            "WoutL": woutL,
        })
        in_maps.append(m)
    return in_maps


def kernel(**inputs):
    in_maps = _prep_in_maps(inputs)
    nc = _get_nc()
    r = run_bass_kernel_spmd(nc, in_maps, list(range(NCORES)))
    logits = np.concatenate(
        [r.results[c]["logits"][:, :VLOC] for c in range(NCORES)], axis=1)
    bout = np.asarray(inputs["bout"]).astype(np.float32)
    if np.any(bout):
        logits = logits + bout[None, :]
    return np.ascontiguousarray(logits.reshape(B, S, V).astype(np.float32))


if __name__ == "__main__":
    _build_nc()
    print("build + compile OK")
